# revision 12
# baseline (speedup 1.0000x reference)
"""Trainium2 Bass kernel for nn_ChartQualityEvaluator.

Data parallel: 32 samples -> 8 cores x 4 samples. Feature-major activations
[128 part, 2 blocks, 4*500 cols]. Matmuls and bulk activations in bf16
(fp32 PSUM accumulation); stats/sinusoid paths stay fp32. Unsafe softmax,
LN via ones-matmul stats + PE broadcast, event scatter via onehot matmul.
Host precomputes index-like preprocessing with f32-exact semantics.
"""
import math
import sys

import numpy as np
import ml_dtypes

_TRN = "/opt/trn_rl_repo"
if _TRN not in sys.path:
    sys.path.insert(0, _TRN)

BF16 = ml_dtypes.bfloat16

D = 256
H = 8
NLAYERS = 6
HALF = 128
S = 500
NEV = 256
NCORES = 8
NS = 4  # samples per core
B = 32
EPS = 1e-5
INV2PI = float(np.float32(1.0 / (2.0 * math.pi)))
TWOPI = 2.0 * math.pi
SC32 = float(np.float32(1.0 / math.sqrt(32.0)))


def _host_prep(inp):
    f = np.float32
    out = {}

    def t2(v):  # [256] -> [128,2]
        return np.ascontiguousarray(np.asarray(v).reshape(2, 128).T.astype(f))

    def b(a):  # -> bf16
        return np.ascontiguousarray(np.asarray(a).astype(np.float32).astype(BF16))

    out["wmelT"] = b(np.asarray(inp["mel_W"]).T)
    out["melb"] = np.ascontiguousarray(np.asarray(inp["mel_b"]).reshape(16, 1).astype(f))
    out["w1t"] = b(np.asarray(inp["conv1_w"]).transpose(1, 2, 0))
    out["c1b"] = np.ascontiguousarray(np.asarray(inp["conv1_b"]).reshape(128, 1).astype(f))
    out["gng"] = np.ascontiguousarray(np.asarray(inp["gn_g"]).reshape(128, 1).astype(f))
    out["gnb"] = np.ascontiguousarray(np.asarray(inp["gn_b"]).reshape(128, 1).astype(f))
    out["w2t"] = b(np.asarray(inp["conv2_w"]).transpose(1, 2, 0))
    out["c2b"] = t2(inp["conv2_b"])
    out["cng"] = t2(inp["cn_g"])

    freq = np.exp(np.arange(HALF, dtype=f) * f(-math.log(10000.0) / (HALF - 1)))
    e32 = (np.arange(S, dtype=f)[None, :] * freq[:, None]).astype(f)
    e64 = e32.astype(np.float64)
    pos_fm = np.concatenate([np.sin(e64), np.cos(e64)], axis=0)  # [256,500]
    out["posT"] = np.ascontiguousarray(
        pos_fm.reshape(2, 128, S).transpose(1, 0, 2).astype(f))
    out["freqv"] = np.ascontiguousarray(freq.reshape(128, 1))

    out["epW1T"] = b(np.asarray(inp["ep_W1"]).T.reshape(6, 128, 256).transpose(1, 0, 2))
    out["epb1"] = t2(inp["ep_b1"])
    out["epW2T"] = b(np.asarray(inp["ep_W2"]).T.reshape(2, 128, 256).transpose(1, 0, 2))
    out["epb2row"] = np.ascontiguousarray(
        np.tile(np.asarray(inp["ep_b2"]).astype(f)[None, :], (128, 1)))

    def wT(w, kc, m):  # w [m, k] -> [128, kc, m] bf16
        return b(np.asarray(w).T.reshape(kc, 128, m).transpose(1, 0, 2))

    out["wqkvT"] = np.stack([wT(inp["tl_Wqkv"][i], 2, 768) for i in range(NLAYERS)])
    out["bqkv"] = np.stack([np.ascontiguousarray(
        np.asarray(inp["tl_bqkv"][i]).reshape(6, 128).T.astype(f))
        for i in range(NLAYERS)])
    out["woT"] = np.stack([wT(inp["tl_Wo"][i], 2, 256) for i in range(NLAYERS)])
    out["bov"] = np.stack([t2(inp["tl_bo"][i]) for i in range(NLAYERS)])
    out["ln1g"] = np.stack([t2(inp["tl_ln1g"][i]) for i in range(NLAYERS)])
    out["ln1b"] = np.stack([t2(inp["tl_ln1b"][i]) for i in range(NLAYERS)])
    out["ln2g"] = np.stack([t2(inp["tl_ln2g"][i]) for i in range(NLAYERS)])
    out["ln2b"] = np.stack([t2(inp["tl_ln2b"][i]) for i in range(NLAYERS)])
    out["w1TT"] = np.stack([wT(inp["tl_W1"][i], 2, 1024) for i in range(NLAYERS)])
    out["b1v"] = np.stack([np.ascontiguousarray(
        np.asarray(inp["tl_b1"][i]).reshape(8, 128).T.astype(f))
        for i in range(NLAYERS)])
    out["w2TT"] = np.stack([wT(inp["tl_W2"][i], 8, 256) for i in range(NLAYERS)])
    out["b2v"] = np.stack([t2(inp["tl_b2"][i]) for i in range(NLAYERS)])

    out["poolq"] = t2(inp["pool_q"])
    oz = np.zeros((128, 4, 4), np.float32)
    for c4 in range(4):
        oz[:, c4, c4] = 1.0
    out["onesZc"] = b(oz)
    e4 = np.zeros((4, 128), np.float32)
    for c4 in range(4):
        e4[c4, 32 * c4:32 * c4 + 32] = 1.0
    out["e4m"] = b(e4)
    c1 = (np.asarray(inp["oh_W"])[0] * np.asarray(inp["on_g"])).astype(f)
    out["c1v"] = t2(c1)
    out["sc1"] = float(c1.astype(np.float64).sum())
    out["c2s"] = float((np.asarray(inp["oh_W"])[0].astype(np.float64)
                        * np.asarray(inp["on_b"]).astype(np.float64)).sum()
                       + float(np.asarray(inp["oh_b"])[0]))

    events = np.asarray(inp["events"]).astype(np.int64)
    mask = np.asarray(inp["event_mask"])
    star = np.asarray(inp["star_rating"]).astype(f)
    nb = events.shape[0]
    diff = np.maximum(events[:, 1:] - events[:, :-1], 1)
    g = np.concatenate([diff[:, :1], diff], axis=1)
    gap_ms = (g * 5).astype(f)
    g_f = np.maximum(g.astype(f), f(1.0))
    r = np.clip(g_f[:, 1:] / g_f[:, :-1], f(0.1), f(10.0)).astype(f)
    ones = np.ones((nb, 1), f)
    rb50 = np.trunc(np.concatenate([ones, r], axis=1) * f(50.0)).astype(f)
    ra50 = np.trunc(np.concatenate([r, ones], axis=1) * f(50.0)).astype(f)
    out["_evrows"] = np.ascontiguousarray(np.stack([rb50, ra50, gap_ms], axis=1))
    tp = np.clip(events // 4, 0, S - 1).astype(f)
    keep = (1.0 - mask.astype(f)).astype(f)
    out["_tposv"] = np.ascontiguousarray(tp.reshape(nb, 2, 128).transpose(0, 2, 1))
    out["_keepv"] = np.ascontiguousarray(keep.reshape(nb, 2, 128).transpose(0, 2, 1))
    bucket = np.clip((star / f(0.5)).astype(np.int32), 0, 19)
    sb = (np.asarray(inp["cn_b"])[None, :] + np.asarray(inp["star_table"])[bucket]).astype(f)
    out["_starbias"] = np.ascontiguousarray(sb.reshape(nb, 2, 128).transpose(0, 2, 1))
    out["_mel"] = np.ascontiguousarray(np.asarray(inp["mel"]).astype(f).astype(BF16))
    return out


CONST_KEYS = ["wmelT", "melb", "w1t", "c1b", "gng", "gnb", "w2t", "c2b", "cng",
              "posT", "freqv", "epW1T", "epb1", "epW2T", "epb2row",
              "wqkvT", "bqkv", "woT", "bov", "ln1g", "ln1b", "ln2g", "ln2b",
              "w1TT", "b1v", "w2TT", "b2v", "poolq", "c1v", "onesZc", "e4m"]

# params that are bf16 on device
BF_KEYS = {"wmelT", "w1t", "w2t", "epW1T", "epW2T",
           "wqkvT", "woT", "w1TT", "w2TT", "mel4", "onesZc", "e4m"}


def _build(nl_run=NLAYERS, ns_run=NS, debug=False, sc1=0.0, c2s=0.0):
    import concourse.bacc as bacc
    import concourse.tile as tile
    from concourse import mybir
    from concourse.masks import make_identity

    f32 = mybir.dt.float32
    bf16 = mybir.dt.bfloat16
    A = mybir.AluOpType
    AF = mybir.ActivationFunctionType
    AX = mybir.AxisListType

    nc = bacc.Bacc(None)

    def mm(out, lhsT, rhs, **kw):
        nc.tensor.matmul(out=out, lhsT=lhsT, rhs=rhs, **kw)

    P = {}
    shapes = dict(
        mel4=[ns_run, 80, 2000], evrows=[ns_run, 3, 256], tposv=[ns_run, 128, 2],
        keepv=[ns_run, 128, 2], starbias=[ns_run, 128, 2],
        wmelT=[80, 16], melb=[16, 1], w1t=[16, 7, 128], c1b=[128, 1],
        gng=[128, 1], gnb=[128, 1], w2t=[128, 7, 256], c2b=[128, 2],
        cng=[128, 2], posT=[128, 2, 500], freqv=[128, 1],
        epW1T=[128, 6, 256], epb1=[128, 2], epW2T=[128, 2, 256], epb2row=[128, 256],
        wqkvT=[NLAYERS, 128, 2, 768], bqkv=[NLAYERS, 128, 6],
        woT=[NLAYERS, 128, 2, 256], bov=[NLAYERS, 128, 2],
        ln1g=[NLAYERS, 128, 2], ln1b=[NLAYERS, 128, 2],
        ln2g=[NLAYERS, 128, 2], ln2b=[NLAYERS, 128, 2],
        w1TT=[NLAYERS, 128, 2, 1024], b1v=[NLAYERS, 128, 8],
        w2TT=[NLAYERS, 128, 8, 256], b2v=[NLAYERS, 128, 2],
        poolq=[128, 2], c1v=[128, 2], onesZc=[128, 4, 4], e4m=[4, 128],
    )
    for k, sh in shapes.items():
        P[k] = nc.declare_dram_parameter(k, sh, bf16 if k in BF_KEYS else f32,
                                         isOutput=False)
    Y = nc.declare_dram_parameter("y", [ns_run, 1], f32, isOutput=True)

    with tile.TileContext(nc) as tc:
        sing = tc.alloc_tile_pool(name="sing", bufs=1)
        sc = tc.alloc_tile_pool(name="sc", bufs=1)
        bigx = tc.alloc_tile_pool(name="bigx", bufs=1)
        pm1 = tc.alloc_tile_pool(name="pm1", bufs=2, space="PSUM")
        pm2 = tc.alloc_tile_pool(name="pm2", bufs=1, space="PSUM")
        pst = tc.alloc_tile_pool(name="pst", bufs=1, space="PSUM")
        pmt = tc.alloc_tile_pool(name="pmt", bufs=1, space="PSUM")

        C = {}
        for k in ["wmelT", "melb", "w1t", "c1b", "gng", "gnb", "w2t", "c2b",
                  "cng", "posT", "freqv", "epW1T", "epb1", "epW2T", "epb2row",
                  "poolq", "c1v", "onesZc", "e4m"]:
            C[k] = sing.tile(shapes[k], bf16 if k in BF_KEYS else f32,
                             tag=k, name="c_" + k)
            nc.sync.dma_start(out=C[k][:], in_=P[k][:])
        identb = sing.tile([128, 128], bf16, tag="identb")
        make_identity(nc, identb[:])
        onesPf = sing.tile([128, 1], f32, tag="onesPf")
        nc.vector.memset(onesPf[:], 1.0)
        onesPb = sing.tile([128, 1], bf16, tag="onesPb")
        nc.vector.memset(onesPb[:], 1.0)
        ones1f = sing.tile([1, 128], f32, tag="ones1f")
        nc.vector.memset(ones1f[:], 1.0)
        ones1b = sing.tile([1, 128], bf16, tag="ones1b")
        nc.vector.memset(ones1b[:], 1.0)
        zerov = sing.tile([128, 1], f32, tag="zerov")
        nc.vector.memset(zerov[:], 0.0)
        epsv = sing.tile([128, 1], f32, tag="epsv")
        nc.vector.memset(epsv[:], EPS)
        iotaB = sing.tile([128, 500], f32, tag="iotaB")
        nc.gpsimd.iota(iotaB[:], pattern=[[1, 500]], base=0, channel_multiplier=0,
                       allow_small_or_imprecise_dtypes=True)

        x = bigx.tile([128, 2, 2000], f32, tag="x_fm")

        def s2(t):  # step-2 view of [p, n] -> [p, n//2]
            return t.rearrange("p (t s) -> p s t", s=2)[:, 0, :]

        # small-vector LN stat helper: psum [1,2,500] (s,ss) -> mr (m, r); also
        # writes mrb (bf16 copy of [m, r]) for cheap broadcast matmuls.
        def emit_stats(pstt, mr, mrb, tmp, scale):
            nc.vector.tensor_scalar(out=mr[:1, 0:2, :], in0=pstt[:1, 0:2, 0:500],
                                    scalar1=scale, scalar2=None, op0=A.mult)
            nc.vector.tensor_tensor(out=tmp[:1, 0, :], in0=mr[:1, 0, :],
                                    in1=mr[:1, 0, :], op=A.mult)
            nc.vector.tensor_tensor(out=tmp[:1, 1, :], in0=mr[:1, 1, :],
                                    in1=tmp[:1, 0, :], op=A.subtract)
            nc.scalar.activation(out=tmp[:1, 0, :], in_=tmp[:1, 1, :], func=AF.Ln,
                                 bias=epsv[0:1, :])
            nc.scalar.activation(out=mrb[:1, 1, :], in_=tmp[:1, 0, :], func=AF.Exp,
                                 scale=-0.5)
            nc.vector.tensor_copy(mrb[:1, 0, :], mr[:1, 0, :])

        # ================= front end =================
        fr = tc.alloc_tile_pool(name="fr", bufs=2)
        for s in range(ns_run):
            cs = s * 500
            melp = fr.tile([80, 2006], bf16, tag="melp")
            nc.vector.memset(melp[:, 0:3], 0.0)
            nc.vector.memset(melp[:, 2003:2006], 0.0)
            nc.sync.dma_start(out=melp[:, 3:2003], in_=P["mel4"][s])
            xmelp = fr.tile([16, 2006], bf16, tag="xmelp")
            nc.vector.memset(xmelp[:, 0:3], 0.0)
            nc.vector.memset(xmelp[:, 2003:2006], 0.0)
            for nch in range(4):
                pcm = pm1.tile([128, 500], f32, tag="pm1")
                mm(out=pcm[:16, :], lhsT=C["wmelT"][:],
                   rhs=melp[:, 3 + nch * 500: 3 + nch * 500 + 500],
                   start=True, stop=True)
                nc.scalar.activation(out=xmelp[:, 3 + nch * 500: 3 + nch * 500 + 500],
                                     in_=pcm[:16, :], func=AF.Identity,
                                     bias=C["melb"][:, 0:1])
            pc1 = pm2.tile([128, 2, 512], f32, tag="pm2")
            for half in range(2):
                for k in range(7):
                    mm(out=pc1[:, half, 0:500], lhsT=C["w1t"][:, k, :],
                       rhs=s2(xmelp[:, k + half * 1000: k + half * 1000 + 1000]),
                       start=(k == 0), stop=(k == 6))
            h1g = fr.tile([128, 2, 500], bf16, tag="h1g")
            stg = fr.tile([128, 2], f32, tag="stg")
            nc.scalar.activation(out=h1g[:], in_=pc1[:, :, 0:500], func=AF.Gelu,
                                 bias=C["c1b"][:, 0:1], accum_out=stg[:, 0:1])
            sqf = fr.tile([128, 2, 500], f32, tag="sqf")
            nc.scalar.activation(out=sqf[:], in_=h1g[:], func=AF.Square,
                                 accum_out=stg[:, 1:2])
            pg = pst.tile([1, 2, 512], f32, tag="pst")
            mm(out=pg[:1, 0, 0:2], lhsT=onesPf[:], rhs=stg[:], start=True, stop=True)
            sn = sc.tile([1, 8], f32, tag="sn")
            nc.vector.tensor_scalar(out=sn[:, 0:2], in0=pg[:1, 0, 0:2],
                                    scalar1=1.0 / 128000.0, scalar2=None, op0=A.mult)
            nc.vector.tensor_tensor(out=sn[:, 2:3], in0=sn[:, 0:1], in1=sn[:, 0:1],
                                    op=A.mult)
            nc.vector.tensor_tensor(out=sn[:, 3:4], in0=sn[:, 1:2], in1=sn[:, 2:3],
                                    op=A.subtract)
            nc.scalar.activation(out=sn[:, 4:5], in_=sn[:, 3:4], func=AF.Sqrt,
                                 bias=epsv[0:1, :])
            nc.vector.reciprocal_approx_fast(out=sn[:, 1:2], in_=sn[:, 4:5])
            pgb = pm1.tile([128, 500], f32, tag="pm1")
            mm(out=pgb[:, 0:2], lhsT=ones1f[:], rhs=sn[:, 0:2], start=True, stop=True)
            sv = sc.tile([128, 2], f32, tag="sv")
            nc.vector.tensor_tensor(out=sv[:, 0:1], in0=pgb[:, 1:2], in1=C["gng"][:],
                                    op=A.mult)
            nc.vector.tensor_tensor(out=sv[:, 1:2], in0=pgb[:, 0:1], in1=sv[:, 0:1],
                                    op=A.mult)
            nc.vector.tensor_tensor(out=sv[:, 1:2], in0=C["gnb"][:], in1=sv[:, 1:2],
                                    op=A.subtract)
            x2p = fr.tile([128, 1006], bf16, tag="x2p")
            nc.vector.memset(x2p[:, 0:3], 0.0)
            nc.vector.memset(x2p[:, 1003:1006], 0.0)
            nc.scalar.activation(out=x2p[:, 3:1003],
                                 in_=h1g.rearrange("p a b -> p (a b)"),
                                 func=AF.Identity, scale=sv[:, 0:1], bias=sv[:, 1:2])
            pc2 = pm2.tile([128, 2, 512], f32, tag="pm2")
            for mb in range(2):
                for k in range(7):
                    mm(out=pc2[:, mb, 0:500],
                       lhsT=C["w2t"][:, k, mb * 128:(mb + 1) * 128],
                       rhs=s2(x2p[:, k:k + 1000]),
                       start=(k == 0), stop=(k == 6))
            for mb in range(2):
                nc.scalar.activation(out=x[:, mb, cs:cs + 500], in_=pc2[:, mb, 0:500],
                                     func=AF.Gelu, bias=C["c2b"][:, mb:mb + 1])
            # CN layernorm + starbias + pos
            sbv = fr.tile([128, 2], f32, tag="sbv")
            nc.sync.dma_start(out=sbv[:], in_=P["starbias"][s])
            nc.scalar.activation(out=sqf[:], in_=x[:, :, cs:cs + 500], func=AF.Square)
            pstt = pst.tile([1, 2, 512], f32, tag="pst")
            for blk in range(2):
                mm(out=pstt[:1, 0, 0:500], lhsT=onesPf[:],
                   rhs=x[:, blk, cs:cs + 500], start=(blk == 0), stop=(blk == 1))
            for blk in range(2):
                mm(out=pstt[:1, 1, 0:500], lhsT=onesPf[:],
                   rhs=sqf[:, blk, :], start=(blk == 0), stop=(blk == 1))
            mr = sc.tile([1, 2, 500], f32, tag="mr")
            mrb = sc.tile([1, 2, 500], bf16, tag="mrb")
            tmp = sc.tile([1, 2, 500], f32, tag="tmp1")
            emit_stats(pstt, mr, mrb, tmp, 1.0 / 256.0)
            pbc = pm2.tile([128, 2, 512], f32, tag="pm2")
            for jj in range(2):
                mm(out=pbc[:, jj, 0:500], lhsT=ones1b[:], rhs=mrb[:1, jj, :],
                   start=True, stop=True)
            for blk in range(2):
                nc.vector.tensor_tensor(out=x[:, blk, cs:cs + 500],
                                        in0=x[:, blk, cs:cs + 500],
                                        in1=pbc[:, 0, 0:500], op=A.subtract)
                nc.vector.tensor_tensor(out=x[:, blk, cs:cs + 500],
                                        in0=x[:, blk, cs:cs + 500],
                                        in1=pbc[:, 1, 0:500], op=A.mult)
                nc.scalar.activation(out=x[:, blk, cs:cs + 500],
                                     in_=x[:, blk, cs:cs + 500], func=AF.Identity,
                                     scale=C["cng"][:, blk:blk + 1],
                                     bias=sbv[:, blk:blk + 1])
            nc.vector.tensor_tensor(out=x[:, :, cs:cs + 500], in0=x[:, :, cs:cs + 500],
                                    in1=C["posT"][:], op=A.add)

            # events
            evr = fr.tile([1, 3, 256], f32, tag="evr")
            nc.sync.dma_start(out=evr[:], in_=P["evrows"][s])
            tpv = fr.tile([128, 2], f32, tag="tpv")
            nc.sync.dma_start(out=tpv[:], in_=P["tposv"][s])
            kpv = fr.tile([128, 2], f32, tag="kpv")
            nc.sync.dma_start(out=kpv[:], in_=P["keepv"][s])
            comb = fr.tile([128, 6, 256], bf16, tag="comb")
            for vr in range(3):
                pb = pm1.tile([128, 500], f32, tag="pm1")
                mm(out=pb[:, 0:256], lhsT=ones1f[:], rhs=evr[:1, vr, :],
                   start=True, stop=True)
                arg = fr.tile([128, 256], f32, tag="arg")
                nc.scalar.activation(out=arg[:], in_=pb[:, 0:256], func=AF.Copy,
                                     scale=C["freqv"][:])
                nc.vector.tensor_scalar(out=arg[:], in0=arg[:], scalar1=INV2PI,
                                        scalar2=None, op0=A.mult)
                w1_ = fr.tile([128, 256], f32, tag="w1_")
                ti_ = fr.tile([128, 256], mybir.dt.int32, tag="ti_")
                tf_ = fr.tile([128, 256], f32, tag="tf_")
                nc.vector.tensor_copy(ti_[:], arg[:])
                nc.vector.tensor_copy(tf_[:], ti_[:])
                nc.vector.tensor_tensor(out=w1_[:], in0=arg[:], in1=tf_[:],
                                        op=A.subtract)
                nc.scalar.activation(out=comb[:, 2 * vr, :], in_=w1_[:], func=AF.Sin,
                                     scale=TWOPI, bias=zerov[:])
                nc.vector.tensor_scalar(out=arg[:], in0=arg[:], scalar1=0.25,
                                        scalar2=None, op0=A.add)
                nc.vector.tensor_copy(ti_[:], arg[:])
                nc.vector.tensor_copy(tf_[:], ti_[:])
                nc.vector.tensor_tensor(out=w1_[:], in0=arg[:], in1=tf_[:],
                                        op=A.subtract)
                nc.scalar.activation(out=comb[:, 2 * vr + 1, :], in_=w1_[:],
                                     func=AF.Sin, scale=TWOPI, bias=zerov[:])
            hmid = fr.tile([128, 2, 256], bf16, tag="hmid")
            for mb in range(2):
                ph = pm1.tile([128, 500], f32, tag="pm1")
                for kc in range(6):
                    mm(out=ph[:, 0:256],
                       lhsT=C["epW1T"][:, kc, mb * 128:(mb + 1) * 128],
                       rhs=comb[:, kc, :], start=(kc == 0), stop=(kc == 5))
                nc.scalar.activation(out=hmid[:, mb, :], in_=ph[:, 0:256],
                                     func=AF.Gelu, bias=C["epb1"][:, mb:mb + 1])
            evt = fr.tile([128, 2, 256], bf16, tag="evt")
            for ec in range(2):
                pe = pm1.tile([128, 500], f32, tag="pm1")
                for kc in range(2):
                    mm(out=pe[:, 0:256],
                       lhsT=hmid[:, kc, ec * 128:(ec + 1) * 128],
                       rhs=C["epW2T"][:, kc, :], start=(kc == 0), stop=(kc == 1))
                nc.vector.tensor_tensor(out=evt[:, ec, :], in0=pe[:, 0:256],
                                        in1=C["epb2row"][:], op=A.add)
                nc.vector.tensor_scalar(out=evt[:, ec, :], in0=evt[:, ec, :],
                                        scalar1=kpv[:, ec:ec + 1], scalar2=None,
                                        op0=A.mult)
            oh = fr.tile([128, 2, 500], bf16, tag="oh")
            for ec in range(2):
                nc.vector.tensor_scalar(out=oh[:, ec, :], in0=iotaB[:],
                                        scalar1=tpv[:, ec:ec + 1], scalar2=None,
                                        op0=A.is_equal)
            for mb in range(2):
                px = pm1.tile([128, 500], f32, tag="pm1")
                for ec in range(2):
                    mm(out=px[:], lhsT=evt[:, ec, mb * 128:(mb + 1) * 128],
                       rhs=oh[:, ec, :], start=(ec == 0), stop=(ec == 1))
                nc.vector.tensor_tensor(out=x[:, mb, cs:cs + 500],
                                        in0=x[:, mb, cs:cs + 500], in1=px[:], op=A.add)
        fr.release()
        wpool = tc.alloc_tile_pool(name="wpool", bufs=2)
        big = tc.alloc_tile_pool(name="big", bufs=1)
        scr = tc.alloc_tile_pool(name="scr", bufs=1)
        scr2 = tc.alloc_tile_pool(name="scr2", bufs=1)

        # ================= transformer =================
        for i in range(nl_run):
            W = {}
            for k, sh, dt in [("wqkvT", [128, 2, 768], bf16), ("bqkv", [128, 6], f32),
                              ("woT", [128, 2, 256], bf16), ("bov", [128, 2], f32),
                              ("ln1g", [128, 2], f32), ("ln1b", [128, 2], f32),
                              ("ln2g", [128, 2], f32), ("ln2b", [128, 2], f32),
                              ("w1TT", [128, 2, 1024], bf16), ("b1v", [128, 8], f32),
                              ("w2TT", [128, 8, 256], bf16), ("b2v", [128, 2], f32)]:
                W[k] = wpool.tile(sh, dt, tag="w_" + k, name=f"w{i}_" + k)
                nc.sync.dma_start(out=W[k][:], in_=P[k][i])

            def emit_ln(gk, bk, xn):
                for nch in range(4):
                    co = nch * 500
                    sq = scr2.tile([128, 2, 500], f32, tag="sq")
                    nc.scalar.activation(out=sq[:], in_=x[:, :, co:co + 500],
                                         func=AF.Square)
                    pstt = pst.tile([1, 2, 512], f32, tag="pst")
                    for blk in range(2):
                        mm(out=pstt[:1, 0, 0:500], lhsT=onesPf[:],
                           rhs=x[:, blk, co:co + 500],
                           start=(blk == 0), stop=(blk == 1))
                    for blk in range(2):
                        mm(out=pstt[:1, 1, 0:500], lhsT=onesPf[:],
                           rhs=sq[:, blk, :], start=(blk == 0), stop=(blk == 1))
                    mr = sc.tile([1, 2, 500], f32, tag="mr")
                    mrb = sc.tile([1, 2, 500], bf16, tag="mrb")
                    tmp = sc.tile([1, 2, 500], f32, tag="tmp1")
                    emit_stats(pstt, mr, mrb, tmp, 1.0 / 256.0)
                    pbc = pm2.tile([128, 2, 512], f32, tag="pm2")
                    for jj in range(2):
                        mm(out=pbc[:, jj, 0:500], lhsT=ones1b[:],
                           rhs=mrb[:1, jj, :], start=True, stop=True)
                    for blk in range(2):
                        nc.vector.tensor_tensor(out=xn[:, blk, co:co + 500],
                                                in0=x[:, blk, co:co + 500],
                                                in1=pbc[:, 0, 0:500], op=A.subtract)
                        nc.vector.tensor_tensor(out=xn[:, blk, co:co + 500],
                                                in0=xn[:, blk, co:co + 500],
                                                in1=pbc[:, 1, 0:500], op=A.mult)
                        nc.scalar.activation(out=xn[:, blk, co:co + 500],
                                             in_=xn[:, blk, co:co + 500],
                                             func=AF.Identity,
                                             scale=W[gk][:, blk:blk + 1],
                                             bias=W[bk][:, blk:blk + 1])

            xn = big.tile([128, 2, 2000], bf16, tag="xn")
            emit_ln("ln1g", "ln1b", xn)
            attn = big.tile([128, 2, 2000], bf16, tag="attn")
            for s in range(ns_run):
                cs = s * 500
                qkv = scr.tile([128, 6, 500], bf16, tag="qkv")
                for j in range(6):
                    pq = pm1.tile([128, 500], f32, tag="pm1")
                    for kc in range(2):
                        mm(out=pq[:], lhsT=W["wqkvT"][:, kc, j * 128:(j + 1) * 128],
                           rhs=xn[:, kc, cs:cs + 500],
                           start=(kc == 0), stop=(kc == 1))
                    nc.vector.tensor_scalar(out=qkv[:, j, :], in0=pq[:],
                                            scalar1=W["bqkv"][:, j:j + 1],
                                            scalar2=None, op0=A.add)
                for j in range(2):
                    vt = scr2.tile([128, 4, 128], bf16, tag="vt")
                    pv4 = pmt.tile([128, 4, 128], bf16, tag="pvt")
                    for skc in range(4):
                        nc.tensor.transpose(
                            out=pv4[:125, skc, :],
                            in_=qkv[:, 4 + j, skc * 125: skc * 125 + 125],
                            identity=identb[:])
                    nc.scalar.activation(out=vt[:125, :, :],
                                         in_=pv4[:125, :, :], func=AF.Copy)
                    pot4 = pm1.tile([128, 500], f32, tag="pm1")
                    pcs4 = pst.tile([4, 512], f32, tag="pcs4")
                    for c4 in range(4):
                        poff = 32 * c4
                        h_q = qkv[poff:poff + 32, j, :]
                        h_k = qkv[poff:poff + 32, 2 + j, :]
                        eT = scr.tile([128, 4, 500], bf16, tag="eT")
                        for pair in range(2):
                            psc = pm2.tile([128, 2, 512], f32, tag="pm2")
                            for sub in range(2):
                                skc = 2 * pair + sub
                                mm(out=psc[:125, sub, 0:500],
                                   lhsT=h_k[:, skc * 125: skc * 125 + 125],
                                   rhs=h_q, start=True, stop=True,
                                   tile_position=(poff, 0))
                            nc.scalar.activation(
                                out=eT[:125, 2 * pair: 2 * pair + 2, :],
                                in_=psc[:125, :, 0:500], func=AF.Exp, scale=SC32)
                        for skc in range(4):
                            mm(out=pcs4[0:4, 0:500], lhsT=C["onesZc"][:125, c4, :],
                               rhs=eT[:125, skc, :],
                               start=(c4 == 0 and skc == 0),
                               stop=(c4 == 3 and skc == 3))
                        for skc in range(4):
                            mm(out=pot4[poff:poff + 32, :],
                               lhsT=vt[:125, skc, poff:poff + 32],
                               rhs=eT[:125, skc, :],
                               start=(skc == 0), stop=(skc == 3),
                               tile_position=(0, poff))
                    rrf = sc.tile([4, 500], f32, tag="rrf")
                    nc.vector.reciprocal_approx_fast(out=rrf[:],
                                                     in_=pcs4[0:4, 0:500])
                    rrb4 = sc.tile([4, 500], bf16, tag="rrb4")
                    nc.vector.tensor_copy(rrb4[:], rrf[:])
                    prbF = pm2.tile([128, 2, 512], f32, tag="pm2")
                    mm(out=prbF[:, 0, 0:500], lhsT=C["e4m"][:], rhs=rrb4[:],
                       start=True, stop=True)
                    rbsF = scr2.tile([128, 500], bf16, tag="rbs")
                    nc.scalar.activation(out=rbsF[:], in_=prbF[:, 0, 0:500],
                                         func=AF.Copy)
                    nc.vector.tensor_tensor(out=attn[:, j, cs:cs + 500],
                                            in0=pot4[:], in1=rbsF[:], op=A.mult)
            for mb in range(2):
                for nch in range(4):
                    po = pm1.tile([128, 500], f32, tag="pm1")
                    for kc in range(2):
                        mm(out=po[:], lhsT=W["woT"][:, kc, mb * 128:(mb + 1) * 128],
                           rhs=attn[:, kc, nch * 500:(nch + 1) * 500],
                           start=(kc == 0), stop=(kc == 1))
                    nc.vector.tensor_tensor(out=x[:, mb, nch * 500:(nch + 1) * 500],
                                            in0=x[:, mb, nch * 500:(nch + 1) * 500],
                                            in1=po[:], op=A.add)
                nc.vector.tensor_scalar(out=x[:, mb, :], in0=x[:, mb, :],
                                        scalar1=W["bov"][:, mb:mb + 1], scalar2=None,
                                        op0=A.add)
            xn2 = big.tile([128, 2, 2000], bf16, tag="xn")
            emit_ln("ln2g", "ln2b", xn2)
            for s in range(ns_run):
                cs = s * 500
                fh = scr.tile([128, 8, 500], bf16, tag="fh")
                for hb in range(8):
                    phh = pm1.tile([128, 500], f32, tag="pm1")
                    for kc in range(2):
                        mm(out=phh[:], lhsT=W["w1TT"][:, kc, hb * 128:(hb + 1) * 128],
                           rhs=xn2[:, kc, cs:cs + 500], start=(kc == 0),
                           stop=(kc == 1))
                    nc.scalar.activation(out=fh[:, hb, :], in_=phh[:], func=AF.Gelu,
                                         bias=W["b1v"][:, hb:hb + 1])
                for mb in range(2):
                    pf = pm1.tile([128, 500], f32, tag="pm1")
                    for hb in range(8):
                        mm(out=pf[:], lhsT=W["w2TT"][:, hb, mb * 128:(mb + 1) * 128],
                           rhs=fh[:, hb, :], start=(hb == 0), stop=(hb == 7))
                    nc.vector.tensor_tensor(out=x[:, mb, cs:cs + 500],
                                            in0=x[:, mb, cs:cs + 500], in1=pf[:],
                                            op=A.add)
            for mb in range(2):
                nc.vector.tensor_scalar(out=x[:, mb, :], in0=x[:, mb, :],
                                        scalar1=W["b2v"][:, mb:mb + 1], scalar2=None,
                                        op0=A.add)

        # ================= pooling + head =================
        for s in range(ns_run):
            cs = s * 500
            plg = pst.tile([1, 2, 512], f32, tag="pst")
            for blk in range(2):
                mm(out=plg[:1, 0, 0:500], lhsT=C["poolq"][:, blk:blk + 1],
                   rhs=x[:, blk, cs:cs + 500], start=(blk == 0), stop=(blk == 1))
            wrow = sc.tile([1, 500], f32, tag="wrow")
            nc.scalar.activation(out=wrow[:], in_=plg[:1, 0, 0:500], func=AF.Exp,
                                 scale=1.0 / 16.0)
            pwb = pm1.tile([128, 500], f32, tag="pm1")
            mm(out=pwb[:], lhsT=ones1f[:], rhs=wrow[:], start=True, stop=True)
            wx = scr2.tile([128, 2, 500], f32, tag="sq")
            for blk in range(2):
                nc.vector.tensor_tensor(out=wx[:, blk, :], in0=x[:, blk, cs:cs + 500],
                                        in1=pwb[:], op=A.mult)
            pooled = sc.tile([128, 4], f32, tag="pooled")
            nc.vector.tensor_reduce(out=pooled[:, 0:2], in_=wx[:], axis=AX.X,
                                    op=A.add)
            nc.scalar.activation(out=pooled[:, 2:4], in_=pooled[:, 0:2],
                                 func=AF.Square)
            pps = pst.tile([1, 2, 512], f32, tag="pst")
            mm(out=pps[:1, 0, 0:4], lhsT=onesPf[:], rhs=pooled[:], start=True,
               stop=True)
            z = sc.tile([1, 16], f32, tag="z")
            nc.vector.tensor_copy(z[:, 12:16], pps[:1, 0, 0:4])
            nc.vector.tensor_tensor(out=z[:, 0:1], in0=z[:, 12:13],
                                    in1=z[:, 13:14], op=A.add)
            nc.vector.tensor_tensor(out=z[:, 1:2], in0=z[:, 14:15],
                                    in1=z[:, 15:16], op=A.add)
            nc.vector.tensor_scalar(out=z[:, 2:3], in0=z[:, 0:1],
                                    scalar1=1.0 / 256.0, scalar2=None, op0=A.mult)
            nc.vector.tensor_tensor(out=z[:, 3:4], in0=z[:, 2:3], in1=z[:, 2:3],
                                    op=A.mult)
            nc.vector.tensor_scalar(out=z[:, 4:5], in0=z[:, 1:2],
                                    scalar1=1.0 / 256.0, scalar2=None, op0=A.mult)
            nc.vector.tensor_tensor(out=z[:, 4:5], in0=z[:, 4:5], in1=z[:, 3:4],
                                    op=A.subtract)
            nc.scalar.activation(out=z[:, 5:6], in_=z[:, 4:5], func=AF.Sqrt,
                                 bias=epsv[0:1, :])
            nc.vector.reciprocal_approx_fast(out=z[:, 6:7], in_=z[:, 5:6])
            cp = sc.tile([128, 2], f32, tag="cp")
            nc.vector.tensor_tensor(out=cp[:], in0=pooled[:, 0:2], in1=C["c1v"][:],
                                    op=A.mult)
            pa = pst.tile([1, 2, 512], f32, tag="pst")
            mm(out=pa[:1, 0, 0:2], lhsT=onesPf[:], rhs=cp[:], start=True, stop=True)
            nc.vector.tensor_copy(z[:, 10:12], pa[:1, 0, 0:2])
            nc.vector.tensor_tensor(out=z[:, 7:8], in0=z[:, 10:11],
                                    in1=z[:, 11:12], op=A.add)
            nc.vector.tensor_scalar(out=z[:, 8:9], in0=z[:, 2:3], scalar1=sc1,
                                    scalar2=None, op0=A.mult)
            nc.vector.tensor_tensor(out=z[:, 8:9], in0=z[:, 7:8], in1=z[:, 8:9],
                                    op=A.subtract)
            nc.vector.tensor_tensor(out=z[:, 8:9], in0=z[:, 8:9], in1=z[:, 6:7],
                                    op=A.mult)
            nc.vector.tensor_scalar(out=z[:, 9:10], in0=z[:, 8:9], scalar1=c2s,
                                    scalar2=None, op0=A.add)
            nc.sync.dma_start(out=Y[s:s + 1, :], in_=z[:, 9:10])

        for p in [pmt, pst, pm2, scr2, scr, big, wpool, pm1, bigx, sc, sing]:
            p.release()

    nc.compile()
    return nc


_BUILT = {}


def _get_nc(key, **kw):
    if key not in _BUILT:
        _BUILT[key] = _build(**kw)
    return _BUILT[key]


def _make_in_maps(prep, ns=NS, ncores=NCORES):
    in_maps = []
    for c in range(ncores):
        sl = slice(c * ns, (c + 1) * ns)
        m = {k: prep[k] for k in CONST_KEYS}
        m["mel4"] = prep["_mel"][sl]
        m["evrows"] = prep["_evrows"][sl]
        m["tposv"] = prep["_tposv"][sl]
        m["keepv"] = prep["_keepv"][sl]
        m["starbias"] = prep["_starbias"][sl]
        in_maps.append(m)
    return in_maps


def kernel(**inputs):
    from concourse.bass_utils import run_bass_kernel_spmd

    prep = _host_prep(inputs)
    nc = _get_nc("full", nl_run=NLAYERS, ns_run=NS, debug=False,
                 sc1=prep["sc1"], c2s=prep["c2s"])
    res = run_bass_kernel_spmd(nc, _make_in_maps(prep), list(range(NCORES)))
    y = np.concatenate([res.results[c]["y"].reshape(-1) for c in range(NCORES)])
    return y.astype(np.float32)


# revision 13
# speedup vs baseline: 1.0222x; 1.0222x over previous
"""Trainium2 Bass kernel for nn_ChartQualityEvaluator.

Data parallel: 32 samples -> 8 cores x 4 samples. Feature-major activations
[128 part, 2 blocks, 4*500 cols]. Matmuls and bulk activations in bf16
(fp32 PSUM accumulation); stats/sinusoid paths stay fp32. Unsafe softmax,
LN via ones-matmul stats + PE broadcast, event scatter via onehot matmul.
Host precomputes index-like preprocessing with f32-exact semantics.
"""
import math
import sys

import numpy as np
import ml_dtypes

_TRN = "/opt/trn_rl_repo"
if _TRN not in sys.path:
    sys.path.insert(0, _TRN)

BF16 = ml_dtypes.bfloat16

D = 256
H = 8
NLAYERS = 6
HALF = 128
S = 500
NEV = 256
NCORES = 8
NS = 4  # samples per core
B = 32
EPS = 1e-5
INV2PI = float(np.float32(1.0 / (2.0 * math.pi)))
TWOPI = 2.0 * math.pi
SC32 = float(np.float32(1.0 / math.sqrt(32.0)))


def _host_prep(inp):
    f = np.float32
    out = {}

    def t2(v):  # [256] -> [128,2]
        return np.ascontiguousarray(np.asarray(v).reshape(2, 128).T.astype(f))

    def b(a):  # -> bf16
        return np.ascontiguousarray(np.asarray(a).astype(np.float32).astype(BF16))

    out["wmelT"] = b(np.asarray(inp["mel_W"]).T)
    out["melb"] = np.ascontiguousarray(np.asarray(inp["mel_b"]).reshape(16, 1).astype(f))
    out["w1t"] = b(np.asarray(inp["conv1_w"]).transpose(1, 2, 0))
    out["c1b"] = np.ascontiguousarray(np.asarray(inp["conv1_b"]).reshape(128, 1).astype(f))
    out["gng"] = np.ascontiguousarray(np.asarray(inp["gn_g"]).reshape(128, 1).astype(f))
    out["gnb"] = np.ascontiguousarray(np.asarray(inp["gn_b"]).reshape(128, 1).astype(f))
    out["w2t"] = b(np.asarray(inp["conv2_w"]).transpose(1, 2, 0))
    out["c2b"] = t2(inp["conv2_b"])
    out["cng"] = t2(inp["cn_g"])

    freq = np.exp(np.arange(HALF, dtype=f) * f(-math.log(10000.0) / (HALF - 1)))
    e32 = (np.arange(S, dtype=f)[None, :] * freq[:, None]).astype(f)
    e64 = e32.astype(np.float64)
    pos_fm = np.concatenate([np.sin(e64), np.cos(e64)], axis=0)  # [256,500]
    out["posT"] = np.ascontiguousarray(
        pos_fm.reshape(2, 128, S).transpose(1, 0, 2).astype(f))
    out["freqv"] = np.ascontiguousarray(freq.reshape(128, 1))

    out["epW1T"] = b(np.asarray(inp["ep_W1"]).T.reshape(6, 128, 256).transpose(1, 0, 2))
    out["epb1"] = t2(inp["ep_b1"])
    out["epW2T"] = b(np.asarray(inp["ep_W2"]).T.reshape(2, 128, 256).transpose(1, 0, 2))
    out["epb2row"] = np.ascontiguousarray(
        np.tile(np.asarray(inp["ep_b2"]).astype(f)[None, :], (128, 1)))

    def wT(w, kc, m):  # w [m, k] -> [128, kc, m] bf16
        return b(np.asarray(w).T.reshape(kc, 128, m).transpose(1, 0, 2))

    out["wqkvT"] = np.stack([wT(inp["tl_Wqkv"][i], 2, 768) for i in range(NLAYERS)])
    out["bqkv"] = np.stack([np.ascontiguousarray(
        np.asarray(inp["tl_bqkv"][i]).reshape(6, 128).T.astype(f))
        for i in range(NLAYERS)])
    out["woT"] = np.stack([wT(inp["tl_Wo"][i], 2, 256) for i in range(NLAYERS)])
    out["bov"] = np.stack([t2(inp["tl_bo"][i]) for i in range(NLAYERS)])
    out["ln1g"] = np.stack([t2(inp["tl_ln1g"][i]) for i in range(NLAYERS)])
    out["ln1b"] = np.stack([t2(inp["tl_ln1b"][i]) for i in range(NLAYERS)])
    out["ln2g"] = np.stack([t2(inp["tl_ln2g"][i]) for i in range(NLAYERS)])
    out["ln2b"] = np.stack([t2(inp["tl_ln2b"][i]) for i in range(NLAYERS)])
    out["w1TT"] = np.stack([wT(inp["tl_W1"][i], 2, 1024) for i in range(NLAYERS)])
    out["b1v"] = np.stack([np.ascontiguousarray(
        np.asarray(inp["tl_b1"][i]).reshape(8, 128).T.astype(f))
        for i in range(NLAYERS)])
    out["w2TT"] = np.stack([wT(inp["tl_W2"][i], 8, 256) for i in range(NLAYERS)])
    out["b2v"] = np.stack([t2(inp["tl_b2"][i]) for i in range(NLAYERS)])

    out["poolq"] = t2(inp["pool_q"])
    oz = np.zeros((128, 4, 4), np.float32)
    for c4 in range(4):
        oz[:, c4, c4] = 1.0
    out["onesZc"] = b(oz)
    e4 = np.zeros((4, 128), np.float32)
    for c4 in range(4):
        e4[c4, 32 * c4:32 * c4 + 32] = 1.0
    out["e4m"] = b(e4)
    c1 = (np.asarray(inp["oh_W"])[0] * np.asarray(inp["on_g"])).astype(f)
    out["c1v"] = t2(c1)
    out["sc1"] = float(c1.astype(np.float64).sum())
    out["c2s"] = float((np.asarray(inp["oh_W"])[0].astype(np.float64)
                        * np.asarray(inp["on_b"]).astype(np.float64)).sum()
                       + float(np.asarray(inp["oh_b"])[0]))

    events = np.asarray(inp["events"]).astype(np.int64)
    mask = np.asarray(inp["event_mask"])
    star = np.asarray(inp["star_rating"]).astype(f)
    nb = events.shape[0]
    diff = np.maximum(events[:, 1:] - events[:, :-1], 1)
    g = np.concatenate([diff[:, :1], diff], axis=1)
    gap_ms = (g * 5).astype(f)
    g_f = np.maximum(g.astype(f), f(1.0))
    r = np.clip(g_f[:, 1:] / g_f[:, :-1], f(0.1), f(10.0)).astype(f)
    ones = np.ones((nb, 1), f)
    rb50 = np.trunc(np.concatenate([ones, r], axis=1) * f(50.0)).astype(f)
    ra50 = np.trunc(np.concatenate([r, ones], axis=1) * f(50.0)).astype(f)
    out["_evrows"] = np.ascontiguousarray(np.stack([rb50, ra50, gap_ms], axis=1))
    tp = np.clip(events // 4, 0, S - 1).astype(f)
    keep = (1.0 - mask.astype(f)).astype(f)
    out["_tposv"] = np.ascontiguousarray(tp.reshape(nb, 2, 128).transpose(0, 2, 1))
    out["_keepv"] = np.ascontiguousarray(keep.reshape(nb, 2, 128).transpose(0, 2, 1))
    bucket = np.clip((star / f(0.5)).astype(np.int32), 0, 19)
    sb = (np.asarray(inp["cn_b"])[None, :] + np.asarray(inp["star_table"])[bucket]).astype(f)
    out["_starbias"] = np.ascontiguousarray(sb.reshape(nb, 2, 128).transpose(0, 2, 1))
    out["_mel"] = np.ascontiguousarray(np.asarray(inp["mel"]).astype(f).astype(BF16))
    return out


CONST_KEYS = ["wmelT", "melb", "w1t", "c1b", "gng", "gnb", "w2t", "c2b", "cng",
              "posT", "freqv", "epW1T", "epb1", "epW2T", "epb2row",
              "wqkvT", "bqkv", "woT", "bov", "ln1g", "ln1b", "ln2g", "ln2b",
              "w1TT", "b1v", "w2TT", "b2v", "poolq", "c1v", "onesZc", "e4m"]

# params that are bf16 on device
BF_KEYS = {"wmelT", "w1t", "w2t", "epW1T", "epW2T",
           "wqkvT", "woT", "w1TT", "w2TT", "mel4", "onesZc", "e4m"}


def _build(nl_run=NLAYERS, ns_run=NS, debug=False, sc1=0.0, c2s=0.0):
    import concourse.bacc as bacc
    import concourse.tile as tile
    from concourse import mybir
    from concourse.masks import make_identity

    f32 = mybir.dt.float32
    bf16 = mybir.dt.bfloat16
    A = mybir.AluOpType
    AF = mybir.ActivationFunctionType
    AX = mybir.AxisListType

    nc = bacc.Bacc(None)

    def mm(out, lhsT, rhs, **kw):
        nc.tensor.matmul(out=out, lhsT=lhsT, rhs=rhs, **kw)

    P = {}
    shapes = dict(
        mel4=[ns_run, 80, 2000], evrows=[ns_run, 3, 256], tposv=[ns_run, 128, 2],
        keepv=[ns_run, 128, 2], starbias=[ns_run, 128, 2],
        wmelT=[80, 16], melb=[16, 1], w1t=[16, 7, 128], c1b=[128, 1],
        gng=[128, 1], gnb=[128, 1], w2t=[128, 7, 256], c2b=[128, 2],
        cng=[128, 2], posT=[128, 2, 500], freqv=[128, 1],
        epW1T=[128, 6, 256], epb1=[128, 2], epW2T=[128, 2, 256], epb2row=[128, 256],
        wqkvT=[NLAYERS, 128, 2, 768], bqkv=[NLAYERS, 128, 6],
        woT=[NLAYERS, 128, 2, 256], bov=[NLAYERS, 128, 2],
        ln1g=[NLAYERS, 128, 2], ln1b=[NLAYERS, 128, 2],
        ln2g=[NLAYERS, 128, 2], ln2b=[NLAYERS, 128, 2],
        w1TT=[NLAYERS, 128, 2, 1024], b1v=[NLAYERS, 128, 8],
        w2TT=[NLAYERS, 128, 8, 256], b2v=[NLAYERS, 128, 2],
        poolq=[128, 2], c1v=[128, 2], onesZc=[128, 4, 4], e4m=[4, 128],
    )
    for k, sh in shapes.items():
        P[k] = nc.declare_dram_parameter(k, sh, bf16 if k in BF_KEYS else f32,
                                         isOutput=False)
    Y = nc.declare_dram_parameter("y", [ns_run, 1], f32, isOutput=True)

    with tile.TileContext(nc) as tc:
        sing = tc.alloc_tile_pool(name="sing", bufs=1)
        sc = tc.alloc_tile_pool(name="sc", bufs=1)
        bigx = tc.alloc_tile_pool(name="bigx", bufs=1)
        pm1 = tc.alloc_tile_pool(name="pm1", bufs=2, space="PSUM")
        pm2 = tc.alloc_tile_pool(name="pm2", bufs=1, space="PSUM")
        pst = tc.alloc_tile_pool(name="pst", bufs=1, space="PSUM")
        pmt = tc.alloc_tile_pool(name="pmt", bufs=1, space="PSUM")

        C = {}
        for k in ["wmelT", "melb", "w1t", "c1b", "gng", "gnb", "w2t", "c2b",
                  "cng", "posT", "freqv", "epW1T", "epb1", "epW2T", "epb2row",
                  "poolq", "c1v", "onesZc", "e4m"]:
            C[k] = sing.tile(shapes[k], bf16 if k in BF_KEYS else f32,
                             tag=k, name="c_" + k)
            nc.sync.dma_start(out=C[k][:], in_=P[k][:])
        identb = sing.tile([128, 128], bf16, tag="identb")
        make_identity(nc, identb[:])
        onesPf = sing.tile([128, 1], f32, tag="onesPf")
        nc.vector.memset(onesPf[:], 1.0)
        onesPb = sing.tile([128, 1], bf16, tag="onesPb")
        nc.vector.memset(onesPb[:], 1.0)
        ones1f = sing.tile([1, 128], f32, tag="ones1f")
        nc.vector.memset(ones1f[:], 1.0)
        ones1b = sing.tile([1, 128], bf16, tag="ones1b")
        nc.vector.memset(ones1b[:], 1.0)
        zerov = sing.tile([128, 1], f32, tag="zerov")
        nc.vector.memset(zerov[:], 0.0)
        epsv = sing.tile([128, 1], f32, tag="epsv")
        nc.vector.memset(epsv[:], EPS)
        iotaB = sing.tile([128, 500], f32, tag="iotaB")
        nc.gpsimd.iota(iotaB[:], pattern=[[1, 500]], base=0, channel_multiplier=0,
                       allow_small_or_imprecise_dtypes=True)

        x = bigx.tile([128, 2, 2000], f32, tag="x_fm")

        def s2(t):  # step-2 view of [p, n] -> [p, n//2]
            return t.rearrange("p (t s) -> p s t", s=2)[:, 0, :]

        # small-vector LN stat helper: psum [1,2,500] (s,ss) -> mr (m, r); also
        # writes mrb (bf16 copy of [m, r]) for cheap broadcast matmuls.
        def emit_stats(pstt, mr, mrb, tmp, scale):
            nc.vector.tensor_scalar(out=mr[:1, 0:2, :], in0=pstt[:1, 0:2, 0:500],
                                    scalar1=scale, scalar2=None, op0=A.mult)
            nc.vector.tensor_tensor(out=tmp[:1, 0, :], in0=mr[:1, 0, :],
                                    in1=mr[:1, 0, :], op=A.mult)
            nc.vector.tensor_tensor(out=tmp[:1, 1, :], in0=mr[:1, 1, :],
                                    in1=tmp[:1, 0, :], op=A.subtract)
            nc.scalar.activation(out=tmp[:1, 0, :], in_=tmp[:1, 1, :], func=AF.Sqrt,
                                 bias=epsv[0:1, :])
            nc.vector.reciprocal_approx_fast(out=mr[:1, 1, :], in_=tmp[:1, 0, :])
            nc.vector.tensor_copy(mrb[:1, 0:2, :], mr[:1, 0:2, :])

        # ================= front end =================
        fr = tc.alloc_tile_pool(name="fr", bufs=2)
        for s in range(ns_run):
            cs = s * 500
            melp = fr.tile([80, 2006], bf16, tag="melp")
            nc.vector.memset(melp[:, 0:3], 0.0)
            nc.vector.memset(melp[:, 2003:2006], 0.0)
            nc.sync.dma_start(out=melp[:, 3:2003], in_=P["mel4"][s])
            xmelp = fr.tile([16, 2006], bf16, tag="xmelp")
            nc.vector.memset(xmelp[:, 0:3], 0.0)
            nc.vector.memset(xmelp[:, 2003:2006], 0.0)
            for nch in range(4):
                pcm = pm1.tile([128, 500], f32, tag="pm1")
                mm(out=pcm[:16, :], lhsT=C["wmelT"][:],
                   rhs=melp[:, 3 + nch * 500: 3 + nch * 500 + 500],
                   start=True, stop=True)
                nc.scalar.activation(out=xmelp[:, 3 + nch * 500: 3 + nch * 500 + 500],
                                     in_=pcm[:16, :], func=AF.Identity,
                                     bias=C["melb"][:, 0:1])
            pc1 = pm2.tile([128, 2, 512], f32, tag="pm2")
            for half in range(2):
                for k in range(7):
                    mm(out=pc1[:, half, 0:500], lhsT=C["w1t"][:, k, :],
                       rhs=s2(xmelp[:, k + half * 1000: k + half * 1000 + 1000]),
                       start=(k == 0), stop=(k == 6))
            h1g = fr.tile([128, 2, 500], bf16, tag="h1g")
            stg = fr.tile([128, 2], f32, tag="stg")
            nc.scalar.activation(out=h1g[:], in_=pc1[:, :, 0:500], func=AF.Gelu,
                                 bias=C["c1b"][:, 0:1], accum_out=stg[:, 0:1])
            sqf = fr.tile([128, 2, 500], f32, tag="sqf")
            nc.scalar.activation(out=sqf[:], in_=h1g[:], func=AF.Square,
                                 accum_out=stg[:, 1:2])
            pg = pst.tile([1, 2, 512], f32, tag="pst")
            mm(out=pg[:1, 0, 0:2], lhsT=onesPf[:], rhs=stg[:], start=True, stop=True)
            sn = sc.tile([1, 8], f32, tag="sn")
            nc.vector.tensor_scalar(out=sn[:, 0:2], in0=pg[:1, 0, 0:2],
                                    scalar1=1.0 / 128000.0, scalar2=None, op0=A.mult)
            nc.vector.tensor_tensor(out=sn[:, 2:3], in0=sn[:, 0:1], in1=sn[:, 0:1],
                                    op=A.mult)
            nc.vector.tensor_tensor(out=sn[:, 3:4], in0=sn[:, 1:2], in1=sn[:, 2:3],
                                    op=A.subtract)
            nc.scalar.activation(out=sn[:, 4:5], in_=sn[:, 3:4], func=AF.Sqrt,
                                 bias=epsv[0:1, :])
            nc.vector.reciprocal_approx_fast(out=sn[:, 1:2], in_=sn[:, 4:5])
            pgb = pm1.tile([128, 500], f32, tag="pm1")
            mm(out=pgb[:, 0:2], lhsT=ones1f[:], rhs=sn[:, 0:2], start=True, stop=True)
            sv = sc.tile([128, 2], f32, tag="sv")
            nc.vector.tensor_tensor(out=sv[:, 0:1], in0=pgb[:, 1:2], in1=C["gng"][:],
                                    op=A.mult)
            nc.vector.tensor_tensor(out=sv[:, 1:2], in0=pgb[:, 0:1], in1=sv[:, 0:1],
                                    op=A.mult)
            nc.vector.tensor_tensor(out=sv[:, 1:2], in0=C["gnb"][:], in1=sv[:, 1:2],
                                    op=A.subtract)
            x2p = fr.tile([128, 1006], bf16, tag="x2p")
            nc.vector.memset(x2p[:, 0:3], 0.0)
            nc.vector.memset(x2p[:, 1003:1006], 0.0)
            nc.scalar.activation(out=x2p[:, 3:1003],
                                 in_=h1g.rearrange("p a b -> p (a b)"),
                                 func=AF.Identity, scale=sv[:, 0:1], bias=sv[:, 1:2])
            pc2 = pm2.tile([128, 2, 512], f32, tag="pm2")
            for mb in range(2):
                for k in range(7):
                    mm(out=pc2[:, mb, 0:500],
                       lhsT=C["w2t"][:, k, mb * 128:(mb + 1) * 128],
                       rhs=s2(x2p[:, k:k + 1000]),
                       start=(k == 0), stop=(k == 6))
            for mb in range(2):
                nc.scalar.activation(out=x[:, mb, cs:cs + 500], in_=pc2[:, mb, 0:500],
                                     func=AF.Gelu, bias=C["c2b"][:, mb:mb + 1])
            # CN layernorm + starbias + pos
            sbv = fr.tile([128, 2], f32, tag="sbv")
            nc.sync.dma_start(out=sbv[:], in_=P["starbias"][s])
            nc.scalar.activation(out=sqf[:], in_=x[:, :, cs:cs + 500], func=AF.Square)
            pstt = pst.tile([1, 2, 512], f32, tag="pst")
            for blk in range(2):
                mm(out=pstt[:1, 0, 0:500], lhsT=onesPf[:],
                   rhs=x[:, blk, cs:cs + 500], start=(blk == 0), stop=(blk == 1))
            for blk in range(2):
                mm(out=pstt[:1, 1, 0:500], lhsT=onesPf[:],
                   rhs=sqf[:, blk, :], start=(blk == 0), stop=(blk == 1))
            mr = sc.tile([1, 2, 500], f32, tag="mr")
            mrb = sc.tile([1, 2, 500], bf16, tag="mrb")
            tmp = sc.tile([1, 2, 500], f32, tag="tmp1")
            emit_stats(pstt, mr, mrb, tmp, 1.0 / 256.0)
            pbc = pm2.tile([128, 2, 512], f32, tag="pm2")
            for jj in range(2):
                mm(out=pbc[:, jj, 0:500], lhsT=ones1b[:], rhs=mrb[:1, jj, :],
                   start=True, stop=True)
            for blk in range(2):
                nc.vector.tensor_tensor(out=x[:, blk, cs:cs + 500],
                                        in0=x[:, blk, cs:cs + 500],
                                        in1=pbc[:, 0, 0:500], op=A.subtract)
                nc.vector.tensor_tensor(out=x[:, blk, cs:cs + 500],
                                        in0=x[:, blk, cs:cs + 500],
                                        in1=pbc[:, 1, 0:500], op=A.mult)
                nc.scalar.activation(out=x[:, blk, cs:cs + 500],
                                     in_=x[:, blk, cs:cs + 500], func=AF.Identity,
                                     scale=C["cng"][:, blk:blk + 1],
                                     bias=sbv[:, blk:blk + 1])
            nc.vector.tensor_tensor(out=x[:, :, cs:cs + 500], in0=x[:, :, cs:cs + 500],
                                    in1=C["posT"][:], op=A.add)

            # events
            evr = fr.tile([1, 3, 256], f32, tag="evr")
            nc.sync.dma_start(out=evr[:], in_=P["evrows"][s])
            tpv = fr.tile([128, 2], f32, tag="tpv")
            nc.sync.dma_start(out=tpv[:], in_=P["tposv"][s])
            kpv = fr.tile([128, 2], f32, tag="kpv")
            nc.sync.dma_start(out=kpv[:], in_=P["keepv"][s])
            comb = fr.tile([128, 6, 256], bf16, tag="comb")
            for vr in range(3):
                pb = pm1.tile([128, 500], f32, tag="pm1")
                mm(out=pb[:, 0:256], lhsT=ones1f[:], rhs=evr[:1, vr, :],
                   start=True, stop=True)
                arg = fr.tile([128, 256], f32, tag="arg")
                nc.scalar.activation(out=arg[:], in_=pb[:, 0:256], func=AF.Copy,
                                     scale=C["freqv"][:])
                nc.vector.tensor_scalar(out=arg[:], in0=arg[:], scalar1=INV2PI,
                                        scalar2=None, op0=A.mult)
                w1_ = fr.tile([128, 256], f32, tag="w1_")
                ti_ = fr.tile([128, 256], mybir.dt.int32, tag="ti_")
                tf_ = fr.tile([128, 256], f32, tag="tf_")
                nc.vector.tensor_copy(ti_[:], arg[:])
                nc.vector.tensor_copy(tf_[:], ti_[:])
                nc.vector.tensor_tensor(out=w1_[:], in0=arg[:], in1=tf_[:],
                                        op=A.subtract)
                nc.scalar.activation(out=comb[:, 2 * vr, :], in_=w1_[:], func=AF.Sin,
                                     scale=TWOPI, bias=zerov[:])
                nc.vector.tensor_scalar(out=arg[:], in0=arg[:], scalar1=0.25,
                                        scalar2=None, op0=A.add)
                nc.vector.tensor_copy(ti_[:], arg[:])
                nc.vector.tensor_copy(tf_[:], ti_[:])
                nc.vector.tensor_tensor(out=w1_[:], in0=arg[:], in1=tf_[:],
                                        op=A.subtract)
                nc.scalar.activation(out=comb[:, 2 * vr + 1, :], in_=w1_[:],
                                     func=AF.Sin, scale=TWOPI, bias=zerov[:])
            hmid = fr.tile([128, 2, 256], bf16, tag="hmid")
            for mb in range(2):
                ph = pm1.tile([128, 500], f32, tag="pm1")
                for kc in range(6):
                    mm(out=ph[:, 0:256],
                       lhsT=C["epW1T"][:, kc, mb * 128:(mb + 1) * 128],
                       rhs=comb[:, kc, :], start=(kc == 0), stop=(kc == 5))
                nc.scalar.activation(out=hmid[:, mb, :], in_=ph[:, 0:256],
                                     func=AF.Gelu, bias=C["epb1"][:, mb:mb + 1])
            evt = fr.tile([128, 2, 256], bf16, tag="evt")
            for ec in range(2):
                pe = pm1.tile([128, 500], f32, tag="pm1")
                for kc in range(2):
                    mm(out=pe[:, 0:256],
                       lhsT=hmid[:, kc, ec * 128:(ec + 1) * 128],
                       rhs=C["epW2T"][:, kc, :], start=(kc == 0), stop=(kc == 1))
                nc.vector.tensor_tensor(out=evt[:, ec, :], in0=pe[:, 0:256],
                                        in1=C["epb2row"][:], op=A.add)
                nc.vector.tensor_scalar(out=evt[:, ec, :], in0=evt[:, ec, :],
                                        scalar1=kpv[:, ec:ec + 1], scalar2=None,
                                        op0=A.mult)
            oh = fr.tile([128, 2, 500], bf16, tag="oh")
            for ec in range(2):
                nc.vector.tensor_scalar(out=oh[:, ec, :], in0=iotaB[:],
                                        scalar1=tpv[:, ec:ec + 1], scalar2=None,
                                        op0=A.is_equal)
            for mb in range(2):
                px = pm1.tile([128, 500], f32, tag="pm1")
                for ec in range(2):
                    mm(out=px[:], lhsT=evt[:, ec, mb * 128:(mb + 1) * 128],
                       rhs=oh[:, ec, :], start=(ec == 0), stop=(ec == 1))
                nc.vector.tensor_tensor(out=x[:, mb, cs:cs + 500],
                                        in0=x[:, mb, cs:cs + 500], in1=px[:], op=A.add)
        fr.release()
        wpool = tc.alloc_tile_pool(name="wpool", bufs=2)
        big = tc.alloc_tile_pool(name="big", bufs=1)
        scr = tc.alloc_tile_pool(name="scr", bufs=1)
        scr2 = tc.alloc_tile_pool(name="scr2", bufs=1)

        # ================= transformer =================
        for i in range(nl_run):
            W = {}
            for k, sh, dt in [("wqkvT", [128, 2, 768], bf16), ("bqkv", [128, 6], f32),
                              ("woT", [128, 2, 256], bf16), ("bov", [128, 2], f32),
                              ("ln1g", [128, 2], f32), ("ln1b", [128, 2], f32),
                              ("ln2g", [128, 2], f32), ("ln2b", [128, 2], f32),
                              ("w1TT", [128, 2, 1024], bf16), ("b1v", [128, 8], f32),
                              ("w2TT", [128, 8, 256], bf16), ("b2v", [128, 2], f32)]:
                W[k] = wpool.tile(sh, dt, tag="w_" + k, name=f"w{i}_" + k)
                nc.sync.dma_start(out=W[k][:], in_=P[k][i])

            def emit_ln(gk, bk, xn):
                for nch in range(4):
                    co = nch * 500
                    sq = scr2.tile([128, 2, 500], f32, tag="sq")
                    nc.scalar.activation(out=sq[:], in_=x[:, :, co:co + 500],
                                         func=AF.Square)
                    pstt = pst.tile([1, 2, 512], f32, tag="pst")
                    for blk in range(2):
                        mm(out=pstt[:1, 0, 0:500], lhsT=onesPf[:],
                           rhs=x[:, blk, co:co + 500],
                           start=(blk == 0), stop=(blk == 1))
                    for blk in range(2):
                        mm(out=pstt[:1, 1, 0:500], lhsT=onesPf[:],
                           rhs=sq[:, blk, :], start=(blk == 0), stop=(blk == 1))
                    mr = sc.tile([1, 2, 500], f32, tag="mr")
                    mrb = sc.tile([1, 2, 500], bf16, tag="mrb")
                    tmp = sc.tile([1, 2, 500], f32, tag="tmp1")
                    emit_stats(pstt, mr, mrb, tmp, 1.0 / 256.0)
                    pbc = pm2.tile([128, 2, 512], f32, tag="pm2")
                    for jj in range(2):
                        mm(out=pbc[:, jj, 0:500], lhsT=ones1b[:],
                           rhs=mrb[:1, jj, :], start=True, stop=True)
                    for blk in range(2):
                        nc.vector.tensor_tensor(out=xn[:, blk, co:co + 500],
                                                in0=x[:, blk, co:co + 500],
                                                in1=pbc[:, 0, 0:500], op=A.subtract)
                        nc.vector.tensor_tensor(out=xn[:, blk, co:co + 500],
                                                in0=xn[:, blk, co:co + 500],
                                                in1=pbc[:, 1, 0:500], op=A.mult)
                        nc.scalar.activation(out=xn[:, blk, co:co + 500],
                                             in_=xn[:, blk, co:co + 500],
                                             func=AF.Identity,
                                             scale=W[gk][:, blk:blk + 1],
                                             bias=W[bk][:, blk:blk + 1])

            xn = big.tile([128, 2, 2000], bf16, tag="xn")
            emit_ln("ln1g", "ln1b", xn)
            attn = big.tile([128, 2, 2000], bf16, tag="attn")
            for s in range(ns_run):
                cs = s * 500
                qkv = scr.tile([128, 6, 500], bf16, tag="qkv")
                for j in range(6):
                    pq = pm1.tile([128, 500], f32, tag="pm1")
                    for kc in range(2):
                        mm(out=pq[:], lhsT=W["wqkvT"][:, kc, j * 128:(j + 1) * 128],
                           rhs=xn[:, kc, cs:cs + 500],
                           start=(kc == 0), stop=(kc == 1))
                    nc.vector.tensor_scalar(out=qkv[:, j, :], in0=pq[:],
                                            scalar1=W["bqkv"][:, j:j + 1],
                                            scalar2=None, op0=A.add)
                for j in range(2):
                    vt = scr2.tile([128, 4, 128], bf16, tag="vt")
                    pv4 = pmt.tile([128, 4, 128], bf16, tag="pvt")
                    for skc in range(4):
                        nc.tensor.transpose(
                            out=pv4[:125, skc, :],
                            in_=qkv[:, 4 + j, skc * 125: skc * 125 + 125],
                            identity=identb[:])
                    nc.scalar.activation(out=vt[:125, :, :],
                                         in_=pv4[:125, :, :], func=AF.Copy)
                    pot4 = pm1.tile([128, 500], f32, tag="pm1")
                    pcs4 = pst.tile([4, 512], f32, tag="pcs4")
                    for c4 in range(4):
                        poff = 32 * c4
                        h_q = qkv[poff:poff + 32, j, :]
                        h_k = qkv[poff:poff + 32, 2 + j, :]
                        eT = scr.tile([128, 4, 500], bf16, tag="eT")
                        for pair in range(2):
                            psc = pm2.tile([128, 2, 512], f32, tag="pm2")
                            for sub in range(2):
                                skc = 2 * pair + sub
                                mm(out=psc[:125, sub, 0:500],
                                   lhsT=h_k[:, skc * 125: skc * 125 + 125],
                                   rhs=h_q, start=True, stop=True,
                                   tile_position=(poff, 0))
                            nc.scalar.activation(
                                out=eT[:125, 2 * pair: 2 * pair + 2, :],
                                in_=psc[:125, :, 0:500], func=AF.Exp, scale=SC32)
                        for skc in range(4):
                            mm(out=pcs4[0:4, 0:500], lhsT=C["onesZc"][:125, c4, :],
                               rhs=eT[:125, skc, :],
                               start=(c4 == 0 and skc == 0),
                               stop=(c4 == 3 and skc == 3))
                        for skc in range(4):
                            mm(out=pot4[poff:poff + 32, :],
                               lhsT=vt[:125, skc, poff:poff + 32],
                               rhs=eT[:125, skc, :],
                               start=(skc == 0), stop=(skc == 3),
                               tile_position=(0, poff))
                    rrf = sc.tile([4, 500], f32, tag="rrf")
                    nc.vector.reciprocal_approx_fast(out=rrf[:],
                                                     in_=pcs4[0:4, 0:500])
                    rrb4 = sc.tile([4, 500], bf16, tag="rrb4")
                    nc.vector.tensor_copy(rrb4[:], rrf[:])
                    prbF = pm2.tile([128, 2, 512], f32, tag="pm2")
                    mm(out=prbF[:, 0, 0:500], lhsT=C["e4m"][:], rhs=rrb4[:],
                       start=True, stop=True)
                    rbsF = scr2.tile([128, 500], bf16, tag="rbs")
                    nc.scalar.activation(out=rbsF[:], in_=prbF[:, 0, 0:500],
                                         func=AF.Copy)
                    nc.vector.tensor_tensor(out=attn[:, j, cs:cs + 500],
                                            in0=pot4[:], in1=rbsF[:], op=A.mult)
            for mb in range(2):
                for nch in range(4):
                    po = pm1.tile([128, 500], f32, tag="pm1")
                    for kc in range(2):
                        mm(out=po[:], lhsT=W["woT"][:, kc, mb * 128:(mb + 1) * 128],
                           rhs=attn[:, kc, nch * 500:(nch + 1) * 500],
                           start=(kc == 0), stop=(kc == 1))
                    nc.vector.tensor_tensor(out=x[:, mb, nch * 500:(nch + 1) * 500],
                                            in0=x[:, mb, nch * 500:(nch + 1) * 500],
                                            in1=po[:], op=A.add)
                nc.vector.tensor_scalar(out=x[:, mb, :], in0=x[:, mb, :],
                                        scalar1=W["bov"][:, mb:mb + 1], scalar2=None,
                                        op0=A.add)
            xn2 = big.tile([128, 2, 2000], bf16, tag="xn")
            emit_ln("ln2g", "ln2b", xn2)
            for s in range(ns_run):
                cs = s * 500
                fh = scr.tile([128, 8, 500], bf16, tag="fh")
                for hb in range(8):
                    phh = pm1.tile([128, 500], f32, tag="pm1")
                    for kc in range(2):
                        mm(out=phh[:], lhsT=W["w1TT"][:, kc, hb * 128:(hb + 1) * 128],
                           rhs=xn2[:, kc, cs:cs + 500], start=(kc == 0),
                           stop=(kc == 1))
                    nc.scalar.activation(out=fh[:, hb, :], in_=phh[:], func=AF.Gelu,
                                         bias=W["b1v"][:, hb:hb + 1])
                for mb in range(2):
                    pf = pm1.tile([128, 500], f32, tag="pm1")
                    for hb in range(8):
                        mm(out=pf[:], lhsT=W["w2TT"][:, hb, mb * 128:(mb + 1) * 128],
                           rhs=fh[:, hb, :], start=(hb == 0), stop=(hb == 7))
                    nc.vector.tensor_tensor(out=x[:, mb, cs:cs + 500],
                                            in0=x[:, mb, cs:cs + 500], in1=pf[:],
                                            op=A.add)
            for mb in range(2):
                nc.vector.tensor_scalar(out=x[:, mb, :], in0=x[:, mb, :],
                                        scalar1=W["b2v"][:, mb:mb + 1], scalar2=None,
                                        op0=A.add)

        # ================= pooling + head =================
        for s in range(ns_run):
            cs = s * 500
            plg = pst.tile([1, 2, 512], f32, tag="pst")
            for blk in range(2):
                mm(out=plg[:1, 0, 0:500], lhsT=C["poolq"][:, blk:blk + 1],
                   rhs=x[:, blk, cs:cs + 500], start=(blk == 0), stop=(blk == 1))
            wrow = sc.tile([1, 500], f32, tag="wrow")
            nc.scalar.activation(out=wrow[:], in_=plg[:1, 0, 0:500], func=AF.Exp,
                                 scale=1.0 / 16.0)
            pwb = pm1.tile([128, 500], f32, tag="pm1")
            mm(out=pwb[:], lhsT=ones1f[:], rhs=wrow[:], start=True, stop=True)
            wx = scr2.tile([128, 2, 500], f32, tag="sq")
            for blk in range(2):
                nc.vector.tensor_tensor(out=wx[:, blk, :], in0=x[:, blk, cs:cs + 500],
                                        in1=pwb[:], op=A.mult)
            pooled = sc.tile([128, 4], f32, tag="pooled")
            nc.vector.tensor_reduce(out=pooled[:, 0:2], in_=wx[:], axis=AX.X,
                                    op=A.add)
            nc.scalar.activation(out=pooled[:, 2:4], in_=pooled[:, 0:2],
                                 func=AF.Square)
            pps = pst.tile([1, 2, 512], f32, tag="pst")
            mm(out=pps[:1, 0, 0:4], lhsT=onesPf[:], rhs=pooled[:], start=True,
               stop=True)
            z = sc.tile([1, 16], f32, tag="z")
            nc.vector.tensor_copy(z[:, 12:16], pps[:1, 0, 0:4])
            nc.vector.tensor_tensor(out=z[:, 0:1], in0=z[:, 12:13],
                                    in1=z[:, 13:14], op=A.add)
            nc.vector.tensor_tensor(out=z[:, 1:2], in0=z[:, 14:15],
                                    in1=z[:, 15:16], op=A.add)
            nc.vector.tensor_scalar(out=z[:, 2:3], in0=z[:, 0:1],
                                    scalar1=1.0 / 256.0, scalar2=None, op0=A.mult)
            nc.vector.tensor_tensor(out=z[:, 3:4], in0=z[:, 2:3], in1=z[:, 2:3],
                                    op=A.mult)
            nc.vector.tensor_scalar(out=z[:, 4:5], in0=z[:, 1:2],
                                    scalar1=1.0 / 256.0, scalar2=None, op0=A.mult)
            nc.vector.tensor_tensor(out=z[:, 4:5], in0=z[:, 4:5], in1=z[:, 3:4],
                                    op=A.subtract)
            nc.scalar.activation(out=z[:, 5:6], in_=z[:, 4:5], func=AF.Sqrt,
                                 bias=epsv[0:1, :])
            nc.vector.reciprocal_approx_fast(out=z[:, 6:7], in_=z[:, 5:6])
            cp = sc.tile([128, 2], f32, tag="cp")
            nc.vector.tensor_tensor(out=cp[:], in0=pooled[:, 0:2], in1=C["c1v"][:],
                                    op=A.mult)
            pa = pst.tile([1, 2, 512], f32, tag="pst")
            mm(out=pa[:1, 0, 0:2], lhsT=onesPf[:], rhs=cp[:], start=True, stop=True)
            nc.vector.tensor_copy(z[:, 10:12], pa[:1, 0, 0:2])
            nc.vector.tensor_tensor(out=z[:, 7:8], in0=z[:, 10:11],
                                    in1=z[:, 11:12], op=A.add)
            nc.vector.tensor_scalar(out=z[:, 8:9], in0=z[:, 2:3], scalar1=sc1,
                                    scalar2=None, op0=A.mult)
            nc.vector.tensor_tensor(out=z[:, 8:9], in0=z[:, 7:8], in1=z[:, 8:9],
                                    op=A.subtract)
            nc.vector.tensor_tensor(out=z[:, 8:9], in0=z[:, 8:9], in1=z[:, 6:7],
                                    op=A.mult)
            nc.vector.tensor_scalar(out=z[:, 9:10], in0=z[:, 8:9], scalar1=c2s,
                                    scalar2=None, op0=A.add)
            nc.sync.dma_start(out=Y[s:s + 1, :], in_=z[:, 9:10])

        for p in [pmt, pst, pm2, scr2, scr, big, wpool, pm1, bigx, sc, sing]:
            p.release()

    nc.compile()
    return nc


_BUILT = {}


def _get_nc(key, **kw):
    if key not in _BUILT:
        _BUILT[key] = _build(**kw)
    return _BUILT[key]


def _make_in_maps(prep, ns=NS, ncores=NCORES):
    in_maps = []
    for c in range(ncores):
        sl = slice(c * ns, (c + 1) * ns)
        m = {k: prep[k] for k in CONST_KEYS}
        m["mel4"] = prep["_mel"][sl]
        m["evrows"] = prep["_evrows"][sl]
        m["tposv"] = prep["_tposv"][sl]
        m["keepv"] = prep["_keepv"][sl]
        m["starbias"] = prep["_starbias"][sl]
        in_maps.append(m)
    return in_maps


def kernel(**inputs):
    from concourse.bass_utils import run_bass_kernel_spmd

    prep = _host_prep(inputs)
    nc = _get_nc("full", nl_run=NLAYERS, ns_run=NS, debug=False,
                 sc1=prep["sc1"], c2s=prep["c2s"])
    res = run_bass_kernel_spmd(nc, _make_in_maps(prep), list(range(NCORES)))
    y = np.concatenate([res.results[c]["y"].reshape(-1) for c in range(NCORES)])
    return y.astype(np.float32)


# revision 14
# speedup vs baseline: 1.0959x; 1.0721x over previous
"""Trainium2 Bass kernel for nn_ChartQualityEvaluator.

Data parallel: 32 samples -> 8 cores x 4 samples. Feature-major activations
[128 part, 2 blocks, 4*500 cols]. Matmuls and bulk activations in bf16
(fp32 PSUM accumulation); stats/sinusoid paths stay fp32. Unsafe softmax,
LN via ones-matmul stats + PE broadcast, event scatter via onehot matmul.
Host precomputes index-like preprocessing with f32-exact semantics.
"""
import math
import sys

import numpy as np
import ml_dtypes

_TRN = "/opt/trn_rl_repo"
if _TRN not in sys.path:
    sys.path.insert(0, _TRN)

BF16 = ml_dtypes.bfloat16

D = 256
H = 8
NLAYERS = 6
HALF = 128
S = 500
NEV = 256
NCORES = 8
NS = 4  # samples per core
B = 32
EPS = 1e-5
INV2PI = float(np.float32(1.0 / (2.0 * math.pi)))
TWOPI = 2.0 * math.pi
SC32 = float(np.float32(1.0 / math.sqrt(32.0)))


def _host_prep(inp):
    f = np.float32
    out = {}

    def t2(v):  # [256] -> [128,2]
        return np.ascontiguousarray(np.asarray(v).reshape(2, 128).T.astype(f))

    def b(a):  # -> bf16
        return np.ascontiguousarray(np.asarray(a).astype(np.float32).astype(BF16))

    out["wmelT"] = b(np.asarray(inp["mel_W"]).T)
    out["melb"] = np.ascontiguousarray(np.asarray(inp["mel_b"]).reshape(16, 1).astype(f))
    out["w1t"] = b(np.asarray(inp["conv1_w"]).transpose(1, 2, 0))
    out["c1b"] = np.ascontiguousarray(np.asarray(inp["conv1_b"]).reshape(128, 1).astype(f))
    out["gng"] = np.ascontiguousarray(np.asarray(inp["gn_g"]).reshape(128, 1).astype(f))
    out["gnb"] = np.ascontiguousarray(np.asarray(inp["gn_b"]).reshape(128, 1).astype(f))
    out["w2t"] = b(np.asarray(inp["conv2_w"]).transpose(1, 2, 0))
    out["c2b"] = t2(inp["conv2_b"])
    out["cng"] = t2(inp["cn_g"])

    freq = np.exp(np.arange(HALF, dtype=f) * f(-math.log(10000.0) / (HALF - 1)))
    e32 = (np.arange(S, dtype=f)[None, :] * freq[:, None]).astype(f)
    e64 = e32.astype(np.float64)
    pos_fm = np.concatenate([np.sin(e64), np.cos(e64)], axis=0)  # [256,500]
    out["posT"] = np.ascontiguousarray(
        pos_fm.reshape(2, 128, S).transpose(1, 0, 2).astype(f))
    out["freqv"] = np.ascontiguousarray(freq.reshape(128, 1))

    out["epW1T"] = b(np.asarray(inp["ep_W1"]).T.reshape(6, 128, 256).transpose(1, 0, 2))
    out["epb1"] = t2(inp["ep_b1"])
    out["epW2T"] = b(np.asarray(inp["ep_W2"]).T.reshape(2, 128, 256).transpose(1, 0, 2))
    out["epb2row"] = np.ascontiguousarray(
        np.tile(np.asarray(inp["ep_b2"]).astype(f)[None, :], (128, 1)))

    def wT(w, kc, m):  # w [m, k] -> [128, kc, m] bf16
        return b(np.asarray(w).T.reshape(kc, 128, m).transpose(1, 0, 2))

    out["wqkvT"] = np.stack([wT(inp["tl_Wqkv"][i], 2, 768) for i in range(NLAYERS)])
    out["bqkv"] = np.stack([np.ascontiguousarray(
        np.asarray(inp["tl_bqkv"][i]).reshape(6, 128).T.astype(f))
        for i in range(NLAYERS)])
    out["woT"] = np.stack([wT(inp["tl_Wo"][i], 2, 256) for i in range(NLAYERS)])
    out["bov"] = np.stack([t2(inp["tl_bo"][i]) for i in range(NLAYERS)])
    out["ln1g"] = np.stack([t2(inp["tl_ln1g"][i]) for i in range(NLAYERS)])
    out["ln1b"] = np.stack([t2(inp["tl_ln1b"][i]) for i in range(NLAYERS)])
    out["ln2g"] = np.stack([t2(inp["tl_ln2g"][i]) for i in range(NLAYERS)])
    out["ln2b"] = np.stack([t2(inp["tl_ln2b"][i]) for i in range(NLAYERS)])
    out["w1TT"] = np.stack([wT(inp["tl_W1"][i], 2, 1024) for i in range(NLAYERS)])
    out["b1v"] = np.stack([np.ascontiguousarray(
        np.asarray(inp["tl_b1"][i]).reshape(8, 128).T.astype(f))
        for i in range(NLAYERS)])
    out["w2TT"] = np.stack([wT(inp["tl_W2"][i], 8, 256) for i in range(NLAYERS)])
    out["b2v"] = np.stack([t2(inp["tl_b2"][i]) for i in range(NLAYERS)])

    out["poolq"] = t2(inp["pool_q"])
    oz = np.zeros((128, 4, 4), np.float32)
    for c4 in range(4):
        oz[:, c4, c4] = 1.0
    out["onesZc"] = b(oz)
    e4 = np.zeros((4, 128), np.float32)
    for c4 in range(4):
        e4[c4, 32 * c4:32 * c4 + 32] = 1.0
    out["e4m"] = b(e4)
    c1 = (np.asarray(inp["oh_W"])[0] * np.asarray(inp["on_g"])).astype(f)
    out["c1v"] = t2(c1)
    out["sc1"] = float(c1.astype(np.float64).sum())
    out["c2s"] = float((np.asarray(inp["oh_W"])[0].astype(np.float64)
                        * np.asarray(inp["on_b"]).astype(np.float64)).sum()
                       + float(np.asarray(inp["oh_b"])[0]))

    events = np.asarray(inp["events"]).astype(np.int64)
    mask = np.asarray(inp["event_mask"])
    star = np.asarray(inp["star_rating"]).astype(f)
    nb = events.shape[0]
    diff = np.maximum(events[:, 1:] - events[:, :-1], 1)
    g = np.concatenate([diff[:, :1], diff], axis=1)
    gap_ms = (g * 5).astype(f)
    g_f = np.maximum(g.astype(f), f(1.0))
    r = np.clip(g_f[:, 1:] / g_f[:, :-1], f(0.1), f(10.0)).astype(f)
    ones = np.ones((nb, 1), f)
    rb50 = np.trunc(np.concatenate([ones, r], axis=1) * f(50.0)).astype(f)
    ra50 = np.trunc(np.concatenate([r, ones], axis=1) * f(50.0)).astype(f)
    out["_evrows"] = np.ascontiguousarray(np.stack([rb50, ra50, gap_ms], axis=1))
    tp = np.clip(events // 4, 0, S - 1).astype(f)
    keep = (1.0 - mask.astype(f)).astype(f)
    out["_tposv"] = np.ascontiguousarray(tp.reshape(nb, 2, 128).transpose(0, 2, 1))
    out["_keepv"] = np.ascontiguousarray(keep.reshape(nb, 2, 128).transpose(0, 2, 1))
    bucket = np.clip((star / f(0.5)).astype(np.int32), 0, 19)
    sb = (np.asarray(inp["cn_b"])[None, :] + np.asarray(inp["star_table"])[bucket]).astype(f)
    out["_starbias"] = np.ascontiguousarray(sb.reshape(nb, 2, 128).transpose(0, 2, 1))
    out["_mel"] = np.ascontiguousarray(np.asarray(inp["mel"]).astype(f).astype(BF16))
    return out


CONST_KEYS = ["wmelT", "melb", "w1t", "c1b", "gng", "gnb", "w2t", "c2b", "cng",
              "posT", "freqv", "epW1T", "epb1", "epW2T", "epb2row",
              "wqkvT", "bqkv", "woT", "bov", "ln1g", "ln1b", "ln2g", "ln2b",
              "w1TT", "b1v", "w2TT", "b2v", "poolq", "c1v", "onesZc", "e4m"]

# params that are bf16 on device
BF_KEYS = {"wmelT", "w1t", "w2t", "epW1T", "epW2T",
           "wqkvT", "woT", "w1TT", "w2TT", "mel4", "onesZc", "e4m"}


def _build(nl_run=NLAYERS, ns_run=NS, debug=False, sc1=0.0, c2s=0.0):
    import concourse.bacc as bacc
    import concourse.tile as tile
    from concourse import mybir
    from concourse.masks import make_identity

    f32 = mybir.dt.float32
    bf16 = mybir.dt.bfloat16
    A = mybir.AluOpType
    AF = mybir.ActivationFunctionType
    AX = mybir.AxisListType

    nc = bacc.Bacc(None)

    def mm(out, lhsT, rhs, **kw):
        nc.tensor.matmul(out=out, lhsT=lhsT, rhs=rhs, **kw)

    P = {}
    shapes = dict(
        mel4=[ns_run, 80, 2000], evrows=[ns_run, 3, 256], tposv=[ns_run, 128, 2],
        keepv=[ns_run, 128, 2], starbias=[ns_run, 128, 2],
        wmelT=[80, 16], melb=[16, 1], w1t=[16, 7, 128], c1b=[128, 1],
        gng=[128, 1], gnb=[128, 1], w2t=[128, 7, 256], c2b=[128, 2],
        cng=[128, 2], posT=[128, 2, 500], freqv=[128, 1],
        epW1T=[128, 6, 256], epb1=[128, 2], epW2T=[128, 2, 256], epb2row=[128, 256],
        wqkvT=[NLAYERS, 128, 2, 768], bqkv=[NLAYERS, 128, 6],
        woT=[NLAYERS, 128, 2, 256], bov=[NLAYERS, 128, 2],
        ln1g=[NLAYERS, 128, 2], ln1b=[NLAYERS, 128, 2],
        ln2g=[NLAYERS, 128, 2], ln2b=[NLAYERS, 128, 2],
        w1TT=[NLAYERS, 128, 2, 1024], b1v=[NLAYERS, 128, 8],
        w2TT=[NLAYERS, 128, 8, 256], b2v=[NLAYERS, 128, 2],
        poolq=[128, 2], c1v=[128, 2], onesZc=[128, 4, 4], e4m=[4, 128],
    )
    for k, sh in shapes.items():
        P[k] = nc.declare_dram_parameter(k, sh, bf16 if k in BF_KEYS else f32,
                                         isOutput=False)
    Y = nc.declare_dram_parameter("y", [ns_run, 1], f32, isOutput=True)

    with tile.TileContext(nc) as tc:
        sing = tc.alloc_tile_pool(name="sing", bufs=1)
        sc = tc.alloc_tile_pool(name="sc", bufs=1)
        bigx = tc.alloc_tile_pool(name="bigx", bufs=1)
        pm1 = tc.alloc_tile_pool(name="pm1", bufs=2, space="PSUM")
        pm2 = tc.alloc_tile_pool(name="pm2", bufs=2, space="PSUM")
        pst = tc.alloc_tile_pool(name="pst", bufs=1, space="PSUM")
        pmt = tc.alloc_tile_pool(name="pmt", bufs=1, space="PSUM")

        C = {}
        for k in ["wmelT", "melb", "w1t", "c1b", "gng", "gnb", "w2t", "c2b",
                  "cng", "posT", "freqv", "epW1T", "epb1", "epW2T", "epb2row",
                  "poolq", "c1v", "onesZc", "e4m"]:
            C[k] = sing.tile(shapes[k], bf16 if k in BF_KEYS else f32,
                             tag=k, name="c_" + k)
            nc.sync.dma_start(out=C[k][:], in_=P[k][:])
        identb = sing.tile([128, 128], bf16, tag="identb")
        make_identity(nc, identb[:])
        onesPf = sing.tile([128, 1], f32, tag="onesPf")
        nc.vector.memset(onesPf[:], 1.0)
        onesPb = sing.tile([128, 1], bf16, tag="onesPb")
        nc.vector.memset(onesPb[:], 1.0)
        ones1f = sing.tile([1, 128], f32, tag="ones1f")
        nc.vector.memset(ones1f[:], 1.0)
        ones1b = sing.tile([1, 128], bf16, tag="ones1b")
        nc.vector.memset(ones1b[:], 1.0)
        zerov = sing.tile([128, 1], f32, tag="zerov")
        nc.vector.memset(zerov[:], 0.0)
        epsv = sing.tile([128, 1], f32, tag="epsv")
        nc.vector.memset(epsv[:], EPS)
        iotaB = sing.tile([128, 500], f32, tag="iotaB")
        nc.gpsimd.iota(iotaB[:], pattern=[[1, 500]], base=0, channel_multiplier=0,
                       allow_small_or_imprecise_dtypes=True)

        x = bigx.tile([128, 2, 2000], f32, tag="x_fm")

        def s2(t):  # step-2 view of [p, n] -> [p, n//2]
            return t.rearrange("p (t s) -> p s t", s=2)[:, 0, :]

        # small-vector LN stat helper: psum [1,2,500] (s,ss) -> mr (m, r); also
        # writes mrb (bf16 copy of [m, r]) for cheap broadcast matmuls.
        def emit_stats(pstt, mr, mrb, tmp, scale):
            nc.vector.tensor_scalar(out=mr[:1, 0:2, :], in0=pstt[:1, 0:2, 0:500],
                                    scalar1=scale, scalar2=None, op0=A.mult)
            nc.vector.tensor_tensor(out=tmp[:1, 0, :], in0=mr[:1, 0, :],
                                    in1=mr[:1, 0, :], op=A.mult)
            nc.vector.tensor_tensor(out=tmp[:1, 1, :], in0=mr[:1, 1, :],
                                    in1=tmp[:1, 0, :], op=A.subtract)
            nc.scalar.activation(out=tmp[:1, 0, :], in_=tmp[:1, 1, :], func=AF.Sqrt,
                                 bias=epsv[0:1, :])
            nc.vector.reciprocal_approx_fast(out=mr[:1, 1, :], in_=tmp[:1, 0, :])
            nc.vector.tensor_copy(mrb[:1, 0:2, :], mr[:1, 0:2, :])

        # ================= front end =================
        fr = tc.alloc_tile_pool(name="fr", bufs=2)
        for s in range(ns_run):
            cs = s * 500
            melp = fr.tile([80, 2006], bf16, tag="melp")
            nc.vector.memset(melp[:, 0:3], 0.0)
            nc.vector.memset(melp[:, 2003:2006], 0.0)
            nc.sync.dma_start(out=melp[:, 3:2003], in_=P["mel4"][s])
            xmelp = fr.tile([16, 2006], bf16, tag="xmelp")
            nc.vector.memset(xmelp[:, 0:3], 0.0)
            nc.vector.memset(xmelp[:, 2003:2006], 0.0)
            for nch in range(4):
                pcm = pm1.tile([128, 500], f32, tag="pm1")
                mm(out=pcm[:16, :], lhsT=C["wmelT"][:],
                   rhs=melp[:, 3 + nch * 500: 3 + nch * 500 + 500],
                   start=True, stop=True)
                nc.scalar.activation(out=xmelp[:, 3 + nch * 500: 3 + nch * 500 + 500],
                                     in_=pcm[:16, :], func=AF.Identity,
                                     bias=C["melb"][:, 0:1])
            h1g = fr.tile([128, 2, 500], bf16, tag="h1g")
            stg = fr.tile([128, 4], f32, tag="stg")
            for half in range(2):
                pc1 = pm2.tile([128, 512], f32, tag="pm2")
                for k in range(7):
                    mm(out=pc1[:, 0:500], lhsT=C["w1t"][:, k, :],
                       rhs=s2(xmelp[:, k + half * 1000: k + half * 1000 + 1000]),
                       start=(k == 0), stop=(k == 6))
                nc.scalar.activation(out=h1g[:, half, :], in_=pc1[:, 0:500],
                                     func=AF.Gelu, bias=C["c1b"][:, 0:1],
                                     accum_out=stg[:, half:half + 1])
            sqf = fr.tile([128, 2, 500], f32, tag="sqf")
            nc.scalar.activation(out=sqf[:], in_=h1g[:], func=AF.Square,
                                 accum_out=stg[:, 2:3])
            pg = pst.tile([1, 2, 512], f32, tag="pst")
            mm(out=pg[:1, 0, 0:3], lhsT=onesPf[:], rhs=stg[:, 0:3], start=True,
               stop=True)
            sn = sc.tile([1, 8], f32, tag="sn")
            nc.vector.tensor_scalar(out=sn[:, 0:2], in0=pg[:1, 0, 1:3],
                                    scalar1=1.0 / 128000.0, scalar2=None, op0=A.mult)
            nc.vector.tensor_scalar(out=sn[:, 6:7], in0=pg[:1, 0, 0:1],
                                    scalar1=1.0 / 128000.0, scalar2=None, op0=A.mult)
            nc.vector.tensor_tensor(out=sn[:, 0:1], in0=sn[:, 0:1], in1=sn[:, 6:7],
                                    op=A.add)
            nc.vector.tensor_tensor(out=sn[:, 2:3], in0=sn[:, 0:1], in1=sn[:, 0:1],
                                    op=A.mult)
            nc.vector.tensor_tensor(out=sn[:, 3:4], in0=sn[:, 1:2], in1=sn[:, 2:3],
                                    op=A.subtract)
            nc.scalar.activation(out=sn[:, 4:5], in_=sn[:, 3:4], func=AF.Sqrt,
                                 bias=epsv[0:1, :])
            nc.vector.reciprocal_approx_fast(out=sn[:, 1:2], in_=sn[:, 4:5])
            pgb = pm1.tile([128, 500], f32, tag="pm1")
            mm(out=pgb[:, 0:2], lhsT=ones1f[:], rhs=sn[:, 0:2], start=True, stop=True)
            sv = sc.tile([128, 2], f32, tag="sv")
            nc.vector.tensor_tensor(out=sv[:, 0:1], in0=pgb[:, 1:2], in1=C["gng"][:],
                                    op=A.mult)
            nc.vector.tensor_tensor(out=sv[:, 1:2], in0=pgb[:, 0:1], in1=sv[:, 0:1],
                                    op=A.mult)
            nc.vector.tensor_tensor(out=sv[:, 1:2], in0=C["gnb"][:], in1=sv[:, 1:2],
                                    op=A.subtract)
            x2p = fr.tile([128, 1006], bf16, tag="x2p")
            nc.vector.memset(x2p[:, 0:3], 0.0)
            nc.vector.memset(x2p[:, 1003:1006], 0.0)
            nc.scalar.activation(out=x2p[:, 3:1003],
                                 in_=h1g.rearrange("p a b -> p (a b)"),
                                 func=AF.Identity, scale=sv[:, 0:1], bias=sv[:, 1:2])
            for mb in range(2):
                pc2 = pm2.tile([128, 512], f32, tag="pm2")
                for k in range(7):
                    mm(out=pc2[:, 0:500],
                       lhsT=C["w2t"][:, k, mb * 128:(mb + 1) * 128],
                       rhs=s2(x2p[:, k:k + 1000]),
                       start=(k == 0), stop=(k == 6))
                nc.scalar.activation(out=x[:, mb, cs:cs + 500], in_=pc2[:, 0:500],
                                     func=AF.Gelu, bias=C["c2b"][:, mb:mb + 1])
            # CN layernorm + starbias + pos
            sbv = fr.tile([128, 2], f32, tag="sbv")
            nc.sync.dma_start(out=sbv[:], in_=P["starbias"][s])
            nc.scalar.activation(out=sqf[:], in_=x[:, :, cs:cs + 500], func=AF.Square)
            pstt = pst.tile([1, 2, 512], f32, tag="pst")
            for blk in range(2):
                mm(out=pstt[:1, 0, 0:500], lhsT=onesPf[:],
                   rhs=x[:, blk, cs:cs + 500], start=(blk == 0), stop=(blk == 1))
            for blk in range(2):
                mm(out=pstt[:1, 1, 0:500], lhsT=onesPf[:],
                   rhs=sqf[:, blk, :], start=(blk == 0), stop=(blk == 1))
            mr = sc.tile([1, 2, 500], f32, tag="mr")
            mrb = sc.tile([1, 2, 500], bf16, tag="mrb")
            tmp = sc.tile([1, 2, 500], f32, tag="tmp1")
            emit_stats(pstt, mr, mrb, tmp, 1.0 / 256.0)
            pbcM = pm2.tile([128, 512], f32, tag="pm2")
            mm(out=pbcM[:, 0:500], lhsT=ones1b[:], rhs=mrb[:1, 0, :],
               start=True, stop=True)
            pbcR = pm2.tile([128, 512], f32, tag="pm2")
            mm(out=pbcR[:, 0:500], lhsT=ones1b[:], rhs=mrb[:1, 1, :],
               start=True, stop=True)
            for blk in range(2):
                nc.vector.tensor_tensor(out=x[:, blk, cs:cs + 500],
                                        in0=x[:, blk, cs:cs + 500],
                                        in1=pbcM[:, 0:500], op=A.subtract)
                nc.vector.tensor_tensor(out=x[:, blk, cs:cs + 500],
                                        in0=x[:, blk, cs:cs + 500],
                                        in1=pbcR[:, 0:500], op=A.mult)
                nc.scalar.activation(out=x[:, blk, cs:cs + 500],
                                     in_=x[:, blk, cs:cs + 500], func=AF.Identity,
                                     scale=C["cng"][:, blk:blk + 1],
                                     bias=sbv[:, blk:blk + 1])
            nc.vector.tensor_tensor(out=x[:, :, cs:cs + 500], in0=x[:, :, cs:cs + 500],
                                    in1=C["posT"][:], op=A.add)

            # events
            evr = fr.tile([1, 3, 256], f32, tag="evr")
            nc.sync.dma_start(out=evr[:], in_=P["evrows"][s])
            tpv = fr.tile([128, 2], f32, tag="tpv")
            nc.sync.dma_start(out=tpv[:], in_=P["tposv"][s])
            kpv = fr.tile([128, 2], f32, tag="kpv")
            nc.sync.dma_start(out=kpv[:], in_=P["keepv"][s])
            comb = fr.tile([128, 6, 256], bf16, tag="comb")
            for vr in range(3):
                pb = pm1.tile([128, 500], f32, tag="pm1")
                mm(out=pb[:, 0:256], lhsT=ones1f[:], rhs=evr[:1, vr, :],
                   start=True, stop=True)
                arg = fr.tile([128, 256], f32, tag="arg")
                nc.scalar.activation(out=arg[:], in_=pb[:, 0:256], func=AF.Copy,
                                     scale=C["freqv"][:])
                nc.vector.tensor_scalar(out=arg[:], in0=arg[:], scalar1=INV2PI,
                                        scalar2=None, op0=A.mult)
                w1_ = fr.tile([128, 256], f32, tag="w1_")
                ti_ = fr.tile([128, 256], mybir.dt.int32, tag="ti_")
                tf_ = fr.tile([128, 256], f32, tag="tf_")
                nc.vector.tensor_copy(ti_[:], arg[:])
                nc.vector.tensor_copy(tf_[:], ti_[:])
                nc.vector.tensor_tensor(out=w1_[:], in0=arg[:], in1=tf_[:],
                                        op=A.subtract)
                nc.scalar.activation(out=comb[:, 2 * vr, :], in_=w1_[:], func=AF.Sin,
                                     scale=TWOPI, bias=zerov[:])
                nc.vector.tensor_scalar(out=arg[:], in0=arg[:], scalar1=0.25,
                                        scalar2=None, op0=A.add)
                nc.vector.tensor_copy(ti_[:], arg[:])
                nc.vector.tensor_copy(tf_[:], ti_[:])
                nc.vector.tensor_tensor(out=w1_[:], in0=arg[:], in1=tf_[:],
                                        op=A.subtract)
                nc.scalar.activation(out=comb[:, 2 * vr + 1, :], in_=w1_[:],
                                     func=AF.Sin, scale=TWOPI, bias=zerov[:])
            hmid = fr.tile([128, 2, 256], bf16, tag="hmid")
            for mb in range(2):
                ph = pm1.tile([128, 500], f32, tag="pm1")
                for kc in range(6):
                    mm(out=ph[:, 0:256],
                       lhsT=C["epW1T"][:, kc, mb * 128:(mb + 1) * 128],
                       rhs=comb[:, kc, :], start=(kc == 0), stop=(kc == 5))
                nc.scalar.activation(out=hmid[:, mb, :], in_=ph[:, 0:256],
                                     func=AF.Gelu, bias=C["epb1"][:, mb:mb + 1])
            evt = fr.tile([128, 2, 256], bf16, tag="evt")
            for ec in range(2):
                pe = pm1.tile([128, 500], f32, tag="pm1")
                for kc in range(2):
                    mm(out=pe[:, 0:256],
                       lhsT=hmid[:, kc, ec * 128:(ec + 1) * 128],
                       rhs=C["epW2T"][:, kc, :], start=(kc == 0), stop=(kc == 1))
                nc.vector.tensor_tensor(out=evt[:, ec, :], in0=pe[:, 0:256],
                                        in1=C["epb2row"][:], op=A.add)
                nc.vector.tensor_scalar(out=evt[:, ec, :], in0=evt[:, ec, :],
                                        scalar1=kpv[:, ec:ec + 1], scalar2=None,
                                        op0=A.mult)
            oh = fr.tile([128, 2, 500], bf16, tag="oh")
            for ec in range(2):
                nc.vector.tensor_scalar(out=oh[:, ec, :], in0=iotaB[:],
                                        scalar1=tpv[:, ec:ec + 1], scalar2=None,
                                        op0=A.is_equal)
            for mb in range(2):
                px = pm1.tile([128, 500], f32, tag="pm1")
                for ec in range(2):
                    mm(out=px[:], lhsT=evt[:, ec, mb * 128:(mb + 1) * 128],
                       rhs=oh[:, ec, :], start=(ec == 0), stop=(ec == 1))
                nc.vector.tensor_tensor(out=x[:, mb, cs:cs + 500],
                                        in0=x[:, mb, cs:cs + 500], in1=px[:], op=A.add)
        fr.release()
        wpool = tc.alloc_tile_pool(name="wpool", bufs=2)
        big = tc.alloc_tile_pool(name="big", bufs=1)
        scr = tc.alloc_tile_pool(name="scr", bufs=1)
        scr2 = tc.alloc_tile_pool(name="scr2", bufs=1)

        # ================= transformer =================
        for i in range(nl_run):
            W = {}
            for k, sh, dt in [("wqkvT", [128, 2, 768], bf16), ("bqkv", [128, 6], f32),
                              ("woT", [128, 2, 256], bf16), ("bov", [128, 2], f32),
                              ("ln1g", [128, 2], f32), ("ln1b", [128, 2], f32),
                              ("ln2g", [128, 2], f32), ("ln2b", [128, 2], f32),
                              ("w1TT", [128, 2, 1024], bf16), ("b1v", [128, 8], f32),
                              ("w2TT", [128, 8, 256], bf16), ("b2v", [128, 2], f32)]:
                W[k] = wpool.tile(sh, dt, tag="w_" + k, name=f"w{i}_" + k)
                nc.sync.dma_start(out=W[k][:], in_=P[k][i])

            def emit_ln(gk, bk, xn):
                for nch in range(4):
                    co = nch * 500
                    sq = scr2.tile([128, 2, 500], f32, tag="sq")
                    nc.scalar.activation(out=sq[:], in_=x[:, :, co:co + 500],
                                         func=AF.Square)
                    pstt = pst.tile([1, 2, 512], f32, tag="pst")
                    for blk in range(2):
                        mm(out=pstt[:1, 0, 0:500], lhsT=onesPf[:],
                           rhs=x[:, blk, co:co + 500],
                           start=(blk == 0), stop=(blk == 1))
                    for blk in range(2):
                        mm(out=pstt[:1, 1, 0:500], lhsT=onesPf[:],
                           rhs=sq[:, blk, :], start=(blk == 0), stop=(blk == 1))
                    mr = sc.tile([1, 2, 500], f32, tag="mr")
                    mrb = sc.tile([1, 2, 500], bf16, tag="mrb")
                    tmp = sc.tile([1, 2, 500], f32, tag="tmp1")
                    emit_stats(pstt, mr, mrb, tmp, 1.0 / 256.0)
                    pbcM = pm2.tile([128, 512], f32, tag="pm2")
                    mm(out=pbcM[:, 0:500], lhsT=ones1b[:], rhs=mrb[:1, 0, :],
                       start=True, stop=True)
                    pbcR = pm2.tile([128, 512], f32, tag="pm2")
                    mm(out=pbcR[:, 0:500], lhsT=ones1b[:], rhs=mrb[:1, 1, :],
                       start=True, stop=True)
                    for blk in range(2):
                        nc.vector.tensor_tensor(out=xn[:, blk, co:co + 500],
                                                in0=x[:, blk, co:co + 500],
                                                in1=pbcM[:, 0:500], op=A.subtract)
                        nc.vector.tensor_tensor(out=xn[:, blk, co:co + 500],
                                                in0=xn[:, blk, co:co + 500],
                                                in1=pbcR[:, 0:500], op=A.mult)
                        nc.scalar.activation(out=xn[:, blk, co:co + 500],
                                             in_=xn[:, blk, co:co + 500],
                                             func=AF.Identity,
                                             scale=W[gk][:, blk:blk + 1],
                                             bias=W[bk][:, blk:blk + 1])

            xn = big.tile([128, 2, 2000], bf16, tag="xn")
            emit_ln("ln1g", "ln1b", xn)
            attn = big.tile([128, 2, 2000], bf16, tag="attn")
            for s in range(ns_run):
                cs = s * 500
                qkv = scr.tile([128, 6, 500], bf16, tag="qkv")
                for j in range(6):
                    pq = pm1.tile([128, 500], f32, tag="pm1")
                    for kc in range(2):
                        mm(out=pq[:], lhsT=W["wqkvT"][:, kc, j * 128:(j + 1) * 128],
                           rhs=xn[:, kc, cs:cs + 500],
                           start=(kc == 0), stop=(kc == 1))
                    nc.vector.tensor_scalar(out=qkv[:, j, :], in0=pq[:],
                                            scalar1=W["bqkv"][:, j:j + 1],
                                            scalar2=None, op0=A.add)
                for j in range(2):
                    vt = scr2.tile([128, 4, 128], bf16, tag="vt")
                    pv4 = pmt.tile([128, 4, 128], bf16, tag="pvt")
                    for skc in range(4):
                        nc.tensor.transpose(
                            out=pv4[:125, skc, :],
                            in_=qkv[:, 4 + j, skc * 125: skc * 125 + 125],
                            identity=identb[:])
                    nc.scalar.activation(out=vt[:125, :, :],
                                         in_=pv4[:125, :, :], func=AF.Copy)
                    pot4 = pm1.tile([128, 500], f32, tag="pm1")
                    pcs4 = pst.tile([4, 512], f32, tag="pcs4")
                    for c4 in range(4):
                        poff = 32 * c4
                        h_q = qkv[poff:poff + 32, j, :]
                        h_k = qkv[poff:poff + 32, 2 + j, :]
                        eT = scr.tile([128, 4, 500], bf16, tag="eT")
                        for skc in range(4):
                            psc = pm2.tile([128, 512], f32, tag="pm2")
                            mm(out=psc[:125, 0:500],
                               lhsT=h_k[:, skc * 125: skc * 125 + 125],
                               rhs=h_q, start=True, stop=True,
                               tile_position=(poff, 0))
                            nc.scalar.activation(
                                out=eT[:125, skc, :],
                                in_=psc[:125, 0:500], func=AF.Exp, scale=SC32)
                        for skc in range(4):
                            mm(out=pcs4[0:4, 0:500], lhsT=C["onesZc"][:125, c4, :],
                               rhs=eT[:125, skc, :],
                               start=(c4 == 0 and skc == 0),
                               stop=(c4 == 3 and skc == 3))
                        for skc in range(4):
                            mm(out=pot4[poff:poff + 32, :],
                               lhsT=vt[:125, skc, poff:poff + 32],
                               rhs=eT[:125, skc, :],
                               start=(skc == 0), stop=(skc == 3),
                               tile_position=(0, poff))
                    rrf = sc.tile([4, 500], f32, tag="rrf")
                    nc.vector.reciprocal_approx_fast(out=rrf[:],
                                                     in_=pcs4[0:4, 0:500])
                    rrb4 = sc.tile([4, 500], bf16, tag="rrb4")
                    nc.vector.tensor_copy(rrb4[:], rrf[:])
                    prbF = pm2.tile([128, 512], f32, tag="pm2")
                    mm(out=prbF[:, 0:500], lhsT=C["e4m"][:], rhs=rrb4[:],
                       start=True, stop=True)
                    rbsF = scr2.tile([128, 500], bf16, tag="rbs")
                    nc.scalar.activation(out=rbsF[:], in_=prbF[:, 0:500],
                                         func=AF.Copy)
                    nc.vector.tensor_tensor(out=attn[:, j, cs:cs + 500],
                                            in0=pot4[:], in1=rbsF[:], op=A.mult)
            for mb in range(2):
                for nch in range(4):
                    po = pm1.tile([128, 500], f32, tag="pm1")
                    for kc in range(2):
                        mm(out=po[:], lhsT=W["woT"][:, kc, mb * 128:(mb + 1) * 128],
                           rhs=attn[:, kc, nch * 500:(nch + 1) * 500],
                           start=(kc == 0), stop=(kc == 1))
                    nc.vector.tensor_tensor(out=x[:, mb, nch * 500:(nch + 1) * 500],
                                            in0=x[:, mb, nch * 500:(nch + 1) * 500],
                                            in1=po[:], op=A.add)
                nc.vector.tensor_scalar(out=x[:, mb, :], in0=x[:, mb, :],
                                        scalar1=W["bov"][:, mb:mb + 1], scalar2=None,
                                        op0=A.add)
            xn2 = big.tile([128, 2, 2000], bf16, tag="xn")
            emit_ln("ln2g", "ln2b", xn2)
            for s in range(ns_run):
                cs = s * 500
                fh = scr.tile([128, 8, 500], bf16, tag="fh")
                for hb in range(8):
                    phh = pm1.tile([128, 500], f32, tag="pm1")
                    for kc in range(2):
                        mm(out=phh[:], lhsT=W["w1TT"][:, kc, hb * 128:(hb + 1) * 128],
                           rhs=xn2[:, kc, cs:cs + 500], start=(kc == 0),
                           stop=(kc == 1))
                    nc.scalar.activation(out=fh[:, hb, :], in_=phh[:], func=AF.Gelu,
                                         bias=W["b1v"][:, hb:hb + 1])
                for mb in range(2):
                    pf = pm1.tile([128, 500], f32, tag="pm1")
                    for hb in range(8):
                        mm(out=pf[:], lhsT=W["w2TT"][:, hb, mb * 128:(mb + 1) * 128],
                           rhs=fh[:, hb, :], start=(hb == 0), stop=(hb == 7))
                    nc.vector.tensor_tensor(out=x[:, mb, cs:cs + 500],
                                            in0=x[:, mb, cs:cs + 500], in1=pf[:],
                                            op=A.add)
            for mb in range(2):
                nc.vector.tensor_scalar(out=x[:, mb, :], in0=x[:, mb, :],
                                        scalar1=W["b2v"][:, mb:mb + 1], scalar2=None,
                                        op0=A.add)

        # ================= pooling + head =================
        for s in range(ns_run):
            cs = s * 500
            plg = pst.tile([1, 2, 512], f32, tag="pst")
            for blk in range(2):
                mm(out=plg[:1, 0, 0:500], lhsT=C["poolq"][:, blk:blk + 1],
                   rhs=x[:, blk, cs:cs + 500], start=(blk == 0), stop=(blk == 1))
            wrow = sc.tile([1, 500], f32, tag="wrow")
            nc.scalar.activation(out=wrow[:], in_=plg[:1, 0, 0:500], func=AF.Exp,
                                 scale=1.0 / 16.0)
            pwb = pm1.tile([128, 500], f32, tag="pm1")
            mm(out=pwb[:], lhsT=ones1f[:], rhs=wrow[:], start=True, stop=True)
            wx = scr2.tile([128, 2, 500], f32, tag="sq")
            for blk in range(2):
                nc.vector.tensor_tensor(out=wx[:, blk, :], in0=x[:, blk, cs:cs + 500],
                                        in1=pwb[:], op=A.mult)
            pooled = sc.tile([128, 4], f32, tag="pooled")
            nc.vector.tensor_reduce(out=pooled[:, 0:2], in_=wx[:], axis=AX.X,
                                    op=A.add)
            nc.scalar.activation(out=pooled[:, 2:4], in_=pooled[:, 0:2],
                                 func=AF.Square)
            pps = pst.tile([1, 2, 512], f32, tag="pst")
            mm(out=pps[:1, 0, 0:4], lhsT=onesPf[:], rhs=pooled[:], start=True,
               stop=True)
            z = sc.tile([1, 16], f32, tag="z")
            nc.vector.tensor_copy(z[:, 12:16], pps[:1, 0, 0:4])
            nc.vector.tensor_tensor(out=z[:, 0:1], in0=z[:, 12:13],
                                    in1=z[:, 13:14], op=A.add)
            nc.vector.tensor_tensor(out=z[:, 1:2], in0=z[:, 14:15],
                                    in1=z[:, 15:16], op=A.add)
            nc.vector.tensor_scalar(out=z[:, 2:3], in0=z[:, 0:1],
                                    scalar1=1.0 / 256.0, scalar2=None, op0=A.mult)
            nc.vector.tensor_tensor(out=z[:, 3:4], in0=z[:, 2:3], in1=z[:, 2:3],
                                    op=A.mult)
            nc.vector.tensor_scalar(out=z[:, 4:5], in0=z[:, 1:2],
                                    scalar1=1.0 / 256.0, scalar2=None, op0=A.mult)
            nc.vector.tensor_tensor(out=z[:, 4:5], in0=z[:, 4:5], in1=z[:, 3:4],
                                    op=A.subtract)
            nc.scalar.activation(out=z[:, 5:6], in_=z[:, 4:5], func=AF.Sqrt,
                                 bias=epsv[0:1, :])
            nc.vector.reciprocal_approx_fast(out=z[:, 6:7], in_=z[:, 5:6])
            cp = sc.tile([128, 2], f32, tag="cp")
            nc.vector.tensor_tensor(out=cp[:], in0=pooled[:, 0:2], in1=C["c1v"][:],
                                    op=A.mult)
            pa = pst.tile([1, 2, 512], f32, tag="pst")
            mm(out=pa[:1, 0, 0:2], lhsT=onesPf[:], rhs=cp[:], start=True, stop=True)
            nc.vector.tensor_copy(z[:, 10:12], pa[:1, 0, 0:2])
            nc.vector.tensor_tensor(out=z[:, 7:8], in0=z[:, 10:11],
                                    in1=z[:, 11:12], op=A.add)
            nc.vector.tensor_scalar(out=z[:, 8:9], in0=z[:, 2:3], scalar1=sc1,
                                    scalar2=None, op0=A.mult)
            nc.vector.tensor_tensor(out=z[:, 8:9], in0=z[:, 7:8], in1=z[:, 8:9],
                                    op=A.subtract)
            nc.vector.tensor_tensor(out=z[:, 8:9], in0=z[:, 8:9], in1=z[:, 6:7],
                                    op=A.mult)
            nc.vector.tensor_scalar(out=z[:, 9:10], in0=z[:, 8:9], scalar1=c2s,
                                    scalar2=None, op0=A.add)
            nc.sync.dma_start(out=Y[s:s + 1, :], in_=z[:, 9:10])

        for p in [pmt, pst, pm2, scr2, scr, big, wpool, pm1, bigx, sc, sing]:
            p.release()

    nc.compile()
    return nc


_BUILT = {}


def _get_nc(key, **kw):
    if key not in _BUILT:
        _BUILT[key] = _build(**kw)
    return _BUILT[key]


def _make_in_maps(prep, ns=NS, ncores=NCORES):
    in_maps = []
    for c in range(ncores):
        sl = slice(c * ns, (c + 1) * ns)
        m = {k: prep[k] for k in CONST_KEYS}
        m["mel4"] = prep["_mel"][sl]
        m["evrows"] = prep["_evrows"][sl]
        m["tposv"] = prep["_tposv"][sl]
        m["keepv"] = prep["_keepv"][sl]
        m["starbias"] = prep["_starbias"][sl]
        in_maps.append(m)
    return in_maps


def kernel(**inputs):
    from concourse.bass_utils import run_bass_kernel_spmd

    prep = _host_prep(inputs)
    nc = _get_nc("full", nl_run=NLAYERS, ns_run=NS, debug=False,
                 sc1=prep["sc1"], c2s=prep["c2s"])
    res = run_bass_kernel_spmd(nc, _make_in_maps(prep), list(range(NCORES)))
    y = np.concatenate([res.results[c]["y"].reshape(-1) for c in range(NCORES)])
    return y.astype(np.float32)


# revision 15
# speedup vs baseline: 1.1132x; 1.0158x over previous
"""Trainium2 Bass kernel for nn_ChartQualityEvaluator.

Data parallel: 32 samples -> 8 cores x 4 samples. Feature-major activations
[128 part, 2 blocks, 4*500 cols]. Matmuls and bulk activations in bf16
(fp32 PSUM accumulation); stats/sinusoid paths stay fp32. Unsafe softmax,
LN via ones-matmul stats + PE broadcast, event scatter via onehot matmul.
Host precomputes index-like preprocessing with f32-exact semantics.
"""
import math
import sys

import numpy as np
import ml_dtypes

_TRN = "/opt/trn_rl_repo"
if _TRN not in sys.path:
    sys.path.insert(0, _TRN)

BF16 = ml_dtypes.bfloat16

D = 256
H = 8
NLAYERS = 6
HALF = 128
S = 500
NEV = 256
NCORES = 8
NS = 4  # samples per core
B = 32
EPS = 1e-5
INV2PI = float(np.float32(1.0 / (2.0 * math.pi)))
TWOPI = 2.0 * math.pi
SC32 = float(np.float32(1.0 / math.sqrt(32.0)))


def _host_prep(inp):
    f = np.float32
    out = {}

    def t2(v):  # [256] -> [128,2]
        return np.ascontiguousarray(np.asarray(v).reshape(2, 128).T.astype(f))

    def b(a):  # -> bf16
        return np.ascontiguousarray(np.asarray(a).astype(np.float32).astype(BF16))

    out["wmelT"] = b(np.asarray(inp["mel_W"]).T)
    out["melb"] = np.ascontiguousarray(np.asarray(inp["mel_b"]).reshape(16, 1).astype(f))
    out["w1t"] = b(np.asarray(inp["conv1_w"]).transpose(1, 2, 0))
    out["c1b"] = np.ascontiguousarray(np.asarray(inp["conv1_b"]).reshape(128, 1).astype(f))
    out["gng"] = np.ascontiguousarray(np.asarray(inp["gn_g"]).reshape(128, 1).astype(f))
    out["gnb"] = np.ascontiguousarray(np.asarray(inp["gn_b"]).reshape(128, 1).astype(f))
    out["w2t"] = b(np.asarray(inp["conv2_w"]).transpose(1, 2, 0))
    out["c2b"] = t2(inp["conv2_b"])
    out["cng"] = t2(inp["cn_g"])

    freq = np.exp(np.arange(HALF, dtype=f) * f(-math.log(10000.0) / (HALF - 1)))
    e32 = (np.arange(S, dtype=f)[None, :] * freq[:, None]).astype(f)
    e64 = e32.astype(np.float64)
    pos_fm = np.concatenate([np.sin(e64), np.cos(e64)], axis=0)  # [256,500]
    out["posT"] = np.ascontiguousarray(
        pos_fm.reshape(2, 128, S).transpose(1, 0, 2).astype(f))
    out["freqv"] = np.ascontiguousarray(freq.reshape(128, 1))

    out["epW1T"] = b(np.asarray(inp["ep_W1"]).T.reshape(6, 128, 256).transpose(1, 0, 2))
    out["epb1"] = t2(inp["ep_b1"])
    out["epW2T"] = b(np.asarray(inp["ep_W2"]).T.reshape(2, 128, 256).transpose(1, 0, 2))
    out["epb2row"] = np.ascontiguousarray(
        np.tile(np.asarray(inp["ep_b2"]).astype(f)[None, :], (128, 1)))

    def wT(w, kc, m):  # w [m, k] -> [128, kc, m] bf16
        return b(np.asarray(w).T.reshape(kc, 128, m).transpose(1, 0, 2))

    out["wqkvT"] = np.stack([wT(inp["tl_Wqkv"][i], 2, 768) for i in range(NLAYERS)])
    out["bvrow"] = np.stack([b(np.asarray(inp["tl_bqkv"][i])[512:768].reshape(1, 256))
                             for i in range(NLAYERS)])
    out["bqkv"] = np.stack([np.ascontiguousarray(
        np.asarray(inp["tl_bqkv"][i]).reshape(6, 128).T.astype(f))
        for i in range(NLAYERS)])
    out["woT"] = np.stack([wT(inp["tl_Wo"][i], 2, 256) for i in range(NLAYERS)])
    out["bov"] = np.stack([t2(inp["tl_bo"][i]) for i in range(NLAYERS)])
    out["ln1g"] = np.stack([t2(inp["tl_ln1g"][i]) for i in range(NLAYERS)])
    out["ln1b"] = np.stack([t2(inp["tl_ln1b"][i]) for i in range(NLAYERS)])
    out["ln2g"] = np.stack([t2(inp["tl_ln2g"][i]) for i in range(NLAYERS)])
    out["ln2b"] = np.stack([t2(inp["tl_ln2b"][i]) for i in range(NLAYERS)])
    out["w1TT"] = np.stack([wT(inp["tl_W1"][i], 2, 1024) for i in range(NLAYERS)])
    out["b1v"] = np.stack([np.ascontiguousarray(
        np.asarray(inp["tl_b1"][i]).reshape(8, 128).T.astype(f))
        for i in range(NLAYERS)])
    out["w2TT"] = np.stack([wT(inp["tl_W2"][i], 8, 256) for i in range(NLAYERS)])
    out["b2v"] = np.stack([t2(inp["tl_b2"][i]) for i in range(NLAYERS)])

    out["poolq"] = t2(inp["pool_q"])
    oz = np.zeros((128, 4, 4), np.float32)
    for c4 in range(4):
        oz[:, c4, c4] = 1.0
    out["onesZc"] = b(oz)
    e4 = np.zeros((4, 128), np.float32)
    for c4 in range(4):
        e4[c4, 32 * c4:32 * c4 + 32] = 1.0
    out["e4m"] = b(e4)
    c1 = (np.asarray(inp["oh_W"])[0] * np.asarray(inp["on_g"])).astype(f)
    out["c1v"] = t2(c1)
    out["sc1"] = float(c1.astype(np.float64).sum())
    out["c2s"] = float((np.asarray(inp["oh_W"])[0].astype(np.float64)
                        * np.asarray(inp["on_b"]).astype(np.float64)).sum()
                       + float(np.asarray(inp["oh_b"])[0]))

    events = np.asarray(inp["events"]).astype(np.int64)
    mask = np.asarray(inp["event_mask"])
    star = np.asarray(inp["star_rating"]).astype(f)
    nb = events.shape[0]
    diff = np.maximum(events[:, 1:] - events[:, :-1], 1)
    g = np.concatenate([diff[:, :1], diff], axis=1)
    gap_ms = (g * 5).astype(f)
    g_f = np.maximum(g.astype(f), f(1.0))
    r = np.clip(g_f[:, 1:] / g_f[:, :-1], f(0.1), f(10.0)).astype(f)
    ones = np.ones((nb, 1), f)
    rb50 = np.trunc(np.concatenate([ones, r], axis=1) * f(50.0)).astype(f)
    ra50 = np.trunc(np.concatenate([r, ones], axis=1) * f(50.0)).astype(f)
    out["_evrows"] = np.ascontiguousarray(np.stack([rb50, ra50, gap_ms], axis=1))
    tp = np.clip(events // 4, 0, S - 1).astype(f)
    keep = (1.0 - mask.astype(f)).astype(f)
    out["_tposv"] = np.ascontiguousarray(tp.reshape(nb, 2, 128).transpose(0, 2, 1))
    out["_keepv"] = np.ascontiguousarray(keep.reshape(nb, 2, 128).transpose(0, 2, 1))
    bucket = np.clip((star / f(0.5)).astype(np.int32), 0, 19)
    sb = (np.asarray(inp["cn_b"])[None, :] + np.asarray(inp["star_table"])[bucket]).astype(f)
    out["_starbias"] = np.ascontiguousarray(sb.reshape(nb, 2, 128).transpose(0, 2, 1))
    out["_mel"] = np.ascontiguousarray(np.asarray(inp["mel"]).astype(f).astype(BF16))
    return out


CONST_KEYS = ["wmelT", "melb", "w1t", "c1b", "gng", "gnb", "w2t", "c2b", "cng",
              "posT", "freqv", "epW1T", "epb1", "epW2T", "epb2row",
              "wqkvT", "bqkv", "woT", "bov", "ln1g", "ln1b", "ln2g", "ln2b",
              "w1TT", "b1v", "w2TT", "b2v", "poolq", "c1v", "onesZc", "e4m",
              "bvrow"]

# params that are bf16 on device
BF_KEYS = {"wmelT", "w1t", "w2t", "epW1T", "epW2T",
           "wqkvT", "woT", "w1TT", "w2TT", "mel4", "onesZc", "e4m", "bvrow"}


def _build(nl_run=NLAYERS, ns_run=NS, debug=False, sc1=0.0, c2s=0.0):
    import concourse.bacc as bacc
    import concourse.tile as tile
    from concourse import mybir
    from concourse.masks import make_identity

    f32 = mybir.dt.float32
    bf16 = mybir.dt.bfloat16
    A = mybir.AluOpType
    AF = mybir.ActivationFunctionType
    AX = mybir.AxisListType

    nc = bacc.Bacc(None)

    def mm(out, lhsT, rhs, **kw):
        nc.tensor.matmul(out=out, lhsT=lhsT, rhs=rhs, **kw)

    P = {}
    shapes = dict(
        mel4=[ns_run, 80, 2000], evrows=[ns_run, 3, 256], tposv=[ns_run, 128, 2],
        keepv=[ns_run, 128, 2], starbias=[ns_run, 128, 2],
        wmelT=[80, 16], melb=[16, 1], w1t=[16, 7, 128], c1b=[128, 1],
        gng=[128, 1], gnb=[128, 1], w2t=[128, 7, 256], c2b=[128, 2],
        cng=[128, 2], posT=[128, 2, 500], freqv=[128, 1],
        epW1T=[128, 6, 256], epb1=[128, 2], epW2T=[128, 2, 256], epb2row=[128, 256],
        wqkvT=[NLAYERS, 128, 2, 768], bqkv=[NLAYERS, 128, 6],
        bvrow=[NLAYERS, 1, 256],
        woT=[NLAYERS, 128, 2, 256], bov=[NLAYERS, 128, 2],
        ln1g=[NLAYERS, 128, 2], ln1b=[NLAYERS, 128, 2],
        ln2g=[NLAYERS, 128, 2], ln2b=[NLAYERS, 128, 2],
        w1TT=[NLAYERS, 128, 2, 1024], b1v=[NLAYERS, 128, 8],
        w2TT=[NLAYERS, 128, 8, 256], b2v=[NLAYERS, 128, 2],
        poolq=[128, 2], c1v=[128, 2], onesZc=[128, 4, 4], e4m=[4, 128],
    )
    for k, sh in shapes.items():
        P[k] = nc.declare_dram_parameter(k, sh, bf16 if k in BF_KEYS else f32,
                                         isOutput=False)
    Y = nc.declare_dram_parameter("y", [ns_run, 1], f32, isOutput=True)

    with tile.TileContext(nc) as tc:
        sing = tc.alloc_tile_pool(name="sing", bufs=1)
        sc = tc.alloc_tile_pool(name="sc", bufs=1)
        bigx = tc.alloc_tile_pool(name="bigx", bufs=1)
        pm1 = tc.alloc_tile_pool(name="pm1", bufs=3, space="PSUM")
        pm2 = tc.alloc_tile_pool(name="pm2", bufs=2, space="PSUM")
        pst = tc.alloc_tile_pool(name="pst", bufs=1, space="PSUM")

        C = {}
        for k in ["wmelT", "melb", "w1t", "c1b", "gng", "gnb", "w2t", "c2b",
                  "cng", "posT", "freqv", "epW1T", "epb1", "epW2T", "epb2row",
                  "poolq", "c1v", "onesZc", "e4m"]:
            C[k] = sing.tile(shapes[k], bf16 if k in BF_KEYS else f32,
                             tag=k, name="c_" + k)
            nc.sync.dma_start(out=C[k][:], in_=P[k][:])
        identb = sing.tile([128, 128], bf16, tag="identb")
        make_identity(nc, identb[:])
        onesPf = sing.tile([128, 1], f32, tag="onesPf")
        nc.vector.memset(onesPf[:], 1.0)
        onesPb = sing.tile([128, 1], bf16, tag="onesPb")
        nc.vector.memset(onesPb[:], 1.0)
        ones1f = sing.tile([1, 128], f32, tag="ones1f")
        nc.vector.memset(ones1f[:], 1.0)
        ones1b = sing.tile([1, 128], bf16, tag="ones1b")
        nc.vector.memset(ones1b[:], 1.0)
        zerov = sing.tile([128, 1], f32, tag="zerov")
        nc.vector.memset(zerov[:], 0.0)
        epsv = sing.tile([128, 1], f32, tag="epsv")
        nc.vector.memset(epsv[:], EPS)
        iotaB = sing.tile([128, 500], f32, tag="iotaB")
        nc.gpsimd.iota(iotaB[:], pattern=[[1, 500]], base=0, channel_multiplier=0,
                       allow_small_or_imprecise_dtypes=True)

        x = bigx.tile([128, 2, 2000], f32, tag="x_fm")

        def s2(t):  # step-2 view of [p, n] -> [p, n//2]
            return t.rearrange("p (t s) -> p s t", s=2)[:, 0, :]

        # small-vector LN stat helper: psum [1,2,500] (s,ss) -> mr (m, r); also
        # writes mrb (bf16 copy of [m, r]) for cheap broadcast matmuls.
        def emit_stats(pstt, mr, mrb, tmp, scale):
            nc.vector.tensor_scalar(out=mr[:1, 0:2, :], in0=pstt[:1, 0:2, 0:500],
                                    scalar1=scale, scalar2=None, op0=A.mult)
            nc.vector.tensor_tensor(out=tmp[:1, 0, :], in0=mr[:1, 0, :],
                                    in1=mr[:1, 0, :], op=A.mult)
            nc.vector.tensor_tensor(out=tmp[:1, 1, :], in0=mr[:1, 1, :],
                                    in1=tmp[:1, 0, :], op=A.subtract)
            nc.scalar.activation(out=tmp[:1, 0, :], in_=tmp[:1, 1, :], func=AF.Sqrt,
                                 bias=epsv[0:1, :])
            nc.vector.reciprocal_approx_fast(out=mr[:1, 1, :], in_=tmp[:1, 0, :])
            nc.vector.tensor_copy(mrb[:1, 0:2, :], mr[:1, 0:2, :])

        # ================= front end =================
        fr = tc.alloc_tile_pool(name="fr", bufs=2)
        for s in range(ns_run):
            cs = s * 500
            melp = fr.tile([80, 2006], bf16, tag="melp")
            nc.vector.memset(melp[:, 0:3], 0.0)
            nc.vector.memset(melp[:, 2003:2006], 0.0)
            nc.sync.dma_start(out=melp[:, 3:2003], in_=P["mel4"][s])
            xmelp = fr.tile([16, 2006], bf16, tag="xmelp")
            nc.vector.memset(xmelp[:, 0:3], 0.0)
            nc.vector.memset(xmelp[:, 2003:2006], 0.0)
            for nch in range(4):
                pcm = pm1.tile([128, 500], f32, tag="pm1")
                mm(out=pcm[:16, :], lhsT=C["wmelT"][:],
                   rhs=melp[:, 3 + nch * 500: 3 + nch * 500 + 500],
                   start=True, stop=True)
                nc.scalar.activation(out=xmelp[:, 3 + nch * 500: 3 + nch * 500 + 500],
                                     in_=pcm[:16, :], func=AF.Identity,
                                     bias=C["melb"][:, 0:1])
            h1g = fr.tile([128, 2, 500], bf16, tag="h1g")
            stg = fr.tile([128, 4], f32, tag="stg")
            for half in range(2):
                pc1 = pm2.tile([128, 512], f32, tag="pm2")
                for k in range(7):
                    mm(out=pc1[:, 0:500], lhsT=C["w1t"][:, k, :],
                       rhs=s2(xmelp[:, k + half * 1000: k + half * 1000 + 1000]),
                       start=(k == 0), stop=(k == 6))
                nc.scalar.activation(out=h1g[:, half, :], in_=pc1[:, 0:500],
                                     func=AF.Gelu, bias=C["c1b"][:, 0:1],
                                     accum_out=stg[:, half:half + 1])
            sqf = fr.tile([128, 2, 500], f32, tag="sqf")
            nc.scalar.activation(out=sqf[:], in_=h1g[:], func=AF.Square,
                                 accum_out=stg[:, 2:3])
            pg = pst.tile([1, 2, 512], f32, tag="pst")
            mm(out=pg[:1, 0, 0:3], lhsT=onesPf[:], rhs=stg[:, 0:3], start=True,
               stop=True)
            sn = sc.tile([1, 8], f32, tag="sn")
            nc.vector.tensor_scalar(out=sn[:, 0:2], in0=pg[:1, 0, 1:3],
                                    scalar1=1.0 / 128000.0, scalar2=None, op0=A.mult)
            nc.vector.tensor_scalar(out=sn[:, 6:7], in0=pg[:1, 0, 0:1],
                                    scalar1=1.0 / 128000.0, scalar2=None, op0=A.mult)
            nc.vector.tensor_tensor(out=sn[:, 0:1], in0=sn[:, 0:1], in1=sn[:, 6:7],
                                    op=A.add)
            nc.vector.tensor_tensor(out=sn[:, 2:3], in0=sn[:, 0:1], in1=sn[:, 0:1],
                                    op=A.mult)
            nc.vector.tensor_tensor(out=sn[:, 3:4], in0=sn[:, 1:2], in1=sn[:, 2:3],
                                    op=A.subtract)
            nc.scalar.activation(out=sn[:, 4:5], in_=sn[:, 3:4], func=AF.Sqrt,
                                 bias=epsv[0:1, :])
            nc.vector.reciprocal_approx_fast(out=sn[:, 1:2], in_=sn[:, 4:5])
            pgb = pm1.tile([128, 500], f32, tag="pm1")
            mm(out=pgb[:, 0:2], lhsT=ones1f[:], rhs=sn[:, 0:2], start=True, stop=True)
            sv = sc.tile([128, 2], f32, tag="sv")
            nc.vector.tensor_tensor(out=sv[:, 0:1], in0=pgb[:, 1:2], in1=C["gng"][:],
                                    op=A.mult)
            nc.vector.tensor_tensor(out=sv[:, 1:2], in0=pgb[:, 0:1], in1=sv[:, 0:1],
                                    op=A.mult)
            nc.vector.tensor_tensor(out=sv[:, 1:2], in0=C["gnb"][:], in1=sv[:, 1:2],
                                    op=A.subtract)
            x2p = fr.tile([128, 1006], bf16, tag="x2p")
            nc.vector.memset(x2p[:, 0:3], 0.0)
            nc.vector.memset(x2p[:, 1003:1006], 0.0)
            nc.scalar.activation(out=x2p[:, 3:1003],
                                 in_=h1g.rearrange("p a b -> p (a b)"),
                                 func=AF.Identity, scale=sv[:, 0:1], bias=sv[:, 1:2])
            for mb in range(2):
                pc2 = pm2.tile([128, 512], f32, tag="pm2")
                for k in range(7):
                    mm(out=pc2[:, 0:500],
                       lhsT=C["w2t"][:, k, mb * 128:(mb + 1) * 128],
                       rhs=s2(x2p[:, k:k + 1000]),
                       start=(k == 0), stop=(k == 6))
                nc.scalar.activation(out=x[:, mb, cs:cs + 500], in_=pc2[:, 0:500],
                                     func=AF.Gelu, bias=C["c2b"][:, mb:mb + 1])
            # CN layernorm + starbias + pos
            sbv = fr.tile([128, 2], f32, tag="sbv")
            nc.sync.dma_start(out=sbv[:], in_=P["starbias"][s])
            nc.scalar.activation(out=sqf[:], in_=x[:, :, cs:cs + 500], func=AF.Square)
            pstt = pst.tile([1, 2, 512], f32, tag="pst")
            for blk in range(2):
                mm(out=pstt[:1, 0, 0:500], lhsT=onesPf[:],
                   rhs=x[:, blk, cs:cs + 500], start=(blk == 0), stop=(blk == 1))
            for blk in range(2):
                mm(out=pstt[:1, 1, 0:500], lhsT=onesPf[:],
                   rhs=sqf[:, blk, :], start=(blk == 0), stop=(blk == 1))
            mr = sc.tile([1, 2, 500], f32, tag="mr")
            mrb = sc.tile([1, 2, 500], bf16, tag="mrb")
            tmp = sc.tile([1, 2, 500], f32, tag="tmp1")
            emit_stats(pstt, mr, mrb, tmp, 1.0 / 256.0)
            pbcM = pm2.tile([128, 512], f32, tag="pm2")
            mm(out=pbcM[:, 0:500], lhsT=ones1b[:], rhs=mrb[:1, 0, :],
               start=True, stop=True)
            pbcR = pm2.tile([128, 512], f32, tag="pm2")
            mm(out=pbcR[:, 0:500], lhsT=ones1b[:], rhs=mrb[:1, 1, :],
               start=True, stop=True)
            for blk in range(2):
                nc.vector.tensor_tensor(out=x[:, blk, cs:cs + 500],
                                        in0=x[:, blk, cs:cs + 500],
                                        in1=pbcM[:, 0:500], op=A.subtract)
                nc.vector.tensor_tensor(out=x[:, blk, cs:cs + 500],
                                        in0=x[:, blk, cs:cs + 500],
                                        in1=pbcR[:, 0:500], op=A.mult)
                nc.scalar.activation(out=x[:, blk, cs:cs + 500],
                                     in_=x[:, blk, cs:cs + 500], func=AF.Identity,
                                     scale=C["cng"][:, blk:blk + 1],
                                     bias=sbv[:, blk:blk + 1])
            nc.vector.tensor_tensor(out=x[:, :, cs:cs + 500], in0=x[:, :, cs:cs + 500],
                                    in1=C["posT"][:], op=A.add)

            # events
            evr = fr.tile([1, 3, 256], f32, tag="evr")
            nc.sync.dma_start(out=evr[:], in_=P["evrows"][s])
            tpv = fr.tile([128, 2], f32, tag="tpv")
            nc.sync.dma_start(out=tpv[:], in_=P["tposv"][s])
            kpv = fr.tile([128, 2], f32, tag="kpv")
            nc.sync.dma_start(out=kpv[:], in_=P["keepv"][s])
            comb = fr.tile([128, 6, 256], bf16, tag="comb")
            for vr in range(3):
                pb = pm1.tile([128, 500], f32, tag="pm1")
                mm(out=pb[:, 0:256], lhsT=ones1f[:], rhs=evr[:1, vr, :],
                   start=True, stop=True)
                arg = fr.tile([128, 256], f32, tag="arg")
                nc.scalar.activation(out=arg[:], in_=pb[:, 0:256], func=AF.Copy,
                                     scale=C["freqv"][:])
                nc.vector.tensor_scalar(out=arg[:], in0=arg[:], scalar1=INV2PI,
                                        scalar2=None, op0=A.mult)
                w1_ = fr.tile([128, 256], f32, tag="w1_")
                ti_ = fr.tile([128, 256], mybir.dt.int32, tag="ti_")
                tf_ = fr.tile([128, 256], f32, tag="tf_")
                nc.vector.tensor_copy(ti_[:], arg[:])
                nc.vector.tensor_copy(tf_[:], ti_[:])
                nc.vector.tensor_tensor(out=w1_[:], in0=arg[:], in1=tf_[:],
                                        op=A.subtract)
                nc.scalar.activation(out=comb[:, 2 * vr, :], in_=w1_[:], func=AF.Sin,
                                     scale=TWOPI, bias=zerov[:])
                nc.vector.tensor_scalar(out=arg[:], in0=arg[:], scalar1=0.25,
                                        scalar2=None, op0=A.add)
                nc.vector.tensor_copy(ti_[:], arg[:])
                nc.vector.tensor_copy(tf_[:], ti_[:])
                nc.vector.tensor_tensor(out=w1_[:], in0=arg[:], in1=tf_[:],
                                        op=A.subtract)
                nc.scalar.activation(out=comb[:, 2 * vr + 1, :], in_=w1_[:],
                                     func=AF.Sin, scale=TWOPI, bias=zerov[:])
            hmid = fr.tile([128, 2, 256], bf16, tag="hmid")
            for mb in range(2):
                ph = pm1.tile([128, 500], f32, tag="pm1")
                for kc in range(6):
                    mm(out=ph[:, 0:256],
                       lhsT=C["epW1T"][:, kc, mb * 128:(mb + 1) * 128],
                       rhs=comb[:, kc, :], start=(kc == 0), stop=(kc == 5))
                nc.scalar.activation(out=hmid[:, mb, :], in_=ph[:, 0:256],
                                     func=AF.Gelu, bias=C["epb1"][:, mb:mb + 1])
            evt = fr.tile([128, 2, 256], bf16, tag="evt")
            for ec in range(2):
                pe = pm1.tile([128, 500], f32, tag="pm1")
                for kc in range(2):
                    mm(out=pe[:, 0:256],
                       lhsT=hmid[:, kc, ec * 128:(ec + 1) * 128],
                       rhs=C["epW2T"][:, kc, :], start=(kc == 0), stop=(kc == 1))
                nc.vector.tensor_tensor(out=evt[:, ec, :], in0=pe[:, 0:256],
                                        in1=C["epb2row"][:], op=A.add)
                nc.vector.tensor_scalar(out=evt[:, ec, :], in0=evt[:, ec, :],
                                        scalar1=kpv[:, ec:ec + 1], scalar2=None,
                                        op0=A.mult)
            oh = fr.tile([128, 2, 500], bf16, tag="oh")
            for ec in range(2):
                nc.vector.tensor_scalar(out=oh[:, ec, :], in0=iotaB[:],
                                        scalar1=tpv[:, ec:ec + 1], scalar2=None,
                                        op0=A.is_equal)
            for mb in range(2):
                px = pm1.tile([128, 500], f32, tag="pm1")
                for ec in range(2):
                    mm(out=px[:], lhsT=evt[:, ec, mb * 128:(mb + 1) * 128],
                       rhs=oh[:, ec, :], start=(ec == 0), stop=(ec == 1))
                nc.vector.tensor_tensor(out=x[:, mb, cs:cs + 500],
                                        in0=x[:, mb, cs:cs + 500], in1=px[:], op=A.add)
        fr.release()
        wpool = tc.alloc_tile_pool(name="wpool", bufs=2)
        big = tc.alloc_tile_pool(name="big", bufs=1)
        scr = tc.alloc_tile_pool(name="scr", bufs=1)
        scr2 = tc.alloc_tile_pool(name="scr2", bufs=1)

        # ================= transformer =================
        for i in range(nl_run):
            W = {}
            for k, sh, dt in [("wqkvT", [128, 2, 768], bf16), ("bqkv", [128, 6], f32),
                              ("woT", [128, 2, 256], bf16), ("bov", [128, 2], f32),
                              ("ln1g", [128, 2], f32), ("ln1b", [128, 2], f32),
                              ("ln2g", [128, 2], f32), ("ln2b", [128, 2], f32),
                              ("w1TT", [128, 2, 1024], bf16), ("b1v", [128, 8], f32),
                              ("w2TT", [128, 8, 256], bf16), ("b2v", [128, 2], f32)]:
                W[k] = wpool.tile(sh, dt, tag="w_" + k, name=f"w{i}_" + k)
                nc.sync.dma_start(out=W[k][:], in_=P[k][i])
            bvr = wpool.tile([1, 256], bf16, tag="w_bvr", name=f"w{i}_bvr")
            nc.sync.dma_start(out=bvr[:], in_=P["bvrow"][i])
            pvb = pm2.tile([128, 512], f32, tag="pm2")
            mm(out=pvb[:, 0:256], lhsT=ones1b[:], rhs=bvr[:], start=True, stop=True)
            vbF = wpool.tile([128, 256], bf16, tag="w_vbF", name=f"w{i}_vbF")
            nc.scalar.activation(out=vbF[:], in_=pvb[:, 0:256], func=AF.Copy)

            def emit_ln(gk, bk, xn):
                for nch in range(4):
                    co = nch * 500
                    sq = scr2.tile([128, 2, 500], f32, tag="sq")
                    nc.scalar.activation(out=sq[:], in_=x[:, :, co:co + 500],
                                         func=AF.Square)
                    pstt = pst.tile([1, 2, 512], f32, tag="pst")
                    for blk in range(2):
                        mm(out=pstt[:1, 0, 0:500], lhsT=onesPf[:],
                           rhs=x[:, blk, co:co + 500],
                           start=(blk == 0), stop=(blk == 1))
                    for blk in range(2):
                        mm(out=pstt[:1, 1, 0:500], lhsT=onesPf[:],
                           rhs=sq[:, blk, :], start=(blk == 0), stop=(blk == 1))
                    mr = sc.tile([1, 2, 500], f32, tag="mr")
                    mrb = sc.tile([1, 2, 500], bf16, tag="mrb")
                    tmp = sc.tile([1, 2, 500], f32, tag="tmp1")
                    emit_stats(pstt, mr, mrb, tmp, 1.0 / 256.0)
                    pbcM = pm2.tile([128, 512], f32, tag="pm2")
                    mm(out=pbcM[:, 0:500], lhsT=ones1b[:], rhs=mrb[:1, 0, :],
                       start=True, stop=True)
                    pbcR = pm2.tile([128, 512], f32, tag="pm2")
                    mm(out=pbcR[:, 0:500], lhsT=ones1b[:], rhs=mrb[:1, 1, :],
                       start=True, stop=True)
                    for blk in range(2):
                        nc.vector.tensor_tensor(out=xn[:, blk, co:co + 500],
                                                in0=x[:, blk, co:co + 500],
                                                in1=pbcM[:, 0:500], op=A.subtract)
                        nc.vector.tensor_tensor(out=xn[:, blk, co:co + 500],
                                                in0=xn[:, blk, co:co + 500],
                                                in1=pbcR[:, 0:500], op=A.mult)
                        nc.scalar.activation(out=xn[:, blk, co:co + 500],
                                             in_=xn[:, blk, co:co + 500],
                                             func=AF.Identity,
                                             scale=W[gk][:, blk:blk + 1],
                                             bias=W[bk][:, blk:blk + 1])

            xn = big.tile([128, 2, 2000], bf16, tag="xn")
            emit_ln("ln1g", "ln1b", xn)
            attn = big.tile([128, 2, 2000], bf16, tag="attn")
            for s in range(ns_run):
                cs = s * 500
                qkv = scr.tile([128, 4, 500], bf16, tag="qkv")
                vt = scr2.tile([128, 4, 256], bf16, tag="vt")
                for chunk in range(4):
                    pvt = pm2.tile([128, 512], f32, tag="pm2")
                    for kc in range(2):
                        mm(out=pvt[:125, 0:256],
                           lhsT=xn[:, kc, cs + chunk * 125: cs + chunk * 125 + 125],
                           rhs=W["wqkvT"][:, kc, 512:768],
                           start=(kc == 0), stop=(kc == 1))
                    nc.vector.tensor_tensor(out=vt[:125, chunk, :],
                                            in0=pvt[:125, 0:256], in1=vbF[:125, :],
                                            op=A.add)
                for j in range(4):
                    pq = pm1.tile([128, 500], f32, tag="pm1")
                    for kc in range(2):
                        mm(out=pq[:], lhsT=W["wqkvT"][:, kc, j * 128:(j + 1) * 128],
                           rhs=xn[:, kc, cs:cs + 500],
                           start=(kc == 0), stop=(kc == 1))
                    nc.vector.tensor_scalar(out=qkv[:, j, :], in0=pq[:],
                                            scalar1=W["bqkv"][:, j:j + 1],
                                            scalar2=None, op0=A.add)
                for j in range(2):
                    pot4 = pm1.tile([128, 500], f32, tag="pm1")
                    pcs4 = pst.tile([4, 512], f32, tag="pcs4")
                    for c4 in range(4):
                        poff = 32 * c4
                        h_q = qkv[poff:poff + 32, j, :]
                        h_k = qkv[poff:poff + 32, 2 + j, :]
                        eT = scr.tile([128, 4, 500], bf16, tag="eT")
                        for skc in range(4):
                            psc = pm2.tile([128, 512], f32, tag="pm2")
                            mm(out=psc[:125, 0:500],
                               lhsT=h_k[:, skc * 125: skc * 125 + 125],
                               rhs=h_q, start=True, stop=True,
                               tile_position=(poff, 0))
                            nc.scalar.activation(
                                out=eT[:125, skc, :],
                                in_=psc[:125, 0:500], func=AF.Exp, scale=SC32)
                        for skc in range(4):
                            mm(out=pcs4[0:4, 0:500], lhsT=C["onesZc"][:125, c4, :],
                               rhs=eT[:125, skc, :],
                               start=(c4 == 0 and skc == 0),
                               stop=(c4 == 3 and skc == 3))
                        for skc in range(4):
                            mm(out=pot4[poff:poff + 32, :],
                               lhsT=vt[:125, skc, j * 128 + poff: j * 128 + poff + 32],
                               rhs=eT[:125, skc, :],
                               start=(skc == 0), stop=(skc == 3),
                               tile_position=(0, poff))
                    rrf = sc.tile([4, 500], f32, tag="rrf")
                    nc.vector.reciprocal_approx_fast(out=rrf[:],
                                                     in_=pcs4[0:4, 0:500])
                    rrb4 = sc.tile([4, 500], bf16, tag="rrb4")
                    nc.vector.tensor_copy(rrb4[:], rrf[:])
                    prbF = pm2.tile([128, 512], f32, tag="pm2")
                    mm(out=prbF[:, 0:500], lhsT=C["e4m"][:], rhs=rrb4[:],
                       start=True, stop=True)
                    rbsF = scr2.tile([128, 500], bf16, tag="rbs")
                    nc.scalar.activation(out=rbsF[:], in_=prbF[:, 0:500],
                                         func=AF.Copy)
                    nc.vector.tensor_tensor(out=attn[:, j, cs:cs + 500],
                                            in0=pot4[:], in1=rbsF[:], op=A.mult)
            for mb in range(2):
                for nch in range(4):
                    po = pm1.tile([128, 500], f32, tag="pm1")
                    for kc in range(2):
                        mm(out=po[:], lhsT=W["woT"][:, kc, mb * 128:(mb + 1) * 128],
                           rhs=attn[:, kc, nch * 500:(nch + 1) * 500],
                           start=(kc == 0), stop=(kc == 1))
                    nc.vector.tensor_tensor(out=x[:, mb, nch * 500:(nch + 1) * 500],
                                            in0=x[:, mb, nch * 500:(nch + 1) * 500],
                                            in1=po[:], op=A.add)
                nc.vector.tensor_scalar(out=x[:, mb, :], in0=x[:, mb, :],
                                        scalar1=W["bov"][:, mb:mb + 1], scalar2=None,
                                        op0=A.add)
            xn2 = big.tile([128, 2, 2000], bf16, tag="xn")
            emit_ln("ln2g", "ln2b", xn2)
            for s in range(ns_run):
                cs = s * 500
                fh = scr.tile([128, 8, 500], bf16, tag="fh")
                for hb in range(8):
                    phh = pm1.tile([128, 500], f32, tag="pm1")
                    for kc in range(2):
                        mm(out=phh[:], lhsT=W["w1TT"][:, kc, hb * 128:(hb + 1) * 128],
                           rhs=xn2[:, kc, cs:cs + 500], start=(kc == 0),
                           stop=(kc == 1))
                    nc.scalar.activation(out=fh[:, hb, :], in_=phh[:], func=AF.Gelu,
                                         bias=W["b1v"][:, hb:hb + 1])
                for mb in range(2):
                    pf = pm1.tile([128, 500], f32, tag="pm1")
                    for hb in range(8):
                        mm(out=pf[:], lhsT=W["w2TT"][:, hb, mb * 128:(mb + 1) * 128],
                           rhs=fh[:, hb, :], start=(hb == 0), stop=(hb == 7))
                    nc.vector.tensor_tensor(out=x[:, mb, cs:cs + 500],
                                            in0=x[:, mb, cs:cs + 500], in1=pf[:],
                                            op=A.add)
            for mb in range(2):
                nc.vector.tensor_scalar(out=x[:, mb, :], in0=x[:, mb, :],
                                        scalar1=W["b2v"][:, mb:mb + 1], scalar2=None,
                                        op0=A.add)

        # ================= pooling + head =================
        for s in range(ns_run):
            cs = s * 500
            plg = pst.tile([1, 2, 512], f32, tag="pst")
            for blk in range(2):
                mm(out=plg[:1, 0, 0:500], lhsT=C["poolq"][:, blk:blk + 1],
                   rhs=x[:, blk, cs:cs + 500], start=(blk == 0), stop=(blk == 1))
            wrow = sc.tile([1, 500], f32, tag="wrow")
            nc.scalar.activation(out=wrow[:], in_=plg[:1, 0, 0:500], func=AF.Exp,
                                 scale=1.0 / 16.0)
            pwb = pm1.tile([128, 500], f32, tag="pm1")
            mm(out=pwb[:], lhsT=ones1f[:], rhs=wrow[:], start=True, stop=True)
            wx = scr2.tile([128, 2, 500], f32, tag="sq")
            for blk in range(2):
                nc.vector.tensor_tensor(out=wx[:, blk, :], in0=x[:, blk, cs:cs + 500],
                                        in1=pwb[:], op=A.mult)
            pooled = sc.tile([128, 4], f32, tag="pooled")
            nc.vector.tensor_reduce(out=pooled[:, 0:2], in_=wx[:], axis=AX.X,
                                    op=A.add)
            nc.scalar.activation(out=pooled[:, 2:4], in_=pooled[:, 0:2],
                                 func=AF.Square)
            pps = pst.tile([1, 2, 512], f32, tag="pst")
            mm(out=pps[:1, 0, 0:4], lhsT=onesPf[:], rhs=pooled[:], start=True,
               stop=True)
            z = sc.tile([1, 16], f32, tag="z")
            nc.vector.tensor_copy(z[:, 12:16], pps[:1, 0, 0:4])
            nc.vector.tensor_tensor(out=z[:, 0:1], in0=z[:, 12:13],
                                    in1=z[:, 13:14], op=A.add)
            nc.vector.tensor_tensor(out=z[:, 1:2], in0=z[:, 14:15],
                                    in1=z[:, 15:16], op=A.add)
            nc.vector.tensor_scalar(out=z[:, 2:3], in0=z[:, 0:1],
                                    scalar1=1.0 / 256.0, scalar2=None, op0=A.mult)
            nc.vector.tensor_tensor(out=z[:, 3:4], in0=z[:, 2:3], in1=z[:, 2:3],
                                    op=A.mult)
            nc.vector.tensor_scalar(out=z[:, 4:5], in0=z[:, 1:2],
                                    scalar1=1.0 / 256.0, scalar2=None, op0=A.mult)
            nc.vector.tensor_tensor(out=z[:, 4:5], in0=z[:, 4:5], in1=z[:, 3:4],
                                    op=A.subtract)
            nc.scalar.activation(out=z[:, 5:6], in_=z[:, 4:5], func=AF.Sqrt,
                                 bias=epsv[0:1, :])
            nc.vector.reciprocal_approx_fast(out=z[:, 6:7], in_=z[:, 5:6])
            cp = sc.tile([128, 2], f32, tag="cp")
            nc.vector.tensor_tensor(out=cp[:], in0=pooled[:, 0:2], in1=C["c1v"][:],
                                    op=A.mult)
            pa = pst.tile([1, 2, 512], f32, tag="pst")
            mm(out=pa[:1, 0, 0:2], lhsT=onesPf[:], rhs=cp[:], start=True, stop=True)
            nc.vector.tensor_copy(z[:, 10:12], pa[:1, 0, 0:2])
            nc.vector.tensor_tensor(out=z[:, 7:8], in0=z[:, 10:11],
                                    in1=z[:, 11:12], op=A.add)
            nc.vector.tensor_scalar(out=z[:, 8:9], in0=z[:, 2:3], scalar1=sc1,
                                    scalar2=None, op0=A.mult)
            nc.vector.tensor_tensor(out=z[:, 8:9], in0=z[:, 7:8], in1=z[:, 8:9],
                                    op=A.subtract)
            nc.vector.tensor_tensor(out=z[:, 8:9], in0=z[:, 8:9], in1=z[:, 6:7],
                                    op=A.mult)
            nc.vector.tensor_scalar(out=z[:, 9:10], in0=z[:, 8:9], scalar1=c2s,
                                    scalar2=None, op0=A.add)
            nc.sync.dma_start(out=Y[s:s + 1, :], in_=z[:, 9:10])

        for p in [pst, pm2, scr2, scr, big, wpool, pm1, bigx, sc, sing]:
            p.release()

    nc.compile()
    return nc


_BUILT = {}


def _get_nc(key, **kw):
    if key not in _BUILT:
        _BUILT[key] = _build(**kw)
    return _BUILT[key]


def _make_in_maps(prep, ns=NS, ncores=NCORES):
    in_maps = []
    for c in range(ncores):
        sl = slice(c * ns, (c + 1) * ns)
        m = {k: prep[k] for k in CONST_KEYS}
        m["mel4"] = prep["_mel"][sl]
        m["evrows"] = prep["_evrows"][sl]
        m["tposv"] = prep["_tposv"][sl]
        m["keepv"] = prep["_keepv"][sl]
        m["starbias"] = prep["_starbias"][sl]
        in_maps.append(m)
    return in_maps


def kernel(**inputs):
    from concourse.bass_utils import run_bass_kernel_spmd

    prep = _host_prep(inputs)
    nc = _get_nc("full", nl_run=NLAYERS, ns_run=NS, debug=False,
                 sc1=prep["sc1"], c2s=prep["c2s"])
    res = run_bass_kernel_spmd(nc, _make_in_maps(prep), list(range(NCORES)))
    y = np.concatenate([res.results[c]["y"].reshape(-1) for c in range(NCORES)])
    return y.astype(np.float32)


# revision 16
# speedup vs baseline: 1.1290x; 1.0142x over previous
"""Trainium2 Bass kernel for nn_ChartQualityEvaluator.

Data parallel: 32 samples -> 8 cores x 4 samples. Feature-major activations
[128 part, 2 blocks, 4*500 cols]. Matmuls and bulk activations in bf16
(fp32 PSUM accumulation); stats/sinusoid paths stay fp32. Unsafe softmax,
LN via ones-matmul stats + PE broadcast, event scatter via onehot matmul.
Host precomputes index-like preprocessing with f32-exact semantics.
"""
import math
import sys

import numpy as np
import ml_dtypes

_TRN = "/opt/trn_rl_repo"
if _TRN not in sys.path:
    sys.path.insert(0, _TRN)

BF16 = ml_dtypes.bfloat16

D = 256
H = 8
NLAYERS = 6
HALF = 128
S = 500
NEV = 256
NCORES = 8
NS = 4  # samples per core
B = 32
EPS = 1e-5
INV2PI = float(np.float32(1.0 / (2.0 * math.pi)))
TWOPI = 2.0 * math.pi
SC32 = float(np.float32(1.0 / math.sqrt(32.0)))


def _host_prep(inp):
    f = np.float32
    out = {}

    def t2(v):  # [256] -> [128,2]
        return np.ascontiguousarray(np.asarray(v).reshape(2, 128).T.astype(f))

    def b(a):  # -> bf16
        return np.ascontiguousarray(np.asarray(a).astype(np.float32).astype(BF16))

    out["wmelT"] = b(np.asarray(inp["mel_W"]).T)
    out["melb"] = np.ascontiguousarray(np.asarray(inp["mel_b"]).reshape(16, 1).astype(f))
    out["w1t"] = b(np.asarray(inp["conv1_w"]).transpose(1, 2, 0))
    out["c1b"] = np.ascontiguousarray(np.asarray(inp["conv1_b"]).reshape(128, 1).astype(f))
    out["gng"] = np.ascontiguousarray(np.asarray(inp["gn_g"]).reshape(128, 1).astype(f))
    out["gnb"] = np.ascontiguousarray(np.asarray(inp["gn_b"]).reshape(128, 1).astype(f))
    out["w2t"] = b(np.asarray(inp["conv2_w"]).transpose(1, 2, 0))
    out["c2b"] = t2(inp["conv2_b"])
    out["cng"] = t2(inp["cn_g"])

    freq = np.exp(np.arange(HALF, dtype=f) * f(-math.log(10000.0) / (HALF - 1)))
    e32 = (np.arange(S, dtype=f)[None, :] * freq[:, None]).astype(f)
    e64 = e32.astype(np.float64)
    pos_fm = np.concatenate([np.sin(e64), np.cos(e64)], axis=0)  # [256,500]
    out["posT"] = np.ascontiguousarray(
        pos_fm.reshape(2, 128, S).transpose(1, 0, 2).astype(f))
    out["freqv"] = np.ascontiguousarray(freq.reshape(128, 1))

    out["epW1T"] = b(np.asarray(inp["ep_W1"]).T.reshape(6, 128, 256).transpose(1, 0, 2))
    out["epb1"] = t2(inp["ep_b1"])
    out["epW2T"] = b(np.asarray(inp["ep_W2"]).T.reshape(2, 128, 256).transpose(1, 0, 2))
    out["epb2row"] = np.ascontiguousarray(
        np.tile(np.asarray(inp["ep_b2"]).astype(f)[None, :], (128, 1)))

    def wT(w, kc, m):  # w [m, k] -> [128, kc, m] bf16
        return b(np.asarray(w).T.reshape(kc, 128, m).transpose(1, 0, 2))

    out["wqkvT"] = np.stack([wT(inp["tl_Wqkv"][i], 2, 768) for i in range(NLAYERS)])
    out["bvrow"] = np.stack([b(np.asarray(inp["tl_bqkv"][i])[512:768].reshape(1, 256))
                             for i in range(NLAYERS)])
    out["b2row"] = np.stack([b(np.asarray(inp["tl_b2"][i]).reshape(1, 256))
                             for i in range(NLAYERS)])
    out["bqkv"] = np.stack([np.ascontiguousarray(
        np.asarray(inp["tl_bqkv"][i]).reshape(6, 128).T.astype(f))
        for i in range(NLAYERS)])
    out["woT"] = np.stack([wT(inp["tl_Wo"][i], 2, 256) for i in range(NLAYERS)])
    out["bov"] = np.stack([t2(inp["tl_bo"][i]) for i in range(NLAYERS)])
    out["ln1g"] = np.stack([t2(inp["tl_ln1g"][i]) for i in range(NLAYERS)])
    out["ln1b"] = np.stack([t2(inp["tl_ln1b"][i]) for i in range(NLAYERS)])
    out["ln2g"] = np.stack([t2(inp["tl_ln2g"][i]) for i in range(NLAYERS)])
    out["ln2b"] = np.stack([t2(inp["tl_ln2b"][i]) for i in range(NLAYERS)])
    out["w1TT"] = np.stack([wT(inp["tl_W1"][i], 2, 1024) for i in range(NLAYERS)])
    out["b1v"] = np.stack([np.ascontiguousarray(
        np.asarray(inp["tl_b1"][i]).reshape(8, 128).T.astype(f))
        for i in range(NLAYERS)])
    out["w2TT"] = np.stack([wT(inp["tl_W2"][i], 8, 256) for i in range(NLAYERS)])
    out["b2v"] = np.stack([t2(inp["tl_b2"][i]) for i in range(NLAYERS)])

    out["poolq"] = t2(inp["pool_q"])
    oz = np.zeros((128, 4, 4), np.float32)
    for c4 in range(4):
        oz[:, c4, c4] = 1.0
    out["onesZc"] = b(oz)
    e4 = np.zeros((4, 128), np.float32)
    for c4 in range(4):
        e4[c4, 32 * c4:32 * c4 + 32] = 1.0
    out["e4m"] = b(e4)
    c1 = (np.asarray(inp["oh_W"])[0] * np.asarray(inp["on_g"])).astype(f)
    out["c1v"] = t2(c1)
    out["sc1"] = float(c1.astype(np.float64).sum())
    out["c2s"] = float((np.asarray(inp["oh_W"])[0].astype(np.float64)
                        * np.asarray(inp["on_b"]).astype(np.float64)).sum()
                       + float(np.asarray(inp["oh_b"])[0]))

    events = np.asarray(inp["events"]).astype(np.int64)
    mask = np.asarray(inp["event_mask"])
    star = np.asarray(inp["star_rating"]).astype(f)
    nb = events.shape[0]
    diff = np.maximum(events[:, 1:] - events[:, :-1], 1)
    g = np.concatenate([diff[:, :1], diff], axis=1)
    gap_ms = (g * 5).astype(f)
    g_f = np.maximum(g.astype(f), f(1.0))
    r = np.clip(g_f[:, 1:] / g_f[:, :-1], f(0.1), f(10.0)).astype(f)
    ones = np.ones((nb, 1), f)
    rb50 = np.trunc(np.concatenate([ones, r], axis=1) * f(50.0)).astype(f)
    ra50 = np.trunc(np.concatenate([r, ones], axis=1) * f(50.0)).astype(f)
    out["_evrows"] = np.ascontiguousarray(np.stack([rb50, ra50, gap_ms], axis=1))
    tp = np.clip(events // 4, 0, S - 1).astype(f)
    keep = (1.0 - mask.astype(f)).astype(f)
    out["_tposv"] = np.ascontiguousarray(tp.reshape(nb, 2, 128).transpose(0, 2, 1))
    out["_keepv"] = np.ascontiguousarray(keep.reshape(nb, 2, 128).transpose(0, 2, 1))
    bucket = np.clip((star / f(0.5)).astype(np.int32), 0, 19)
    sb = (np.asarray(inp["cn_b"])[None, :] + np.asarray(inp["star_table"])[bucket]).astype(f)
    out["_starbias"] = np.ascontiguousarray(sb.reshape(nb, 2, 128).transpose(0, 2, 1))
    out["_mel"] = np.ascontiguousarray(np.asarray(inp["mel"]).astype(f).astype(BF16))
    return out


CONST_KEYS = ["wmelT", "melb", "w1t", "c1b", "gng", "gnb", "w2t", "c2b", "cng",
              "posT", "freqv", "epW1T", "epb1", "epW2T", "epb2row",
              "wqkvT", "bqkv", "woT", "bov", "ln1g", "ln1b", "ln2g", "ln2b",
              "w1TT", "b1v", "w2TT", "b2v", "poolq", "c1v", "onesZc", "e4m",
              "bvrow", "b2row"]

# params that are bf16 on device
BF_KEYS = {"wmelT", "w1t", "w2t", "epW1T", "epW2T",
           "wqkvT", "woT", "w1TT", "w2TT", "mel4", "onesZc", "e4m", "bvrow",
           "b2row"}


def _build(nl_run=NLAYERS, ns_run=NS, debug=False, sc1=0.0, c2s=0.0):
    import concourse.bacc as bacc
    import concourse.tile as tile
    from concourse import mybir
    from concourse.masks import make_identity

    f32 = mybir.dt.float32
    bf16 = mybir.dt.bfloat16
    A = mybir.AluOpType
    AF = mybir.ActivationFunctionType
    AX = mybir.AxisListType

    nc = bacc.Bacc(None)

    def mm(out, lhsT, rhs, **kw):
        nc.tensor.matmul(out=out, lhsT=lhsT, rhs=rhs, **kw)

    P = {}
    shapes = dict(
        mel4=[ns_run, 80, 2000], evrows=[ns_run, 3, 256], tposv=[ns_run, 128, 2],
        keepv=[ns_run, 128, 2], starbias=[ns_run, 128, 2],
        wmelT=[80, 16], melb=[16, 1], w1t=[16, 7, 128], c1b=[128, 1],
        gng=[128, 1], gnb=[128, 1], w2t=[128, 7, 256], c2b=[128, 2],
        cng=[128, 2], posT=[128, 2, 500], freqv=[128, 1],
        epW1T=[128, 6, 256], epb1=[128, 2], epW2T=[128, 2, 256], epb2row=[128, 256],
        wqkvT=[NLAYERS, 128, 2, 768], bqkv=[NLAYERS, 128, 6],
        bvrow=[NLAYERS, 1, 256], b2row=[NLAYERS, 1, 256],
        woT=[NLAYERS, 128, 2, 256], bov=[NLAYERS, 128, 2],
        ln1g=[NLAYERS, 128, 2], ln1b=[NLAYERS, 128, 2],
        ln2g=[NLAYERS, 128, 2], ln2b=[NLAYERS, 128, 2],
        w1TT=[NLAYERS, 128, 2, 1024], b1v=[NLAYERS, 128, 8],
        w2TT=[NLAYERS, 128, 8, 256], b2v=[NLAYERS, 128, 2],
        poolq=[128, 2], c1v=[128, 2], onesZc=[128, 4, 4], e4m=[4, 128],
    )
    for k, sh in shapes.items():
        P[k] = nc.declare_dram_parameter(k, sh, bf16 if k in BF_KEYS else f32,
                                         isOutput=False)
    Y = nc.declare_dram_parameter("y", [ns_run, 1], f32, isOutput=True)

    with tile.TileContext(nc) as tc:
        sing = tc.alloc_tile_pool(name="sing", bufs=1)
        sc = tc.alloc_tile_pool(name="sc", bufs=1)
        bigx = tc.alloc_tile_pool(name="bigx", bufs=1)
        pm1 = tc.alloc_tile_pool(name="pm1", bufs=3, space="PSUM")
        pm2 = tc.alloc_tile_pool(name="pm2", bufs=2, space="PSUM")
        pst = tc.alloc_tile_pool(name="pst", bufs=1, space="PSUM")

        C = {}
        for k in ["wmelT", "melb", "w1t", "c1b", "gng", "gnb", "w2t", "c2b",
                  "cng", "posT", "freqv", "epW1T", "epb1", "epW2T", "epb2row",
                  "poolq", "c1v", "onesZc", "e4m"]:
            C[k] = sing.tile(shapes[k], bf16 if k in BF_KEYS else f32,
                             tag=k, name="c_" + k)
            nc.sync.dma_start(out=C[k][:], in_=P[k][:])
        identb = sing.tile([128, 128], bf16, tag="identb")
        make_identity(nc, identb[:])
        onesPf = sing.tile([128, 1], f32, tag="onesPf")
        nc.vector.memset(onesPf[:], 1.0)
        onesPb = sing.tile([128, 1], bf16, tag="onesPb")
        nc.vector.memset(onesPb[:], 1.0)
        ones1f = sing.tile([1, 128], f32, tag="ones1f")
        nc.vector.memset(ones1f[:], 1.0)
        ones1b = sing.tile([1, 128], bf16, tag="ones1b")
        nc.vector.memset(ones1b[:], 1.0)
        onesR = sing.tile([1, 512], bf16, tag="onesR")
        nc.vector.memset(onesR[:], 1.0)
        zerov = sing.tile([128, 1], f32, tag="zerov")
        nc.vector.memset(zerov[:], 0.0)
        epsv = sing.tile([128, 1], f32, tag="epsv")
        nc.vector.memset(epsv[:], EPS)
        iotaB = sing.tile([128, 500], f32, tag="iotaB")
        nc.gpsimd.iota(iotaB[:], pattern=[[1, 500]], base=0, channel_multiplier=0,
                       allow_small_or_imprecise_dtypes=True)

        x = bigx.tile([128, 2, 2000], f32, tag="x_fm")

        def s2(t):  # step-2 view of [p, n] -> [p, n//2]
            return t.rearrange("p (t s) -> p s t", s=2)[:, 0, :]

        # small-vector LN stat helper: psum [1,2,500] (s,ss) -> mr (m, r); also
        # writes mrb (bf16 copy of [m, r]) for cheap broadcast matmuls.
        def emit_stats(pstt, mr, mrb, tmp, scale):
            nc.vector.tensor_scalar(out=mr[:1, 0:2, :], in0=pstt[:1, 0:2, 0:500],
                                    scalar1=scale, scalar2=None, op0=A.mult)
            nc.vector.tensor_tensor(out=tmp[:1, 0, :], in0=mr[:1, 0, :],
                                    in1=mr[:1, 0, :], op=A.mult)
            nc.vector.tensor_tensor(out=tmp[:1, 1, :], in0=mr[:1, 1, :],
                                    in1=tmp[:1, 0, :], op=A.subtract)
            nc.scalar.activation(out=tmp[:1, 0, :], in_=tmp[:1, 1, :], func=AF.Sqrt,
                                 bias=epsv[0:1, :])
            nc.vector.reciprocal_approx_fast(out=mr[:1, 1, :], in_=tmp[:1, 0, :])
            nc.vector.tensor_copy(mrb[:1, 0:2, :], mr[:1, 0:2, :])

        # ================= front end =================
        fr = tc.alloc_tile_pool(name="fr", bufs=2)
        for s in range(ns_run):
            cs = s * 500
            melp = fr.tile([80, 2006], bf16, tag="melp")
            nc.vector.memset(melp[:, 0:3], 0.0)
            nc.vector.memset(melp[:, 2003:2006], 0.0)
            nc.sync.dma_start(out=melp[:, 3:2003], in_=P["mel4"][s])
            xmelp = fr.tile([16, 2006], bf16, tag="xmelp")
            nc.vector.memset(xmelp[:, 0:3], 0.0)
            nc.vector.memset(xmelp[:, 2003:2006], 0.0)
            for nch in range(4):
                pcm = pm1.tile([128, 500], f32, tag="pm1")
                mm(out=pcm[:16, :], lhsT=C["wmelT"][:],
                   rhs=melp[:, 3 + nch * 500: 3 + nch * 500 + 500],
                   start=True, stop=True)
                nc.scalar.activation(out=xmelp[:, 3 + nch * 500: 3 + nch * 500 + 500],
                                     in_=pcm[:16, :], func=AF.Identity,
                                     bias=C["melb"][:, 0:1])
            h1g = fr.tile([128, 2, 500], bf16, tag="h1g")
            stg = fr.tile([128, 4], f32, tag="stg")
            for half in range(2):
                pc1 = pm2.tile([128, 512], f32, tag="pm2")
                for k in range(7):
                    mm(out=pc1[:, 0:500], lhsT=C["w1t"][:, k, :],
                       rhs=s2(xmelp[:, k + half * 1000: k + half * 1000 + 1000]),
                       start=(k == 0), stop=(k == 6))
                nc.scalar.activation(out=h1g[:, half, :], in_=pc1[:, 0:500],
                                     func=AF.Gelu, bias=C["c1b"][:, 0:1],
                                     accum_out=stg[:, half:half + 1])
            sqf = fr.tile([128, 2, 500], f32, tag="sqf")
            nc.scalar.activation(out=sqf[:], in_=h1g[:], func=AF.Square,
                                 accum_out=stg[:, 2:3])
            pg = pst.tile([1, 2, 512], f32, tag="pst")
            mm(out=pg[:1, 0, 0:3], lhsT=onesPf[:], rhs=stg[:, 0:3], start=True,
               stop=True)
            sn = sc.tile([1, 8], f32, tag="sn")
            nc.vector.tensor_scalar(out=sn[:, 0:2], in0=pg[:1, 0, 1:3],
                                    scalar1=1.0 / 128000.0, scalar2=None, op0=A.mult)
            nc.vector.tensor_scalar(out=sn[:, 6:7], in0=pg[:1, 0, 0:1],
                                    scalar1=1.0 / 128000.0, scalar2=None, op0=A.mult)
            nc.vector.tensor_tensor(out=sn[:, 0:1], in0=sn[:, 0:1], in1=sn[:, 6:7],
                                    op=A.add)
            nc.vector.tensor_tensor(out=sn[:, 2:3], in0=sn[:, 0:1], in1=sn[:, 0:1],
                                    op=A.mult)
            nc.vector.tensor_tensor(out=sn[:, 3:4], in0=sn[:, 1:2], in1=sn[:, 2:3],
                                    op=A.subtract)
            nc.scalar.activation(out=sn[:, 4:5], in_=sn[:, 3:4], func=AF.Sqrt,
                                 bias=epsv[0:1, :])
            nc.vector.reciprocal_approx_fast(out=sn[:, 1:2], in_=sn[:, 4:5])
            pgb = pm1.tile([128, 500], f32, tag="pm1")
            mm(out=pgb[:, 0:2], lhsT=ones1f[:], rhs=sn[:, 0:2], start=True, stop=True)
            sv = sc.tile([128, 2], f32, tag="sv")
            nc.vector.tensor_tensor(out=sv[:, 0:1], in0=pgb[:, 1:2], in1=C["gng"][:],
                                    op=A.mult)
            nc.vector.tensor_tensor(out=sv[:, 1:2], in0=pgb[:, 0:1], in1=sv[:, 0:1],
                                    op=A.mult)
            nc.vector.tensor_tensor(out=sv[:, 1:2], in0=C["gnb"][:], in1=sv[:, 1:2],
                                    op=A.subtract)
            x2p = fr.tile([128, 1006], bf16, tag="x2p")
            nc.vector.memset(x2p[:, 0:3], 0.0)
            nc.vector.memset(x2p[:, 1003:1006], 0.0)
            nc.scalar.activation(out=x2p[:, 3:1003],
                                 in_=h1g.rearrange("p a b -> p (a b)"),
                                 func=AF.Identity, scale=sv[:, 0:1], bias=sv[:, 1:2])
            for mb in range(2):
                pc2 = pm2.tile([128, 512], f32, tag="pm2")
                for k in range(7):
                    mm(out=pc2[:, 0:500],
                       lhsT=C["w2t"][:, k, mb * 128:(mb + 1) * 128],
                       rhs=s2(x2p[:, k:k + 1000]),
                       start=(k == 0), stop=(k == 6))
                nc.scalar.activation(out=x[:, mb, cs:cs + 500], in_=pc2[:, 0:500],
                                     func=AF.Gelu, bias=C["c2b"][:, mb:mb + 1])
            # CN layernorm + starbias + pos
            sbv = fr.tile([128, 2], f32, tag="sbv")
            nc.sync.dma_start(out=sbv[:], in_=P["starbias"][s])
            nc.scalar.activation(out=sqf[:], in_=x[:, :, cs:cs + 500], func=AF.Square)
            pstt = pst.tile([1, 2, 512], f32, tag="pst")
            for blk in range(2):
                mm(out=pstt[:1, 0, 0:500], lhsT=onesPf[:],
                   rhs=x[:, blk, cs:cs + 500], start=(blk == 0), stop=(blk == 1))
            for blk in range(2):
                mm(out=pstt[:1, 1, 0:500], lhsT=onesPf[:],
                   rhs=sqf[:, blk, :], start=(blk == 0), stop=(blk == 1))
            mr = sc.tile([1, 2, 500], f32, tag="mr")
            mrb = sc.tile([1, 2, 500], bf16, tag="mrb")
            tmp = sc.tile([1, 2, 500], f32, tag="tmp1")
            emit_stats(pstt, mr, mrb, tmp, 1.0 / 256.0)
            pbcM = pm2.tile([128, 512], f32, tag="pm2")
            mm(out=pbcM[:, 0:500], lhsT=ones1b[:], rhs=mrb[:1, 0, :],
               start=True, stop=True)
            pbcR = pm2.tile([128, 512], f32, tag="pm2")
            mm(out=pbcR[:, 0:500], lhsT=ones1b[:], rhs=mrb[:1, 1, :],
               start=True, stop=True)
            for blk in range(2):
                nc.vector.tensor_tensor(out=x[:, blk, cs:cs + 500],
                                        in0=x[:, blk, cs:cs + 500],
                                        in1=pbcM[:, 0:500], op=A.subtract)
                nc.vector.tensor_tensor(out=x[:, blk, cs:cs + 500],
                                        in0=x[:, blk, cs:cs + 500],
                                        in1=pbcR[:, 0:500], op=A.mult)
                nc.scalar.activation(out=x[:, blk, cs:cs + 500],
                                     in_=x[:, blk, cs:cs + 500], func=AF.Identity,
                                     scale=C["cng"][:, blk:blk + 1],
                                     bias=sbv[:, blk:blk + 1])
            nc.vector.tensor_tensor(out=x[:, :, cs:cs + 500], in0=x[:, :, cs:cs + 500],
                                    in1=C["posT"][:], op=A.add)

            # events
            evr = fr.tile([1, 3, 256], f32, tag="evr")
            nc.sync.dma_start(out=evr[:], in_=P["evrows"][s])
            tpv = fr.tile([128, 2], f32, tag="tpv")
            nc.sync.dma_start(out=tpv[:], in_=P["tposv"][s])
            kpv = fr.tile([128, 2], f32, tag="kpv")
            nc.sync.dma_start(out=kpv[:], in_=P["keepv"][s])
            comb = fr.tile([128, 6, 256], bf16, tag="comb")
            for vr in range(3):
                pb = pm1.tile([128, 500], f32, tag="pm1")
                mm(out=pb[:, 0:256], lhsT=ones1f[:], rhs=evr[:1, vr, :],
                   start=True, stop=True)
                arg = fr.tile([128, 256], f32, tag="arg")
                nc.scalar.activation(out=arg[:], in_=pb[:, 0:256], func=AF.Copy,
                                     scale=C["freqv"][:])
                nc.vector.tensor_scalar(out=arg[:], in0=arg[:], scalar1=INV2PI,
                                        scalar2=None, op0=A.mult)
                w1_ = fr.tile([128, 256], f32, tag="w1_")
                ti_ = fr.tile([128, 256], mybir.dt.int32, tag="ti_")
                tf_ = fr.tile([128, 256], f32, tag="tf_")
                nc.vector.tensor_copy(ti_[:], arg[:])
                nc.vector.tensor_copy(tf_[:], ti_[:])
                nc.vector.tensor_tensor(out=w1_[:], in0=arg[:], in1=tf_[:],
                                        op=A.subtract)
                nc.scalar.activation(out=comb[:, 2 * vr, :], in_=w1_[:], func=AF.Sin,
                                     scale=TWOPI, bias=zerov[:])
                nc.vector.tensor_scalar(out=arg[:], in0=arg[:], scalar1=0.25,
                                        scalar2=None, op0=A.add)
                nc.vector.tensor_copy(ti_[:], arg[:])
                nc.vector.tensor_copy(tf_[:], ti_[:])
                nc.vector.tensor_tensor(out=w1_[:], in0=arg[:], in1=tf_[:],
                                        op=A.subtract)
                nc.scalar.activation(out=comb[:, 2 * vr + 1, :], in_=w1_[:],
                                     func=AF.Sin, scale=TWOPI, bias=zerov[:])
            hmid = fr.tile([128, 2, 256], bf16, tag="hmid")
            for mb in range(2):
                ph = pm1.tile([128, 500], f32, tag="pm1")
                for kc in range(6):
                    mm(out=ph[:, 0:256],
                       lhsT=C["epW1T"][:, kc, mb * 128:(mb + 1) * 128],
                       rhs=comb[:, kc, :], start=(kc == 0), stop=(kc == 5))
                nc.scalar.activation(out=hmid[:, mb, :], in_=ph[:, 0:256],
                                     func=AF.Gelu, bias=C["epb1"][:, mb:mb + 1])
            evt = fr.tile([128, 2, 256], bf16, tag="evt")
            for ec in range(2):
                pe = pm1.tile([128, 500], f32, tag="pm1")
                for kc in range(2):
                    mm(out=pe[:, 0:256],
                       lhsT=hmid[:, kc, ec * 128:(ec + 1) * 128],
                       rhs=C["epW2T"][:, kc, :], start=(kc == 0), stop=(kc == 1))
                nc.vector.tensor_tensor(out=evt[:, ec, :], in0=pe[:, 0:256],
                                        in1=C["epb2row"][:], op=A.add)
                nc.vector.tensor_scalar(out=evt[:, ec, :], in0=evt[:, ec, :],
                                        scalar1=kpv[:, ec:ec + 1], scalar2=None,
                                        op0=A.mult)
            oh = fr.tile([128, 2, 500], bf16, tag="oh")
            for ec in range(2):
                nc.vector.tensor_scalar(out=oh[:, ec, :], in0=iotaB[:],
                                        scalar1=tpv[:, ec:ec + 1], scalar2=None,
                                        op0=A.is_equal)
            for mb in range(2):
                px = pm1.tile([128, 500], f32, tag="pm1")
                for ec in range(2):
                    mm(out=px[:], lhsT=evt[:, ec, mb * 128:(mb + 1) * 128],
                       rhs=oh[:, ec, :], start=(ec == 0), stop=(ec == 1))
                nc.vector.tensor_tensor(out=x[:, mb, cs:cs + 500],
                                        in0=x[:, mb, cs:cs + 500], in1=px[:], op=A.add)
        fr.release()
        wpool = tc.alloc_tile_pool(name="wpool", bufs=2)
        big = tc.alloc_tile_pool(name="big", bufs=1)
        scr = tc.alloc_tile_pool(name="scr", bufs=1)
        scr2 = tc.alloc_tile_pool(name="scr2", bufs=1)

        # ================= transformer =================
        for i in range(nl_run):
            W = {}
            for k, sh, dt in [("wqkvT", [128, 2, 768], bf16), ("bqkv", [128, 6], f32),
                              ("woT", [128, 2, 256], bf16), ("bov", [128, 2], f32),
                              ("ln1g", [128, 2], f32), ("ln1b", [128, 2], f32),
                              ("ln2g", [128, 2], f32), ("ln2b", [128, 2], f32),
                              ("w1TT", [128, 2, 1024], bf16), ("b1v", [128, 8], f32),
                              ("w2TT", [128, 8, 256], bf16), ("b2v", [128, 2], f32)]:
                W[k] = wpool.tile(sh, dt, tag="w_" + k, name=f"w{i}_" + k)
                nc.sync.dma_start(out=W[k][:], in_=P[k][i])
            bvr = wpool.tile([1, 256], bf16, tag="w_bvr", name=f"w{i}_bvr")
            nc.sync.dma_start(out=bvr[:], in_=P["bvrow"][i])
            b2r = wpool.tile([1, 256], bf16, tag="w_b2r", name=f"w{i}_b2r")
            nc.sync.dma_start(out=b2r[:], in_=P["b2row"][i])
            pvb = pm2.tile([128, 512], f32, tag="pm2")
            mm(out=pvb[:, 0:256], lhsT=ones1b[:], rhs=bvr[:], start=True, stop=True)
            vbF = wpool.tile([128, 256], bf16, tag="w_vbF", name=f"w{i}_vbF")
            nc.scalar.activation(out=vbF[:], in_=pvb[:, 0:256], func=AF.Copy)

            def emit_ln(gk, bk, xn):
                for nch in range(4):
                    co = nch * 500
                    sq = scr2.tile([128, 2, 500], f32, tag="sq")
                    nc.scalar.activation(out=sq[:], in_=x[:, :, co:co + 500],
                                         func=AF.Square)
                    pstt = pst.tile([1, 2, 512], f32, tag="pst")
                    for blk in range(2):
                        mm(out=pstt[:1, 0, 0:500], lhsT=onesPf[:],
                           rhs=x[:, blk, co:co + 500],
                           start=(blk == 0), stop=(blk == 1))
                    for blk in range(2):
                        mm(out=pstt[:1, 1, 0:500], lhsT=onesPf[:],
                           rhs=sq[:, blk, :], start=(blk == 0), stop=(blk == 1))
                    mr = sc.tile([1, 2, 500], f32, tag="mr")
                    mrb = sc.tile([1, 2, 500], bf16, tag="mrb")
                    tmp = sc.tile([1, 2, 500], f32, tag="tmp1")
                    emit_stats(pstt, mr, mrb, tmp, 1.0 / 256.0)
                    pbcM = pm2.tile([128, 512], f32, tag="pm2")
                    mm(out=pbcM[:, 0:500], lhsT=ones1b[:], rhs=mrb[:1, 0, :],
                       start=True, stop=True)
                    pbcR = pm2.tile([128, 512], f32, tag="pm2")
                    mm(out=pbcR[:, 0:500], lhsT=ones1b[:], rhs=mrb[:1, 1, :],
                       start=True, stop=True)
                    for blk in range(2):
                        nc.vector.tensor_tensor(out=xn[:, blk, co:co + 500],
                                                in0=x[:, blk, co:co + 500],
                                                in1=pbcM[:, 0:500], op=A.subtract)
                        nc.vector.tensor_tensor(out=xn[:, blk, co:co + 500],
                                                in0=xn[:, blk, co:co + 500],
                                                in1=pbcR[:, 0:500], op=A.mult)
                        nc.scalar.activation(out=xn[:, blk, co:co + 500],
                                             in_=xn[:, blk, co:co + 500],
                                             func=AF.Identity,
                                             scale=W[gk][:, blk:blk + 1],
                                             bias=W[bk][:, blk:blk + 1])

            xn = big.tile([128, 2, 2000], bf16, tag="xn")
            emit_ln("ln1g", "ln1b", xn)
            attn = big.tile([128, 2, 2000], bf16, tag="attn")
            for s in range(ns_run):
                cs = s * 500
                qkv = scr.tile([128, 4, 500], bf16, tag="qkv")
                vt = scr2.tile([128, 4, 256], bf16, tag="vt")
                for chunk in range(4):
                    pvt = pm2.tile([128, 512], f32, tag="pm2")
                    for kc in range(2):
                        mm(out=pvt[:125, 0:256],
                           lhsT=xn[:, kc, cs + chunk * 125: cs + chunk * 125 + 125],
                           rhs=W["wqkvT"][:, kc, 512:768],
                           start=(kc == 0), stop=(kc == 1))
                    nc.vector.tensor_tensor(out=vt[:125, chunk, :],
                                            in0=pvt[:125, 0:256], in1=vbF[:125, :],
                                            op=A.add)
                for j in range(4):
                    pq = pm1.tile([128, 500], f32, tag="pm1")
                    for kc in range(2):
                        mm(out=pq[:], lhsT=W["wqkvT"][:, kc, j * 128:(j + 1) * 128],
                           rhs=xn[:, kc, cs:cs + 500],
                           start=(kc == 0), stop=(kc == 1))
                    nc.vector.tensor_scalar(out=qkv[:, j, :], in0=pq[:],
                                            scalar1=W["bqkv"][:, j:j + 1],
                                            scalar2=None, op0=A.add)
                for j in range(2):
                    pot4 = pm1.tile([128, 500], f32, tag="pm1")
                    pcs4 = pst.tile([4, 512], f32, tag="pcs4")
                    for c4 in range(4):
                        poff = 32 * c4
                        h_q = qkv[poff:poff + 32, j, :]
                        h_k = qkv[poff:poff + 32, 2 + j, :]
                        eT = scr.tile([128, 4, 500], bf16, tag="eT")
                        for skc in range(4):
                            psc = pm2.tile([128, 512], f32, tag="pm2")
                            mm(out=psc[:125, 0:500],
                               lhsT=h_k[:, skc * 125: skc * 125 + 125],
                               rhs=h_q, start=True, stop=True,
                               tile_position=(poff, 0))
                            nc.scalar.activation(
                                out=eT[:125, skc, :],
                                in_=psc[:125, 0:500], func=AF.Exp, scale=SC32)
                        for skc in range(4):
                            mm(out=pcs4[0:4, 0:500], lhsT=C["onesZc"][:125, c4, :],
                               rhs=eT[:125, skc, :],
                               start=(c4 == 0 and skc == 0),
                               stop=(c4 == 3 and skc == 3))
                        for skc in range(4):
                            mm(out=pot4[poff:poff + 32, :],
                               lhsT=vt[:125, skc, j * 128 + poff: j * 128 + poff + 32],
                               rhs=eT[:125, skc, :],
                               start=(skc == 0), stop=(skc == 3),
                               tile_position=(0, poff))
                    rrf = sc.tile([4, 500], f32, tag="rrf")
                    nc.vector.reciprocal_approx_fast(out=rrf[:],
                                                     in_=pcs4[0:4, 0:500])
                    rrb4 = sc.tile([4, 500], bf16, tag="rrb4")
                    nc.vector.tensor_copy(rrb4[:], rrf[:])
                    prbF = pm2.tile([128, 512], f32, tag="pm2")
                    mm(out=prbF[:, 0:500], lhsT=C["e4m"][:], rhs=rrb4[:],
                       start=True, stop=True)
                    rbsF = scr2.tile([128, 500], bf16, tag="rbs")
                    nc.scalar.activation(out=rbsF[:], in_=prbF[:, 0:500],
                                         func=AF.Copy)
                    nc.vector.tensor_tensor(out=attn[:, j, cs:cs + 500],
                                            in0=pot4[:], in1=rbsF[:], op=A.mult)
            for mb in range(2):
                for nch in range(4):
                    po = pm1.tile([128, 500], f32, tag="pm1")
                    for kc in range(2):
                        mm(out=po[:], lhsT=W["woT"][:, kc, mb * 128:(mb + 1) * 128],
                           rhs=attn[:, kc, nch * 500:(nch + 1) * 500],
                           start=(kc == 0), stop=(kc == 1))
                    nc.vector.tensor_tensor(out=x[:, mb, nch * 500:(nch + 1) * 500],
                                            in0=x[:, mb, nch * 500:(nch + 1) * 500],
                                            in1=po[:], op=A.add)
                nc.vector.tensor_scalar(out=x[:, mb, :], in0=x[:, mb, :],
                                        scalar1=W["bov"][:, mb:mb + 1], scalar2=None,
                                        op0=A.add)
            xn2 = big.tile([128, 2, 2000], bf16, tag="xn")
            emit_ln("ln2g", "ln2b", xn2)
            for s in range(ns_run):
                cs = s * 500
                fh = scr.tile([128, 8, 500], bf16, tag="fh")
                for hb in range(8):
                    phh = pm1.tile([128, 500], f32, tag="pm1")
                    for kc in range(2):
                        mm(out=phh[:], lhsT=W["w1TT"][:, kc, hb * 128:(hb + 1) * 128],
                           rhs=xn2[:, kc, cs:cs + 500], start=(kc == 0),
                           stop=(kc == 1))
                    nc.scalar.activation(out=fh[:, hb, :], in_=phh[:], func=AF.Gelu,
                                         bias=W["b1v"][:, hb:hb + 1])
                for mb in range(2):
                    pf = pm1.tile([128, 500], f32, tag="pm1")
                    for hb in range(8):
                        mm(out=pf[:], lhsT=W["w2TT"][:, hb, mb * 128:(mb + 1) * 128],
                           rhs=fh[:, hb, :], start=(hb == 0), stop=False)
                    mm(out=pf[:], lhsT=b2r[0:1, mb * 128:(mb + 1) * 128],
                       rhs=onesR[0:1, 0:500], start=False, stop=True)
                    nc.vector.tensor_tensor(out=x[:, mb, cs:cs + 500],
                                            in0=x[:, mb, cs:cs + 500], in1=pf[:],
                                            op=A.add)

        # ================= pooling + head =================
        for s in range(ns_run):
            cs = s * 500
            plg = pst.tile([1, 2, 512], f32, tag="pst")
            for blk in range(2):
                mm(out=plg[:1, 0, 0:500], lhsT=C["poolq"][:, blk:blk + 1],
                   rhs=x[:, blk, cs:cs + 500], start=(blk == 0), stop=(blk == 1))
            wrow = sc.tile([1, 500], f32, tag="wrow")
            nc.scalar.activation(out=wrow[:], in_=plg[:1, 0, 0:500], func=AF.Exp,
                                 scale=1.0 / 16.0)
            pwb = pm1.tile([128, 500], f32, tag="pm1")
            mm(out=pwb[:], lhsT=ones1f[:], rhs=wrow[:], start=True, stop=True)
            wx = scr2.tile([128, 2, 500], f32, tag="sq")
            for blk in range(2):
                nc.vector.tensor_tensor(out=wx[:, blk, :], in0=x[:, blk, cs:cs + 500],
                                        in1=pwb[:], op=A.mult)
            pooled = sc.tile([128, 4], f32, tag="pooled")
            nc.vector.tensor_reduce(out=pooled[:, 0:2], in_=wx[:], axis=AX.X,
                                    op=A.add)
            nc.scalar.activation(out=pooled[:, 2:4], in_=pooled[:, 0:2],
                                 func=AF.Square)
            pps = pst.tile([1, 2, 512], f32, tag="pst")
            mm(out=pps[:1, 0, 0:4], lhsT=onesPf[:], rhs=pooled[:], start=True,
               stop=True)
            z = sc.tile([1, 16], f32, tag="z")
            nc.vector.tensor_copy(z[:, 12:16], pps[:1, 0, 0:4])
            nc.vector.tensor_tensor(out=z[:, 0:1], in0=z[:, 12:13],
                                    in1=z[:, 13:14], op=A.add)
            nc.vector.tensor_tensor(out=z[:, 1:2], in0=z[:, 14:15],
                                    in1=z[:, 15:16], op=A.add)
            nc.vector.tensor_scalar(out=z[:, 2:3], in0=z[:, 0:1],
                                    scalar1=1.0 / 256.0, scalar2=None, op0=A.mult)
            nc.vector.tensor_tensor(out=z[:, 3:4], in0=z[:, 2:3], in1=z[:, 2:3],
                                    op=A.mult)
            nc.vector.tensor_scalar(out=z[:, 4:5], in0=z[:, 1:2],
                                    scalar1=1.0 / 256.0, scalar2=None, op0=A.mult)
            nc.vector.tensor_tensor(out=z[:, 4:5], in0=z[:, 4:5], in1=z[:, 3:4],
                                    op=A.subtract)
            nc.scalar.activation(out=z[:, 5:6], in_=z[:, 4:5], func=AF.Sqrt,
                                 bias=epsv[0:1, :])
            nc.vector.reciprocal_approx_fast(out=z[:, 6:7], in_=z[:, 5:6])
            cp = sc.tile([128, 2], f32, tag="cp")
            nc.vector.tensor_tensor(out=cp[:], in0=pooled[:, 0:2], in1=C["c1v"][:],
                                    op=A.mult)
            pa = pst.tile([1, 2, 512], f32, tag="pst")
            mm(out=pa[:1, 0, 0:2], lhsT=onesPf[:], rhs=cp[:], start=True, stop=True)
            nc.vector.tensor_copy(z[:, 10:12], pa[:1, 0, 0:2])
            nc.vector.tensor_tensor(out=z[:, 7:8], in0=z[:, 10:11],
                                    in1=z[:, 11:12], op=A.add)
            nc.vector.tensor_scalar(out=z[:, 8:9], in0=z[:, 2:3], scalar1=sc1,
                                    scalar2=None, op0=A.mult)
            nc.vector.tensor_tensor(out=z[:, 8:9], in0=z[:, 7:8], in1=z[:, 8:9],
                                    op=A.subtract)
            nc.vector.tensor_tensor(out=z[:, 8:9], in0=z[:, 8:9], in1=z[:, 6:7],
                                    op=A.mult)
            nc.vector.tensor_scalar(out=z[:, 9:10], in0=z[:, 8:9], scalar1=c2s,
                                    scalar2=None, op0=A.add)
            nc.sync.dma_start(out=Y[s:s + 1, :], in_=z[:, 9:10])

        for p in [pst, pm2, scr2, scr, big, wpool, pm1, bigx, sc, sing]:
            p.release()

    nc.compile()
    return nc


_BUILT = {}


def _get_nc(key, **kw):
    if key not in _BUILT:
        _BUILT[key] = _build(**kw)
    return _BUILT[key]


def _make_in_maps(prep, ns=NS, ncores=NCORES):
    in_maps = []
    for c in range(ncores):
        sl = slice(c * ns, (c + 1) * ns)
        m = {k: prep[k] for k in CONST_KEYS}
        m["mel4"] = prep["_mel"][sl]
        m["evrows"] = prep["_evrows"][sl]
        m["tposv"] = prep["_tposv"][sl]
        m["keepv"] = prep["_keepv"][sl]
        m["starbias"] = prep["_starbias"][sl]
        in_maps.append(m)
    return in_maps


def kernel(**inputs):
    from concourse.bass_utils import run_bass_kernel_spmd

    prep = _host_prep(inputs)
    nc = _get_nc("full", nl_run=NLAYERS, ns_run=NS, debug=False,
                 sc1=prep["sc1"], c2s=prep["c2s"])
    res = run_bass_kernel_spmd(nc, _make_in_maps(prep), list(range(NCORES)))
    y = np.concatenate([res.results[c]["y"].reshape(-1) for c in range(NCORES)])
    return y.astype(np.float32)
